# revision 1
# baseline (speedup 1.0000x reference)
"""2-layer GATv2 (PyG GATv2Conv semantics) on 8 Trainium2 NeuronCores.

Strategy:
  - Nodes are sharded across 8 cores (balanced by in-degree), renumbered so
    each core owns NPC contiguous rows of a global table [8*NPC, *].
  - Layer projections (x @ W) are computed on the owning core; the source-side
    projections (xl) are AllGathered into a full table in each core's HBM.
  - Edges are sharded by destination.  Per destination-node-tile (128 nodes),
    incoming edges are packed into edge tiles of 128; source features are
    fetched with `dma_gather` (int16 indices -> the table is processed as two
    halves: sources owned by cores 0-3 are "lo", cores 4-7 are "hi"; edge
    tiles are built half-pure with a globally uniform K_lo/K_hi split so the
    SPMD program is identical on every core).
  - Per edge tile: one-hot matrices (DVE compare + PE transpose) implement the
    dst->edge broadcast (xr expand) and the edge->dst scatter-add (segment
    softmax numerator/denominator) as TensorE matmuls accumulated in PSUM.
  - Softmax skips the max-subtraction (scores are O(1); exp is safe) which is
    mathematically identity up to fp rounding.
  - Layer-2 projections of elu(h1) are computed per node tile; xl2 is
    AllGathered (rows padded to 64 floats for the 256-byte dma_gather
    granularity) and the same edge structure/indices are reused.
  - log_softmax over the 16 output channels per node on DVE/ACT.

The full (unsharded) inputs come in; full outputs go back out.
"""

import sys
import time

if "/opt/trn_rl_repo" not in sys.path:
    sys.path.insert(0, "/opt/trn_rl_repo")

import numpy as np
import ml_dtypes

NC = 8          # cores
P = 128         # partitions
NEG_SLOPE = 0.2

_plan_cache = {}


# --------------------------------------------------------------------------
# host-side graph preprocessing
# --------------------------------------------------------------------------

def _snake(order, nbins):
    """Assign items (given in priority order) to bins in snake order.
    Returns bin id per item position."""
    n = len(order)
    ids = np.arange(n)
    round_ = ids // nbins
    pos = ids % nbins
    b = np.where(round_ % 2 == 0, pos, nbins - 1 - pos)
    out = np.empty(n, np.int64)
    out[:] = b
    return out


def _preprocess(N, E, edge_index):
    NPC = ((N + NC - 1) // NC + P - 1) // P * P    # padded nodes per core
    NT = NPC // P
    TBL = NC * NPC
    assert TBL // 2 < 32768, "table half must fit int16 row indices"

    src = np.concatenate([edge_index[0].astype(np.int64), np.arange(N)])
    dst = np.concatenate([edge_index[1].astype(np.int64), np.arange(N)])
    deg = np.bincount(dst, minlength=N)

    # --- core assignment: snake over degree-sorted nodes
    order = np.argsort(-deg, kind="stable")
    core_of = np.empty(N, np.int64)
    core_of[order] = _snake(order, NC)

    # --- per-core local tile packing (balance lo/hi in-edge counts per tile)
    lo_src = core_of[src] < NC // 2                # which table half each edge reads
    deg_lo = np.bincount(dst[lo_src], minlength=N)
    deg_hi = deg - deg_lo

    local_of = np.empty(N, np.int64)               # local id within core
    tiles_nodes = {}                               # (core, nt) -> list of global node ids
    for c in range(NC):
        nodes = np.where(core_of == c)[0]
        nodes = nodes[np.argsort(-(deg[nodes]), kind="stable")]
        tile_ids = _snake(np.arange(len(nodes)), NT)
        slot_within = np.zeros(NT, np.int64)
        for v, tl in zip(nodes, tile_ids):
            local_of[v] = tl * P + slot_within[tl]
            slot_within[tl] += 1
        for tl in range(NT):
            tiles_nodes[(c, tl)] = nodes[tile_ids == tl]

    r_of = core_of * NPC + local_of                # global table row of each node

    # --- per (core, tile) edge lists split by half
    e_core = core_of[dst]
    e_tile = local_of[dst] // P
    e_slot = local_of[dst] % P
    lists_lo = {}
    lists_hi = {}
    K_lo = K_hi = 1
    for c in range(NC):
        m_c = e_core == c
        for tl in range(NT):
            m = m_c & (e_tile == tl)
            ml = m & lo_src
            mh = m & ~lo_src
            # (table_row_of_src, dst_slot)
            lists_lo[(c, tl)] = (r_of[src[ml]], e_slot[ml])
            lists_hi[(c, tl)] = (r_of[src[mh]] - TBL // 2, e_slot[mh])
            # fake self-edges for empty (padding) node slots so denominators
            # stay nonzero (outputs of pad slots are discarded by the host)
            npad = P - len(tiles_nodes[(c, tl)])
            if npad:
                pads = np.arange(P - npad, P)
                a, b = lists_lo[(c, tl)]
                lists_lo[(c, tl)] = (np.concatenate([a, np.zeros(npad, np.int64)]),
                                     np.concatenate([b, pads]))
            K_lo = max(K_lo, (len(lists_lo[(c, tl)][0]) + P - 1) // P)
            K_hi = max(K_hi, (len(lists_hi[(c, tl)][0]) + P - 1) // P)

    T = K_lo + K_hi

    # --- per-core arrays
    def pack_idx(flat):
        n = len(flat)
        s = (n + 15) // 16
        arr = np.zeros(s * 16, np.int16)
        arr[:n] = flat
        block = arr.reshape(s, 16).T
        return np.tile(block, (8, 1))

    gidx = np.zeros((NC, P, NT * T * 8), np.int16)
    drel = np.full((NC, P, NT * T), -1.0, np.float32)
    for c in range(NC):
        for tl in range(NT):
            for half, (K, lst) in enumerate(
                    [(K_lo, lists_lo[(c, tl)]), (K_hi, lists_hi[(c, tl)])]):
                rows, slots = lst
                n = len(rows)
                flat = np.zeros(K * P, np.int64)
                flat[:n] = rows
                off = (tl * T + half * K_lo) * 8 if half else tl * T * 8
                gidx[c, :, off:off + K * 8] = pack_idx(flat)
                tcol0 = tl * T + (K_lo if half else 0)
                dr = np.full(K * P, -1.0, np.float32)
                dr[:n] = slots
                drel[c, :, tcol0:tcol0 + K] = dr.reshape(K, P).T

    node_order = np.full((NC, NPC), -1, np.int64)  # local row -> global node id
    for c in range(NC):
        nodes = np.where(core_of == c)[0]
        node_order[c, local_of[nodes]] = nodes

    return dict(NPC=NPC, NT=NT, TBL=TBL, K_lo=K_lo, K_hi=K_hi, T=T,
                gidx=gidx, drel=drel, node_order=node_order,
                core_of=core_of, local_of=local_of)


# --------------------------------------------------------------------------
# bass program
# --------------------------------------------------------------------------

def _build_program(dims, post_passes=True):
    import os
    PHASES = int(os.environ.get("GAT_PHASES", "3"))
    import concourse.bass as bass
    import concourse.mybir as mybir
    import concourse.tile as tile
    from concourse import library_config
    from concourse.bass import _add_dep_helper
    import bass_rust as _br

    fp32 = mybir.dt.float32
    bf = mybir.dt.bfloat16 if os.environ.get("GAT_DTYPE", "f32") == "bf16" else mybir.dt.float32
    i16 = mybir.dt.int16
    AX = mybir.AxisListType
    OP = mybir.AluOpType
    AF = mybir.ActivationFunctionType

    DIN = dims["DIN"]; HC = dims["HC"]; H = dims["H"]; CH = dims["CH"]
    CO = dims["CO"]
    NPC = dims["NPC"]; NT = dims["NT"]; TBL = dims["TBL"]
    K_lo = dims["K_lo"]; K_hi = dims["K_hi"]; T = dims["T"]
    KD = DIN // P
    KH = HC // P
    CO_PAD = 128 if bf != fp32 else 64
    HALF = TBL // 2

    nc = bass.Bass(num_devices=NC)

    xk = nc.dram_tensor("xk", [NPC, DIN], bf, kind="ExternalInput")
    w1l = nc.dram_tensor("w1l", [DIN, HC], bf, kind="ExternalInput")
    w1r = nc.dram_tensor("w1r", [DIN, HC], bf, kind="ExternalInput")
    w2l = nc.dram_tensor("w2l", [HC, CO], bf, kind="ExternalInput")
    w2r = nc.dram_tensor("w2r", [HC, CO], bf, kind="ExternalInput")
    CCOLS = P + P + HC + HC + CO + CO + 1
    consts = nc.dram_tensor("consts", [P, CCOLS], bf, kind="ExternalInput")
    constf = nc.dram_tensor("constf", [P, 2], fp32, kind="ExternalInput")  # alpha | zero
    gidx_d = nc.dram_tensor("gidx", [P, NT * T * 8], i16, kind="ExternalInput")
    drel_d = nc.dram_tensor("drel", [P, NT * T], fp32, kind="ExternalInput")
    h2_out = nc.dram_tensor("h2o", [NPC, CO], fp32, kind="ExternalOutput")
    ls_out = nc.dram_tensor("lso", [NPC, CO], fp32, kind="ExternalOutput")

    with tile.TileContext(nc) as tc:
        with (
            tc.tile_pool(name="dram", bufs=1, space="DRAM") as dram,
            tc.tile_pool(name="cst", bufs=1) as cst,
        ):
            lib = nc.gpsimd.load_library(library_config.mlp)
            reg_klo = nc.gpsimd.to_reg(K_lo * P)
            reg_khi = nc.gpsimd.to_reg(K_hi * P)

            ctile = cst.tile([P, CCOLS], bf)
            nc.sync.dma_start(out=ctile[:], in_=consts[:])
            cftile = cst.tile([P, 2], fp32)
            nc.sync.dma_start(out=cftile[:], in_=constf[:])
            iota = ctile[:, 0:P]
            ident = ctile[:, P:2 * P]
            attB = ctile[:, 2 * P:2 * P + HC]
            b1B = ctile[:, 2 * P + HC:2 * P + 2 * HC]
            att2B = ctile[:, 2 * P + 2 * HC:2 * P + 2 * HC + CO]
            b2B = ctile[:, 2 * P + 2 * HC + CO:2 * P + 2 * HC + 2 * CO]
            alpha = cftile[:, 0:1]

            w1l_sb = cst.tile([P, KD, HC], bf)
            w1r_sb = cst.tile([P, KD, HC], bf)
            nc.sync.dma_start(out=w1l_sb[:], in_=w1l.rearrange("(k p) c -> p k c", p=P))
            nc.sync.dma_start(out=w1r_sb[:], in_=w1r.rearrange("(k p) c -> p k c", p=P))
            w2l_sb = cst.tile([P, KH, CO], bf)
            w2r_sb = cst.tile([P, KH, CO], bf)
            nc.sync.dma_start(out=w2l_sb[:], in_=w2l.rearrange("(k p) c -> p k c", p=P))
            nc.sync.dma_start(out=w2r_sb[:], in_=w2r.rearrange("(k p) c -> p k c", p=P))

            gidx_sb = cst.tile([P, NT * T * 8], i16)
            nc.sync.dma_start(out=gidx_sb[:], in_=gidx_d[:])
            drel_sb = cst.tile([P, NT * T], fp32)
            nc.sync.dma_start(out=drel_sb[:], in_=drel_d[:])

            xr1_all = cst.tile([P, NT, HC], bf)
            xr2_all = cst.tile([P, NT, CO], bf)
            h2_all = cst.tile([P, NT, CO], fp32)
            ls_all = cst.tile([P, NT, CO], fp32)
            nc.vector.memset(h2_all[:], 0.0)
            nc.vector.memset(ls_all[:], 0.0)
            nc.vector.memset(xr2_all[:], 0.0)

            ag1_in = dram.tile([NPC, HC], bf)
            tbl1 = dram.tile([TBL, HC], bf)
            ag2_in = dram.tile([NPC, CO_PAD], bf)
            tbl2 = dram.tile([TBL, CO_PAD], bf)

            # ============ phase A: layer-1 projections ============
            with (tc.tile_pool(name="sbA", bufs=2) as sb,
                  tc.tile_pool(name="psA", bufs=2, space="PSUM") as ps):
                for nt in range(NT):
                    xt = sb.tile([P, DIN], bf, tag="xt")
                    nc.sync.dma_start(out=xt[:], in_=xk[nt * P:(nt + 1) * P, :])
                    xl_ps = ps.tile([P, HC], fp32, tag="mm1", space="PSUM")
                    xr_ps = ps.tile([P, HC], fp32, tag="mm2", space="PSUM")
                    KD2 = (KD + 1) // 2
                    for half in range(2):
                        ks = list(range(half * KD2, min((half + 1) * KD2, KD)))
                        if not ks:
                            continue
                        xT_ps = ps.tile([P, KD2, P], fp32, tag="tp", space="PSUM")
                        for j, k in enumerate(ks):
                            nc.tensor.matmul(
                                out=xT_ps[:, j, :], lhsT=xt[:, k * P:(k + 1) * P],
                                rhs=ident, start=True, stop=True)
                        xT_sb = sb.tile([P, KD2, P], bf, tag="xTs")
                        nc.vector.tensor_copy(out=xT_sb[:], in_=xT_ps[:])
                        for j, k in enumerate(ks):
                            nc.tensor.matmul(out=xl_ps[:], lhsT=xT_sb[:, j, :],
                                             rhs=w1l_sb[:, k, :],
                                             start=(k == 0), stop=(k == KD - 1))
                        for j, k in enumerate(ks):
                            nc.tensor.matmul(out=xr_ps[:], lhsT=xT_sb[:, j, :],
                                             rhs=w1r_sb[:, k, :],
                                             start=(k == 0), stop=(k == KD - 1))
                    xl_sb = sb.tile([P, HC], bf, tag="xls")
                    nc.vector.tensor_copy(out=xl_sb[:], in_=xl_ps[:])
                    nc.vector.tensor_copy(out=xr1_all[:, nt, :], in_=xr_ps[:])
                    nc.sync.dma_start(out=ag1_in[nt * P:(nt + 1) * P, :], in_=xl_sb[:])

            if PHASES >= 2:
                nc.gpsimd.collective_compute(
                    "AllGather", mybir.AluOpType.bypass,
                    replica_groups=[list(range(NC))],
                    ins=[ag1_in[:].opt()], outs=[tbl1[:].opt()],
                )

                # ============ phase B: layer-1 edges ============
                with (tc.tile_pool(name="sbB", bufs=2) as sb,
                      tc.tile_pool(name="psB", bufs=2, space="PSUM") as ps):
                    for nt in range(NT):
                        glo = sb.tile([P, K_lo, HC], bf, tag="glo")
                        ghi = sb.tile([P, K_hi, HC], bf, tag="ghi")
                        off = nt * T * 8
                        g1 = nc.gpsimd.dma_gather(
                            glo[:], tbl1[0:HALF, :], gidx_sb[:, off:off + K_lo * 8],
                            K_lo * P, reg_klo, HC)
                        g2 = nc.gpsimd.dma_gather(
                            ghi[:], tbl1[HALF:TBL, :],
                            gidx_sb[:, off + K_lo * 8:off + T * 8],
                            K_hi * P, reg_khi, HC)
                        _add_dep_helper(g1.ins, lib.ins, sync=False, reason="lib")
                        _add_dep_helper(g2.ins, lib.ins, sync=False, reason="lib")

                        acc = ps.tile([P, HC + H], fp32, tag="acc", space="PSUM")
                        for t in range(T):
                            xl_t = glo[:, t, :] if t < K_lo else ghi[:, t - K_lo, :]
                            dcol = drel_sb[:, nt * T + t:nt * T + t + 1]
                            oh_en = sb.tile([P, P], bf, tag="ohe")
                            nc.vector.tensor_scalar(
                                out=oh_en[:], in0=iota, scalar1=dcol, scalar2=None,
                                op0=OP.is_equal)
                            ne_ps = ps.tile([P, P], fp32, tag="tp", space="PSUM")
                            nc.tensor.matmul(out=ne_ps[:], lhsT=oh_en[:], rhs=ident,
                                             start=True, stop=True)
                            oh_ne = sb.tile([P, P], bf, tag="ohn")
                            nc.scalar.copy(out=oh_ne[:], in_=ne_ps[:])

                            z_ps = ps.tile([P, HC], fp32, tag="mm1", space="PSUM")
                            nc.tensor.matmul(out=z_ps[:], lhsT=oh_ne[:],
                                             rhs=xr1_all[:, nt, :], start=True, stop=False)
                            nc.tensor.matmul(out=z_ps[:], lhsT=ident, rhs=xl_t,
                                             start=False, stop=True)
                            t_sb = sb.tile([P, HC], bf, tag="t")
                            nc.scalar.activation(out=t_sb[:], in_=z_ps[:], func=AF.Prelu,
                                                 alpha=alpha)
                            ta = sb.tile([P, HC], bf, tag="ta")
                            nc.vector.tensor_tensor(out=ta[:], in0=t_sb[:], in1=attB,
                                                    op=OP.mult)
                            sc = sb.tile([P, H], fp32, tag="sc")
                            nc.vector.tensor_reduce(
                                out=sc[:], in_=ta[:].rearrange("p (h c) -> p h c", h=H),
                                axis=AX.X, op=OP.add)
                            ex = sb.tile([P, H], bf, tag="ex")
                            nc.scalar.activation(out=ex[:], in_=sc[:], func=AF.Exp)
                            msg = sb.tile([P, HC + H], bf, tag="msg")
                            nc.vector.tensor_tensor(
                                out=msg[:, 0:HC].rearrange("p (h c) -> p h c", h=H),
                                in0=xl_t.rearrange("p (h c) -> p h c", h=H),
                                in1=ex[:, :, None].to_broadcast([P, H, CH]),
                                op=OP.mult)
                            nc.vector.tensor_copy(out=msg[:, HC:HC + H], in_=ex[:])
                            nc.tensor.matmul(out=acc[:], lhsT=oh_en[:], rhs=msg[:],
                                             start=(t == 0), stop=(t == T - 1))

                        rec = sb.tile([P, H], fp32, tag="rec")
                        nc.vector.reciprocal(out=rec[:], in_=acc[:, HC:HC + H])
                        h1 = sb.tile([P, HC], fp32, tag="h1")
                        nc.vector.tensor_tensor(
                            out=h1[:].rearrange("p (h c) -> p h c", h=H),
                            in0=acc[:, 0:HC].rearrange("p (h c) -> p h c", h=H),
                            in1=rec[:, :, None].to_broadcast([P, H, CH]),
                            op=OP.mult)
                        if dims["add_b1"]:
                            nc.vector.tensor_tensor(out=h1[:], in0=h1[:], in1=b1B, op=OP.add)
                        eh = sb.tile([P, HC], fp32, tag="eh")
                        nc.scalar.activation(out=eh[:], in_=h1[:], func=AF.Exp)
                        em = sb.tile([P, HC], fp32, tag="em")
                        nc.vector.tensor_scalar(
                            out=em[:], in0=eh[:], scalar1=1.0, scalar2=0.0,
                            op0=OP.subtract, op1=OP.min)
                        elu = sb.tile([P, HC], bf, tag="elu")
                        nc.vector.tensor_scalar(out=elu[:], in0=h1[:], scalar1=0.0,
                                                scalar2=None, op0=OP.max)
                        nc.vector.tensor_tensor(out=elu[:], in0=elu[:], in1=em[:], op=OP.add)

                        hT_ps = ps.tile([P, KH, P], fp32, tag="tp", space="PSUM")
                        for k in range(KH):
                            nc.tensor.matmul(out=hT_ps[:, k, :],
                                             lhsT=elu[:, k * P:(k + 1) * P], rhs=ident,
                                             start=True, stop=True)
                        hT_sb = sb.tile([P, KH, P], bf, tag="hTs")
                        nc.vector.tensor_copy(out=hT_sb[:], in_=hT_ps[:])
                        xl2_ps = ps.tile([P, CO], fp32, tag="mm1", space="PSUM")
                        xr2_ps = ps.tile([P, CO], fp32, tag="mm2", space="PSUM")
                        for k in range(KH):
                            nc.tensor.matmul(out=xl2_ps[:], lhsT=hT_sb[:, k, :],
                                             rhs=w2l_sb[:, k, :],
                                             start=(k == 0), stop=(k == KH - 1))
                        for k in range(KH):
                            nc.tensor.matmul(out=xr2_ps[:], lhsT=hT_sb[:, k, :],
                                             rhs=w2r_sb[:, k, :],
                                             start=(k == 0), stop=(k == KH - 1))
                        xl2_sb = sb.tile([P, CO_PAD], bf, tag="xl2s")
                        nc.vector.memset(xl2_sb[:, CO:CO_PAD], 0.0)
                        nc.vector.tensor_copy(out=xl2_sb[:, 0:CO], in_=xl2_ps[:])
                        nc.vector.tensor_copy(out=xr2_all[:, nt, :], in_=xr2_ps[:])
                        nc.sync.dma_start(out=ag2_in[nt * P:(nt + 1) * P, :], in_=xl2_sb[:])

            if PHASES >= 3:
                nc.gpsimd.collective_compute(
                    "AllGather", mybir.AluOpType.bypass,
                    replica_groups=[list(range(NC))],
                    ins=[ag2_in[:].opt()], outs=[tbl2[:].opt()],
                )

                # ============ phase C: layer-2 edges ============
                with (tc.tile_pool(name="sbC", bufs=2) as sb,
                      tc.tile_pool(name="psC", bufs=2, space="PSUM") as ps):
                    for nt in range(NT):
                        g2lo = sb.tile([P, K_lo, CO_PAD], bf, tag="g2lo")
                        g2hi = sb.tile([P, K_hi, CO_PAD], bf, tag="g2hi")
                        off = nt * T * 8
                        g1 = nc.gpsimd.dma_gather(
                            g2lo[:], tbl2[0:HALF, :], gidx_sb[:, off:off + K_lo * 8],
                            K_lo * P, reg_klo, CO_PAD)
                        g2 = nc.gpsimd.dma_gather(
                            g2hi[:], tbl2[HALF:TBL, :],
                            gidx_sb[:, off + K_lo * 8:off + T * 8],
                            K_hi * P, reg_khi, CO_PAD)
                        _add_dep_helper(g1.ins, lib.ins, sync=False, reason="lib")
                        _add_dep_helper(g2.ins, lib.ins, sync=False, reason="lib")

                        acc2 = ps.tile([P, CO + 1], fp32, tag="acc", space="PSUM")
                        for t in range(T):
                            xl2_t = (g2lo[:, t, 0:CO] if t < K_lo
                                     else g2hi[:, t - K_lo, 0:CO])
                            dcol = drel_sb[:, nt * T + t:nt * T + t + 1]
                            oh_en = sb.tile([P, P], bf, tag="ohe")
                            nc.vector.tensor_scalar(
                                out=oh_en[:], in0=iota, scalar1=dcol, scalar2=None,
                                op0=OP.is_equal)
                            ne_ps = ps.tile([P, P], fp32, tag="tp", space="PSUM")
                            nc.tensor.matmul(out=ne_ps[:], lhsT=oh_en[:], rhs=ident,
                                             start=True, stop=True)
                            oh_ne = sb.tile([P, P], bf, tag="ohn")
                            nc.scalar.copy(out=oh_ne[:], in_=ne_ps[:])

                            z2_ps = ps.tile([P, CO], fp32, tag="mm1", space="PSUM")
                            nc.tensor.matmul(out=z2_ps[:], lhsT=oh_ne[:],
                                             rhs=xr2_all[:, nt, :], start=True, stop=False)
                            nc.tensor.matmul(out=z2_ps[:], lhsT=ident, rhs=xl2_t,
                                             start=False, stop=True)
                            t2 = sb.tile([P, CO], bf, tag="t2")
                            nc.scalar.activation(out=t2[:], in_=z2_ps[:], func=AF.Prelu,
                                                 alpha=alpha)
                            t2a = sb.tile([P, CO], bf, tag="t2a")
                            sc2 = sb.tile([P, 1], fp32, tag="sc2")
                            nc.vector.tensor_tensor(out=t2a[:], in0=t2[:], in1=att2B,
                                                    op=OP.mult)
                            nc.vector.tensor_reduce(out=sc2[:], in_=t2a[:], axis=AX.X,
                                                    op=OP.add)
                            ex2 = sb.tile([P, 1], fp32, tag="ex2")
                            nc.scalar.activation(out=ex2[:], in_=sc2[:], func=AF.Exp)
                            msg2 = sb.tile([P, CO + 1], bf, tag="msg2")
                            nc.vector.tensor_scalar(
                                out=msg2[:, 0:CO], in0=xl2_t, scalar1=ex2[:, 0:1],
                                scalar2=None, op0=OP.mult)
                            nc.vector.tensor_copy(out=msg2[:, CO:CO + 1], in_=ex2[:])
                            nc.tensor.matmul(out=acc2[:], lhsT=oh_en[:], rhs=msg2[:],
                                             start=(t == 0), stop=(t == T - 1))

                        rec2 = sb.tile([P, 1], fp32, tag="rec2")
                        nc.vector.reciprocal(out=rec2[:], in_=acc2[:, CO:CO + 1])
                        h2 = sb.tile([P, CO], fp32, tag="h2")
                        nc.vector.tensor_scalar(out=h2[:], in0=acc2[:, 0:CO],
                                                scalar1=rec2[:, 0:1], scalar2=None,
                                                op0=OP.mult)
                        if dims["add_b2"]:
                            nc.vector.tensor_tensor(out=h2[:], in0=h2[:], in1=b2B, op=OP.add)
                        nc.vector.tensor_copy(out=h2_all[:, nt, :], in_=h2[:])
                        nm = sb.tile([P, 1], fp32, tag="nm")
                        nc.vector.tensor_reduce(out=nm[:], in_=h2[:], axis=AX.X,
                                                op=OP.max, negate=True)
                        esc = sb.tile([P, CO], fp32, tag="esc")
                        ssum = sb.tile([P, 1], fp32, tag="ssum")
                        nc.scalar.activation(out=esc[:], in_=h2[:], func=AF.Exp,
                                             bias=nm[:, 0:1], accum_out=ssum[:, 0:1])
                        lns = sb.tile([P, 1], fp32, tag="lns")
                        nc.scalar.activation(out=lns[:], in_=ssum[:], func=AF.Ln)
                        nc.vector.tensor_scalar(
                            out=ls_all[:, nt, :], in0=h2[:], scalar1=nm[:, 0:1],
                            scalar2=lns[:, 0:1], op0=OP.add, op1=OP.subtract)

            nc.sync.dma_start(out=h2_out.rearrange("(a p) d -> p a d", p=P),
                              in_=h2_all[:])
            nc.sync.dma_start(out=ls_out.rearrange("(a p) d -> p a d", p=P),
                              in_=ls_all[:])

    if post_passes:
        _br.generate_event_semaphores(nc)
        _br.codegen_inst_isa_subclasses(nc)
    return nc


# --------------------------------------------------------------------------
# entry point
# --------------------------------------------------------------------------

def kernel(x, edge_index, W1l, W1r, att1, b1, W2l, W2r, att2, b2):
    x = np.asarray(x, np.float32)
    edge_index = np.asarray(edge_index)
    W1l = np.asarray(W1l, np.float32); W1r = np.asarray(W1r, np.float32)
    att1 = np.asarray(att1, np.float32); b1 = np.asarray(b1, np.float32)
    W2l = np.asarray(W2l, np.float32); W2r = np.asarray(W2r, np.float32)
    att2 = np.asarray(att2, np.float32); b2 = np.asarray(b2, np.float32)

    N, DIN = x.shape
    E = edge_index.shape[1]
    H, CH = att1.shape
    HC = W1l.shape[1]
    CO = W2l.shape[1]

    key = (N, E, DIN, H, CH, HC, CO,
           int(np.abs(b1).max() > 0), int(np.abs(b2).max() > 0),
           hash(edge_index.tobytes()))
    if key in _plan_cache:
        pp, nc, dims = _plan_cache[key]
    else:
        pp = _preprocess(N, E, edge_index)
        dims = dict(DIN=DIN, HC=HC, H=H, CH=CH, CO=CO,
                    NPC=pp["NPC"], NT=pp["NT"], TBL=pp["TBL"],
                    K_lo=pp["K_lo"], K_hi=pp["K_hi"], T=pp["T"],
                    add_b1=bool(np.abs(b1).max() > 0),
                    add_b2=bool(np.abs(b2).max() > 0))
        nc = _build_program(dims)
        _plan_cache[key] = (pp, nc, dims)

    NPC = pp["NPC"]

    # consts blob
    iota = np.broadcast_to(np.arange(P, dtype=np.float32)[None, :], (P, P))
    ident = np.eye(P, dtype=np.float32)
    attB = np.broadcast_to(att1.reshape(1, HC), (P, HC))
    b1B = np.broadcast_to(b1.reshape(1, HC), (P, HC))
    att2B = np.broadcast_to(att2.reshape(1, CO), (P, CO))
    b2B = np.broadcast_to(b2.reshape(1, CO), (P, CO))
    alpha = np.full((P, 1), NEG_SLOPE, np.float32)
    import os
    bfdt = ml_dtypes.bfloat16 if os.environ.get("GAT_DTYPE", "f32") == "bf16" else np.float32
    consts = np.concatenate([iota, ident, attB, b1B, att2B, b2B, alpha],
                            axis=1).astype(bfdt)
    constf = np.concatenate([alpha, np.zeros((P, 1), np.float32)], axis=1).astype(np.float32)

    in_maps = []
    for c in range(NC):
        xkc = np.zeros((NPC, DIN), np.float32)
        sel = pp["node_order"][c]
        real = sel >= 0
        xkc[real] = x[sel[real]]
        in_maps.append(dict(
            xk=xkc.astype(bfdt), w1l=W1l.astype(bfdt), w1r=W1r.astype(bfdt),
            w2l=W2l.astype(bfdt), w2r=W2r.astype(bfdt), consts=consts,
            constf=constf,
            gidx=np.ascontiguousarray(pp["gidx"][c]),
            drel=np.ascontiguousarray(pp["drel"][c]),
        ))

    from concourse.bass_utils import run_bass_kernel_spmd
    res = run_bass_kernel_spmd(nc, in_maps, core_ids=list(range(NC)))

    h = np.empty((N, CO), np.float32)
    ls = np.empty((N, CO), np.float32)
    r_core = pp["core_of"]
    r_loc = pp["local_of"]
    for c in range(NC):
        m = r_core == c
        h[m] = res.results[c]["h2o"][r_loc[m]]
        ls[m] = res.results[c]["lso"][r_loc[m]]
    return h, ls



# revision 17
# speedup vs baseline: 1.5046x; 1.5046x over previous
"""2-layer GATv2 (PyG GATv2Conv semantics) on 8 Trainium2 NeuronCores.

Strategy (v2):
  - Nodes sharded across 8 cores; per-core greedy 2D packing balances each
    destination tile's lo/hi in-edge counts (lo = src owned by cores 0-3).
  - x is shipped pre-transposed; layer-1 projections are 6 accumulating
    matmuls per node tile with a combined [W1l|W1r] moving operand.
  - xl tables AllGathered HBM->HBM in 4 row-chunks overlapped with compute.
  - Per destination tile, incoming-edge source rows are fetched with
    dma_gather (int16 idx; two table halves).  The dst->edge one-hot
    (oh_ne, [dst, edge]) is precomputed on host and streamed from HBM;
    the edge->dst one-hot (oh_en) is one batched DVE is_equal per half.
  - Per half (K edge tiles): K z-matmuls (xr broadcast), one batched DVE
    add (z+xl), 2-op leaky, att mult, reduce, exp, msg mult -- all batched
    over the half's K*128 edges -- then K scatter matmuls accumulate
    numerator+denominator in PSUM.
  - Softmax skips max-subtraction (scores O(1)).
  - Layer 2 (heads=1, 16 ch) repeats the edge structure on a 256B-row table.
  - log_softmax on ACT/DVE per node tile.

kernel(**inputs) takes FULL inputs, returns FULL outputs.
"""

import os
import sys

if "/opt/trn_rl_repo" not in sys.path:
    sys.path.insert(0, "/opt/trn_rl_repo")

import numpy as np
import ml_dtypes

NC = 8          # cores
P = 128         # partitions
NEG_SLOPE = 0.2
NGRP = 4        # AllGather chunks

_plan_cache = {}


# --------------------------------------------------------------------------
# host-side graph preprocessing
# --------------------------------------------------------------------------

def _snake(order, nbins):
    n = len(order)
    ids = np.arange(n)
    round_ = ids // nbins
    pos = ids % nbins
    b = np.where(round_ % 2 == 0, pos, nbins - 1 - pos)
    out = np.empty(n, np.int64)
    out[:] = b
    return out


def _preprocess(N, E, edge_index):
    NPC = ((N + NC - 1) // NC + P - 1) // P * P    # padded nodes per core
    NT = NPC // P
    TBL = NC * NPC
    assert TBL // 2 < 32768, "table half must fit int16 row indices"

    src = np.concatenate([edge_index[0].astype(np.int64), np.arange(N)])
    dst = np.concatenate([edge_index[1].astype(np.int64), np.arange(N)])
    deg = np.bincount(dst, minlength=N)

    # --- group assignment (fixes each edge's table half), then cores within
    # each group balanced by OWN-group in-degree (the heavy, self-loop half)
    order = np.argsort(-deg, kind="stable")
    grp_of = np.empty(N, np.int64)
    grp_of[order] = _snake(order, 2)
    e_own = grp_of[src] == grp_of[dst]
    deg_own = np.bincount(dst[e_own], minlength=N)
    core_of = np.empty(N, np.int64)
    half_nc = NC // 2
    for g in range(2):
        nodes_g = np.where(grp_of == g)[0]
        og = nodes_g[np.argsort(-deg_own[nodes_g], kind="stable")]
        core_of[og] = g * half_nc + _snake(og, half_nc)

    lo_src = core_of[src] < NC // 2                # which table half each edge reads
    deg_lo = np.bincount(dst[lo_src], minlength=N)
    deg_hi = deg - deg_lo

    # --- per-core greedy 2D packing: balance (lo, hi) in-edge sums per tile
    slot_of = np.empty(N, np.int64)
    tile_of = np.empty(N, np.int64)
    nlo = np.zeros((NC, NT), np.int64)   # per-tile lo in-edges (incl. pad fakes)
    nhi = np.zeros((NC, NT), np.int64)
    cnt_ct = np.zeros((NC, NT), np.int64)
    for c in range(NC):
        heavy_is_lo = c < NC // 2        # own-group half carries the self-loops
        dh = deg_lo if heavy_is_lo else deg_hi
        dl = deg_hi if heavy_is_lo else deg_lo
        nodes = np.where(core_of == c)[0]
        nodes = nodes[np.argsort(-(dh[nodes] * 64 + dl[nodes]), kind="stable")]
        hv = np.zeros(NT, np.int64)
        lt = np.zeros(NT, np.int64)
        cnt = np.zeros(NT, np.int64)
        for v in nodes:
            cost = (hv + dh[v]).astype(np.float64) \
                + 0.02 * (lt + dl[v]) + 1e-4 * cnt \
                + 1e6 * np.maximum(lt + dl[v] - 5 * P, 0)
            cost[cnt >= P] = 1e18
            t = int(np.argmin(cost))
            tile_of[v] = t
            slot_of[v] = cnt[t]
            cnt[t] += 1
            hv[t] += dh[v]
            lt[t] += dl[v]
        lt += P - cnt                    # pad slots: one fake edge, light half
        if heavy_is_lo:
            nlo[c], nhi[c] = hv, lt
        else:
            nlo[c], nhi[c] = lt, hv
        cnt_ct[c] = cnt

    # --- per-core rank permutation so heavy tiles align across cores
    kl = (nlo + P - 1) // P
    kh = (nhi + P - 1) // P
    perm = np.empty((NC, NT), np.int64)     # rank -> old tile
    for c in range(NC):
        key = (kl[c] + kh[c]) + 1e-3 * kl[c] + 1e-9 * (nlo[c] + nhi[c])
        perm[c] = np.argsort(-key, kind="stable")

    Klo = [int(max(kl[c, perm[c, r]] for c in range(NC))) for r in range(NT)]
    Khi = [int(max(kh[c, perm[c, r]] for c in range(NC))) for r in range(NT)]
    T = [Klo[r] + Khi[r] for r in range(NT)]
    KM = max(max(Klo), max(Khi))

    rank_of = np.empty((NC, NT), np.int64)  # old tile -> rank
    for c in range(NC):
        rank_of[c, perm[c]] = np.arange(NT)
    local_of = rank_of[core_of, tile_of] * P + slot_of
    r_of = core_of * NPC + local_of         # final global table row of each node

    # --- per (core, old-tile) edge lists split by half (final r_of values)
    e_core = core_of[dst]
    e_tile = tile_of[dst]
    e_slot = slot_of[dst]
    lists_lo = {}
    lists_hi = {}
    for c in range(NC):
        m_c = e_core == c
        for tl in range(NT):
            m = m_c & (e_tile == tl)
            ml = m & lo_src
            mh = m & ~lo_src
            lists_lo[(c, tl)] = (r_of[src[ml]], e_slot[ml])
            lists_hi[(c, tl)] = (r_of[src[mh]] - TBL // 2, e_slot[mh])
            # fake self-edges for empty (padding) node slots so denominators
            # stay nonzero (their outputs are discarded by the host); they
            # live in the light half (matches the greedy's accounting)
            npad = P - cnt_ct[c, tl]
            if npad:
                pads = np.arange(P - npad, P)
                key = (c, tl)
                tgt = lists_hi if c < NC // 2 else lists_lo
                a, b = tgt[key]
                tgt[key] = (np.concatenate([a, np.zeros(npad, np.int64)]),
                            np.concatenate([b, pads]))

    # offsets
    od = np.concatenate([[0], np.cumsum(T)]).astype(np.int64)       # drel/ohne cols
    olo = np.concatenate([[0], np.cumsum(Klo)]).astype(np.int64)    # gidx_lo tiles
    ohi = np.concatenate([[0], np.cumsum(Khi)]).astype(np.int64)
    OD = int(od[-1]); OLO = int(olo[-1]); OHI = int(ohi[-1])

    def pack_idx(flat):
        n = len(flat)
        s = (n + 15) // 16
        arr = np.zeros(s * 16, np.int16)
        arr[:n] = flat
        block = arr.reshape(s, 16).T
        return np.tile(block, (8, 1))

    gidx_lo = np.zeros((NC, P, OLO * 8), np.int16)
    gidx_hi = np.zeros((NC, P, OHI * 8), np.int16)
    drel = np.full((NC, P, OD), -1.0, np.float32)
    for c in range(NC):
        for r in range(NT):
            tl = perm[c, r]
            for half, (K, off8, dcol0, lst) in enumerate([
                    (Klo[r], olo[r], od[r], lists_lo[(c, tl)]),
                    (Khi[r], ohi[r], od[r] + Klo[r], lists_hi[(c, tl)])]):
                rows, slots = lst
                # re-slot: dst slots were computed pre-permutation; slot within
                # tile is unchanged (rank remap keeps slot % P)
                n = len(rows)
                flat = np.zeros(K * P, np.int64)
                flat[:n] = rows
                g = gidx_hi if half else gidx_lo
                g[c, :, off8 * 8:(off8 + K) * 8] = pack_idx(flat)
                dr = np.full(K * P, -1.0, np.float32)
                dr[:n] = slots
                drel[c, :, dcol0:dcol0 + K] = dr.reshape(K, P).T

    node_order = np.full((NC, NPC), -1, np.int64)  # local row -> global node id
    for c in range(NC):
        nodes = np.where(core_of == c)[0]
        node_order[c, local_of[nodes]] = nodes

    # dst->edge one-hot, [dst(P), OD, edge(P)] per core
    ohne = (np.arange(P, dtype=np.float32)[None, :, None, None]
            == drel.transpose(0, 2, 1)[:, None, :, :])  # [NC, P, OD, P]
    ohne = ohne.astype(ml_dtypes.bfloat16).reshape(NC, P, OD * P)

    # AllGather groups (tile ranks)
    gsz = (NT + NGRP - 1) // NGRP
    groups = [(g * gsz, min((g + 1) * gsz, NT)) for g in range(NGRP)]
    groups = [(a, b) for a, b in groups if b > a]

    return dict(NPC=NPC, NT=NT, TBL=TBL, Klo=Klo, Khi=Khi, T=T, KM=KM,
                od=od.tolist(), olo=olo.tolist(), ohi=ohi.tolist(),
                OD=OD, OLO=OLO, OHI=OHI, groups=groups,
                gidx_lo=gidx_lo, gidx_hi=gidx_hi, drel=drel, ohne=ohne,
                node_order=node_order, core_of=core_of, local_of=local_of)


# --------------------------------------------------------------------------
# bass program
# --------------------------------------------------------------------------

def _build_program(dims, post_passes=True):
    PHASES = int(os.environ.get("GAT_PHASES", "3"))
    SHARED = os.environ.get("GAT_SHARED", "0") == "1"
    import concourse.bass as bass
    import concourse.mybir as mybir
    import concourse.tile as tile
    from concourse import library_config
    from concourse.bass import _add_dep_helper
    import bass_rust as _br

    fp32 = mybir.dt.float32
    bf = mybir.dt.bfloat16
    i16 = mybir.dt.int16
    AX = mybir.AxisListType
    OP = mybir.AluOpType
    AF = mybir.ActivationFunctionType

    DIN = dims["DIN"]; HC = dims["HC"]; H = dims["H"]; CH = dims["CH"]
    CO = dims["CO"]
    NPC = dims["NPC"]; NT = dims["NT"]; TBL = dims["TBL"]
    Klo = dims["Klo"]; Khi = dims["Khi"]; T = dims["T"]; KM = dims["KM"]
    od = dims["od"]; olo = dims["olo"]; ohi = dims["ohi"]
    OD = dims["OD"]; OLO = dims["OLO"]; OHI = dims["OHI"]
    groups = dims["groups"]
    KD = DIN // P
    KH = HC // P
    CO_PAD = 128
    HALF = TBL // 2
    addr_space = "Shared" if SHARED else "Local"

    nc = bass.Bass(num_devices=NC)

    xkT_d = nc.dram_tensor("xkT", [P, NT * KD * P], bf, kind="ExternalInput")
    w1_d = nc.dram_tensor("w1", [DIN, 2 * HC], bf, kind="ExternalInput")
    w2_d = nc.dram_tensor("w2", [HC, 2 * CO], bf, kind="ExternalInput")
    CCOLS = KM * P + P + KM * HC + HC + KM * CO + CO
    consts = nc.dram_tensor("consts", [P, CCOLS], bf, kind="ExternalInput")
    constf = nc.dram_tensor("constf", [P, 1], fp32, kind="ExternalInput")
    gidx_lo_d = nc.dram_tensor("gidx_lo", [P, OLO * 8], i16, kind="ExternalInput")
    gidx_hi_d = nc.dram_tensor("gidx_hi", [P, OHI * 8], i16, kind="ExternalInput")
    drel_d = nc.dram_tensor("drel", [P, OD], bf, kind="ExternalInput")
    ohne_d = nc.dram_tensor("ohne", [P, OD * P], bf, kind="ExternalInput")
    h2_out = nc.dram_tensor("h2o", [NPC, CO], fp32, kind="ExternalOutput")
    ls_out = nc.dram_tensor("lso", [NPC, CO], fp32, kind="ExternalOutput")

    with tile.TileContext(nc) as tc:
        with (
            tc.tile_pool(name="dram", bufs=1, space="DRAM") as dram,
            tc.tile_pool(name="cst", bufs=1) as cst,
        ):
            lib = nc.gpsimd.load_library(library_config.mlp)
            regs = {}
            for K in sorted({*Klo, *Khi}):
                regs[K] = nc.gpsimd.to_reg(K * P)

            ctile = cst.tile([P, CCOLS], bf)
            nc.sync.dma_start(out=ctile[:], in_=consts[:])
            cftile = cst.tile([P, 1], fp32)
            nc.sync.dma_start(out=cftile[:], in_=constf[:])
            o = 0
            iotaK = ctile[:, o:o + KM * P]; o += KM * P
            ident = ctile[:, o:o + P]; o += P
            attBK = ctile[:, o:o + KM * HC]; o += KM * HC
            b1B = ctile[:, o:o + HC]; o += HC
            att2BK = ctile[:, o:o + KM * CO]; o += KM * CO
            b2B = ctile[:, o:o + CO]; o += CO
            alpha = cftile[:, 0:1]

            w1_sb = cst.tile([P, KD, 2 * HC], bf)
            nc.sync.dma_start(out=w1_sb[:], in_=w1_d.rearrange("(k p) c -> p k c", p=P))
            w2_sb = cst.tile([P, KH, 2 * CO], bf)
            nc.sync.dma_start(out=w2_sb[:], in_=w2_d.rearrange("(k p) c -> p k c", p=P))

            gidx_lo_sb = cst.tile([P, OLO * 8], i16)
            nc.sync.dma_start(out=gidx_lo_sb[:], in_=gidx_lo_d[:])
            gidx_hi_sb = cst.tile([P, OHI * 8], i16)
            nc.sync.dma_start(out=gidx_hi_sb[:], in_=gidx_hi_d[:])
            drel_sb = cst.tile([P, OD], bf)
            nc.sync.dma_start(out=drel_sb[:], in_=drel_d[:])

            xr1_all = cst.tile([P, NT, HC], bf)
            xr2_all = cst.tile([P, NT, CO], bf)
            h2_all = cst.tile([P, NT, CO], fp32)
            ls_all = cst.tile([P, NT, CO], fp32)
            nc.vector.memset(h2_all[:], 0.0)
            nc.vector.memset(ls_all[:], 0.0)
            nc.vector.memset(xr2_all[:], 0.0)

            ag1_in = dram.tile([NPC, HC], bf)
            tbl1 = dram.tile([TBL, HC], bf)
            ag2_in = dram.tile([NPC, CO_PAD], bf)
            tbl2 = dram.tile([TBL, CO_PAD], bf)
            stg1 = {}
            stg2 = {}
            for gi, (g0, g1) in enumerate(groups):
                stg1[gi] = dram.tile([NC * (g1 - g0) * P, HC], bf,
                                     addr_space=addr_space, name=f"stg1_{gi}")
                stg2[gi] = dram.tile([NC * (g1 - g0) * P, CO_PAD], bf,
                                     addr_space=addr_space, name=f"stg2_{gi}")

            def ag_chunk(src, stage, dst, g0, g1):
                nc.gpsimd.collective_compute(
                    "AllGather", mybir.AluOpType.bypass,
                    replica_groups=[list(range(NC))],
                    ins=[src[g0 * P:g1 * P, :].opt()],
                    outs=[stage[:].opt()],
                )
                nc.sync.dma_start(
                    out=dst[:].rearrange("(c n) h -> c n h", c=NC)
                        [:, g0 * P:g1 * P, :],
                    in_=stage[:].rearrange("(c n) h -> c n h", c=NC))

            # ============ phase A: layer-1 projections ============
            with (tc.tile_pool(name="sbA", bufs=3) as sb,
                  tc.tile_pool(name="psA", bufs=2, space="PSUM") as ps):
                for g0, g1 in groups:
                    for nt in range(g0, g1):
                        xt = sb.tile([P, KD, P], bf, tag="xt")
                        nc.sync.dma_start(
                            out=xt[:],
                            in_=xkT_d[:, nt * KD * P:(nt + 1) * KD * P])
                        xlr_ps = ps.tile([P, 2 * HC], fp32, tag="mm", space="PSUM")
                        for k in range(KD):
                            nc.tensor.matmul(out=xlr_ps[:], lhsT=xt[:, k, :],
                                             rhs=w1_sb[:, k, :],
                                             start=(k == 0), stop=(k == KD - 1))
                        xl_sb = sb.tile([P, HC], bf, tag="xls")
                        nc.vector.tensor_copy(out=xl_sb[:], in_=xlr_ps[:, 0:HC])
                        nc.vector.tensor_copy(out=xr1_all[:, nt, :],
                                              in_=xlr_ps[:, HC:2 * HC])
                        nc.sync.dma_start(out=ag1_in[nt * P:(nt + 1) * P, :],
                                          in_=xl_sb[:])
                    if PHASES >= 2:
                        ag_chunk(ag1_in, stg1[groups.index((g0, g1))], tbl1, g0, g1)

            # ============ phase B: layer-1 edges ============
            if PHASES >= 2:
                with (tc.tile_pool(name="sbB", bufs=2) as sb,
                      tc.tile_pool(name="gbB", bufs=3) as gb,
                      tc.tile_pool(name="psB", bufs=2, space="PSUM") as ps):
                    for g0, g1 in groups:
                        for nt in range(g0, g1):
                            T_ = T[nt]; Klo_ = Klo[nt]; Khi_ = Khi[nt]
                            glo = gb.tile([P, KM, HC], bf, tag="glo")
                            ghi = gb.tile([P, KM, HC], bf, tag="ghi")
                            g1i = nc.gpsimd.dma_gather(
                                glo[:, 0:Klo_, :], tbl1[0:HALF, :],
                                gidx_lo_sb[:, olo[nt] * 8:(olo[nt] + Klo_) * 8],
                                Klo_ * P, regs[Klo_], HC)
                            g2i = nc.gpsimd.dma_gather(
                                ghi[:, 0:Khi_, :], tbl1[HALF:TBL, :],
                                gidx_hi_sb[:, ohi[nt] * 8:(ohi[nt] + Khi_) * 8],
                                Khi_ * P, regs[Khi_], HC)
                            _add_dep_helper(g1i.ins, lib.ins, sync=False, reason="lib")
                            _add_dep_helper(g2i.ins, lib.ins, sync=False, reason="lib")
                            ohne_t = gb.tile([P, (2 * KM) * P], bf, tag="ohne")
                            nc.sync.dma_start(
                                out=ohne_t[:, 0:T_ * P],
                                in_=ohne_d[:, od[nt] * P:(od[nt] + T_) * P])

                            acc = ps.tile([P, HC + H], fp32, tag="acc", space="PSUM",
                                          bufs=1)
                            msg = sb.tile([P, 2 * KM, HC + H], bf, tag="msg")
                            for t0, K, gx in [(0, Klo_, glo), (Klo_, Khi_, ghi)]:
                                z_ps = ps.tile([P, KM, HC], fp32, tag="z",
                                               space="PSUM")
                                for j in range(K):
                                    nc.tensor.matmul(
                                        out=z_ps[:, j, :],
                                        lhsT=ohne_t[:, (t0 + j) * P:(t0 + j + 1) * P],
                                        rhs=xr1_all[:, nt, :],
                                        start=True, stop=True)
                                v = sb.tile([P, KM, HC], bf, tag="v")
                                nc.vector.tensor_tensor(
                                    out=v[:, 0:K, :], in0=z_ps[:, 0:K, :],
                                    in1=gx[:, 0:K, :], op=OP.add)
                                va = sb.tile([P, KM, HC], bf, tag="va")
                                nc.vector.tensor_scalar(
                                    out=va[:, 0:K, :], in0=v[:, 0:K, :],
                                    scalar1=NEG_SLOPE, scalar2=None, op0=OP.mult)
                                tp = sb.tile([P, KM, HC], bf, tag="tp")
                                nc.vector.tensor_tensor(
                                    out=tp[:, 0:K, :], in0=v[:, 0:K, :],
                                    in1=va[:, 0:K, :], op=OP.max)
                                ta = sb.tile([P, KM, HC], bf, tag="ta")
                                nc.vector.tensor_tensor(
                                    out=ta[:, 0:K, :], in0=tp[:, 0:K, :],
                                    in1=attBK[:, 0:K * HC].rearrange(
                                        "p (k c) -> p k c", k=K), op=OP.mult)
                                sc = sb.tile([P, KM * H], fp32, tag="sc")
                                nc.vector.tensor_reduce(
                                    out=sc[:, 0:K * H],
                                    in_=ta[:, 0:K, :].rearrange(
                                        "p k (h c) -> p (k h) c", h=H),
                                    axis=AX.X, op=OP.add)
                                ex = sb.tile([P, KM * H], bf, tag="ex")
                                nc.scalar.activation(out=ex[:, 0:K * H],
                                                     in_=sc[:, 0:K * H], func=AF.Exp)
                                nc.vector.tensor_tensor(
                                    out=msg[:, t0:t0 + K, 0:HC].rearrange(
                                        "p k (h c) -> p k h c", h=H),
                                    in0=gx[:, 0:K, :].rearrange(
                                        "p k (h c) -> p k h c", h=H),
                                    in1=ex[:, 0:K * H].rearrange(
                                        "p (k h) -> p k h", k=K)[:, :, :, None]
                                        .to_broadcast([P, K, H, CH]),
                                    op=OP.mult)
                                nc.vector.tensor_copy(
                                    out=msg[:, t0:t0 + K, HC:HC + H],
                                    in_=ex[:, 0:K * H].rearrange(
                                        "p (k h) -> p k h", k=K))
                                ohe = sb.tile([P, KM, P], bf, tag="ohe")
                                nc.vector.tensor_tensor(
                                    out=ohe[:, 0:K, :],
                                    in0=iotaK[:, 0:K * P].rearrange(
                                        "p (k q) -> p k q", k=K),
                                    in1=drel_sb[:, od[nt] + t0:od[nt] + t0 + K]
                                        [:, :, None].to_broadcast([P, K, P]),
                                    op=OP.is_equal)
                                for j in range(K):
                                    nc.tensor.matmul(
                                        out=acc[:], lhsT=ohe[:, j, :],
                                        rhs=msg[:, t0 + j, :],
                                        start=(t0 + j == 0), stop=(t0 + j == T_ - 1))

                            rec = sb.tile([P, H], fp32, tag="rec")
                            nc.vector.reciprocal(out=rec[:], in_=acc[:, HC:HC + H])
                            h1 = sb.tile([P, HC], fp32, tag="h1")
                            nc.vector.tensor_tensor(
                                out=h1[:].rearrange("p (h c) -> p h c", h=H),
                                in0=acc[:, 0:HC].rearrange("p (h c) -> p h c", h=H),
                                in1=rec[:, :, None].to_broadcast([P, H, CH]),
                                op=OP.mult)
                            if dims["add_b1"]:
                                nc.vector.tensor_tensor(out=h1[:], in0=h1[:],
                                                        in1=b1B, op=OP.add)
                            eh = sb.tile([P, HC], fp32, tag="eh")
                            nc.scalar.activation(out=eh[:], in_=h1[:], func=AF.Exp)
                            em = sb.tile([P, HC], fp32, tag="em")
                            nc.vector.tensor_scalar(
                                out=em[:], in0=eh[:], scalar1=1.0, scalar2=0.0,
                                op0=OP.subtract, op1=OP.min)
                            elu = sb.tile([P, HC], bf, tag="elu")
                            nc.vector.tensor_scalar(out=elu[:], in0=h1[:],
                                                    scalar1=0.0, scalar2=None,
                                                    op0=OP.max)
                            nc.vector.tensor_tensor(out=elu[:], in0=elu[:],
                                                    in1=em[:], op=OP.add)

                            tail_ps = ps.tile([P, KH * P + 2 * CO], fp32,
                                              tag="tail", space="PSUM", bufs=1)
                            for k in range(KH):
                                nc.tensor.matmul(
                                    out=tail_ps[:, k * P:(k + 1) * P],
                                    lhsT=elu[:, k * P:(k + 1) * P],
                                    rhs=ident, start=True, stop=True)
                            hT_sb = sb.tile([P, KH, P], bf, tag="hTs")
                            nc.vector.tensor_copy(
                                out=hT_sb[:],
                                in_=tail_ps[:, 0:KH * P].rearrange(
                                    "p (k q) -> p k q", k=KH))
                            x2_ps = tail_ps[:, KH * P:KH * P + 2 * CO]
                            for k in range(KH):
                                nc.tensor.matmul(out=x2_ps, lhsT=hT_sb[:, k, :],
                                                 rhs=w2_sb[:, k, :],
                                                 start=(k == 0), stop=(k == KH - 1))
                            xl2_sb = sb.tile([P, CO], bf, tag="xl2s")
                            nc.vector.tensor_copy(out=xl2_sb[:], in_=x2_ps[:, 0:CO])
                            nc.vector.tensor_copy(out=xr2_all[:, nt, :],
                                                  in_=x2_ps[:, CO:2 * CO])
                            nc.sync.dma_start(
                                out=ag2_in[nt * P:(nt + 1) * P, 0:CO],
                                in_=xl2_sb[:])
                        if PHASES >= 3:
                            ag_chunk(ag2_in, stg2[groups.index((g0, g1))], tbl2, g0, g1)

            # ============ phase C: layer-2 edges ============
            if PHASES >= 3:
                with (tc.tile_pool(name="sbC", bufs=2) as sb,
                      tc.tile_pool(name="gbC", bufs=3) as gb,
                      tc.tile_pool(name="psC", bufs=2, space="PSUM") as ps):
                    for nt in range(NT):
                        T_ = T[nt]; Klo_ = Klo[nt]; Khi_ = Khi[nt]
                        g2lo = gb.tile([P, KM, CO_PAD], bf, tag="g2lo")
                        g2hi = gb.tile([P, KM, CO_PAD], bf, tag="g2hi")
                        g1i = nc.gpsimd.dma_gather(
                            g2lo[:, 0:Klo_, :], tbl2[0:HALF, :],
                            gidx_lo_sb[:, olo[nt] * 8:(olo[nt] + Klo_) * 8],
                            Klo_ * P, regs[Klo_], CO_PAD)
                        g2i = nc.gpsimd.dma_gather(
                            g2hi[:, 0:Khi_, :], tbl2[HALF:TBL, :],
                            gidx_hi_sb[:, ohi[nt] * 8:(ohi[nt] + Khi_) * 8],
                            Khi_ * P, regs[Khi_], CO_PAD)
                        _add_dep_helper(g1i.ins, lib.ins, sync=False, reason="lib")
                        _add_dep_helper(g2i.ins, lib.ins, sync=False, reason="lib")
                        ohne_t = gb.tile([P, (2 * KM) * P], bf, tag="ohne2")
                        nc.sync.dma_start(
                            out=ohne_t[:, 0:T_ * P],
                            in_=ohne_d[:, od[nt] * P:(od[nt] + T_) * P])

                        acc2 = ps.tile([P, CO + 1], fp32, tag="acc2", space="PSUM")
                        msg2 = sb.tile([P, 2 * KM, CO + 1], bf, tag="msg2")
                        for t0, K, gx in [(0, Klo_, g2lo), (Klo_, Khi_, g2hi)]:
                            z_ps = ps.tile([P, KM, CO], fp32, tag="z2", space="PSUM")
                            for j in range(K):
                                nc.tensor.matmul(
                                    out=z_ps[:, j, :],
                                    lhsT=ohne_t[:, (t0 + j) * P:(t0 + j + 1) * P],
                                    rhs=xr2_all[:, nt, :], start=True, stop=True)
                            v = sb.tile([P, KM, CO], bf, tag="v2")
                            nc.vector.tensor_tensor(
                                out=v[:, 0:K, :], in0=z_ps[:, 0:K, :],
                                in1=gx[:, 0:K, 0:CO], op=OP.add)
                            va = sb.tile([P, KM, CO], bf, tag="va2")
                            nc.vector.tensor_scalar(
                                out=va[:, 0:K, :], in0=v[:, 0:K, :],
                                scalar1=NEG_SLOPE, scalar2=None, op0=OP.mult)
                            tp = sb.tile([P, KM, CO], bf, tag="tp2")
                            nc.vector.tensor_tensor(
                                out=tp[:, 0:K, :], in0=v[:, 0:K, :],
                                in1=va[:, 0:K, :], op=OP.max)
                            ta = sb.tile([P, KM, CO], bf, tag="ta2")
                            nc.vector.tensor_tensor(
                                out=ta[:, 0:K, :], in0=tp[:, 0:K, :],
                                in1=att2BK[:, 0:K * CO].rearrange(
                                    "p (k c) -> p k c", k=K), op=OP.mult)
                            sc = sb.tile([P, KM], fp32, tag="sc2")
                            nc.vector.tensor_reduce(
                                out=sc[:, 0:K], in_=ta[:, 0:K, :],
                                axis=AX.X, op=OP.add)
                            ex = sb.tile([P, KM], bf, tag="ex2")
                            nc.scalar.activation(out=ex[:, 0:K], in_=sc[:, 0:K],
                                                 func=AF.Exp)
                            nc.vector.tensor_tensor(
                                out=msg2[:, t0:t0 + K, 0:CO],
                                in0=gx[:, 0:K, 0:CO],
                                in1=ex[:, 0:K][:, :, None].to_broadcast([P, K, CO]),
                                op=OP.mult)
                            nc.vector.tensor_copy(
                                out=msg2[:, t0:t0 + K, CO:CO + 1],
                                in_=ex[:, 0:K][:, :, None])
                            ohe = sb.tile([P, KM, P], bf, tag="ohe2")
                            nc.vector.tensor_tensor(
                                out=ohe[:, 0:K, :],
                                in0=iotaK[:, 0:K * P].rearrange(
                                    "p (k q) -> p k q", k=K),
                                in1=drel_sb[:, od[nt] + t0:od[nt] + t0 + K]
                                    [:, :, None].to_broadcast([P, K, P]),
                                op=OP.is_equal)
                            for j in range(K):
                                nc.tensor.matmul(
                                    out=acc2[:], lhsT=ohe[:, j, :],
                                    rhs=msg2[:, t0 + j, :],
                                    start=(t0 + j == 0), stop=(t0 + j == T_ - 1))

                        rec2 = sb.tile([P, 1], fp32, tag="rec2")
                        nc.vector.reciprocal(out=rec2[:], in_=acc2[:, CO:CO + 1])
                        h2 = sb.tile([P, CO], fp32, tag="h2")
                        nc.vector.tensor_scalar(out=h2[:], in0=acc2[:, 0:CO],
                                                scalar1=rec2[:, 0:1], scalar2=None,
                                                op0=OP.mult)
                        if dims["add_b2"]:
                            nc.vector.tensor_tensor(out=h2[:], in0=h2[:], in1=b2B,
                                                    op=OP.add)
                        nc.vector.tensor_copy(out=h2_all[:, nt, :], in_=h2[:])
                        nm = sb.tile([P, 1], fp32, tag="nm")
                        nc.vector.tensor_reduce(out=nm[:], in_=h2[:], axis=AX.X,
                                                op=OP.max, negate=True)
                        esc = sb.tile([P, CO], fp32, tag="esc")
                        ssum = sb.tile([P, 1], fp32, tag="ssum")
                        nc.scalar.activation(out=esc[:], in_=h2[:], func=AF.Exp,
                                             bias=nm[:, 0:1], accum_out=ssum[:, 0:1])
                        lns = sb.tile([P, 1], fp32, tag="lns")
                        nc.scalar.activation(out=lns[:], in_=ssum[:], func=AF.Ln)
                        nc.vector.tensor_scalar(
                            out=ls_all[:, nt, :], in0=h2[:], scalar1=nm[:, 0:1],
                            scalar2=lns[:, 0:1], op0=OP.add, op1=OP.subtract)

            nc.sync.dma_start(out=h2_out.rearrange("(a p) d -> p a d", p=P),
                              in_=h2_all[:])
            nc.sync.dma_start(out=ls_out.rearrange("(a p) d -> p a d", p=P),
                              in_=ls_all[:])

    if post_passes:
        _br.generate_event_semaphores(nc)
        _br.codegen_inst_isa_subclasses(nc)
    return nc


# --------------------------------------------------------------------------
# entry point
# --------------------------------------------------------------------------

def kernel(x, edge_index, W1l, W1r, att1, b1, W2l, W2r, att2, b2):
    x = np.asarray(x, np.float32)
    edge_index = np.asarray(edge_index)
    W1l = np.asarray(W1l, np.float32); W1r = np.asarray(W1r, np.float32)
    att1 = np.asarray(att1, np.float32); b1 = np.asarray(b1, np.float32)
    W2l = np.asarray(W2l, np.float32); W2r = np.asarray(W2r, np.float32)
    att2 = np.asarray(att2, np.float32); b2 = np.asarray(b2, np.float32)

    N, DIN = x.shape
    E = edge_index.shape[1]
    H, CH = att1.shape
    HC = W1l.shape[1]
    CO = W2l.shape[1]

    key = (N, E, DIN, H, CH, HC, CO,
           int(np.abs(b1).max() > 0), int(np.abs(b2).max() > 0),
           hash(edge_index.tobytes()))
    if key in _plan_cache:
        pp, nc, dims = _plan_cache[key]
    else:
        pp = _preprocess(N, E, edge_index)
        dims = dict(DIN=DIN, HC=HC, H=H, CH=CH, CO=CO,
                    NPC=pp["NPC"], NT=pp["NT"], TBL=pp["TBL"],
                    Klo=pp["Klo"], Khi=pp["Khi"], T=pp["T"], KM=pp["KM"],
                    od=pp["od"], olo=pp["olo"], ohi=pp["ohi"],
                    OD=pp["OD"], OLO=pp["OLO"], OHI=pp["OHI"],
                    groups=pp["groups"],
                    add_b1=bool(np.abs(b1).max() > 0),
                    add_b2=bool(np.abs(b2).max() > 0))
        nc = _build_program(dims)
        _plan_cache[key] = (pp, nc, dims)

    NPC = pp["NPC"]; NT = pp["NT"]; KM = pp["KM"]
    KD = DIN // P
    bfdt = ml_dtypes.bfloat16

    # consts blob
    iotaK = np.broadcast_to(
        np.tile(np.arange(P, dtype=np.float32), KM)[None, :], (P, KM * P))
    ident = np.eye(P, dtype=np.float32)
    attBK = np.broadcast_to(
        np.tile(att1.reshape(HC), KM)[None, :], (P, KM * HC))
    b1B = np.broadcast_to(b1.reshape(1, HC), (P, HC))
    att2BK = np.broadcast_to(
        np.tile(att2.reshape(CO), KM)[None, :], (P, KM * CO))
    b2B = np.broadcast_to(b2.reshape(1, CO), (P, CO))
    consts = np.concatenate([iotaK, ident, attBK, b1B, att2BK, b2B],
                            axis=1).astype(bfdt)
    constf = np.full((P, 1), NEG_SLOPE, np.float32)
    w1cat = np.concatenate([W1l, W1r], axis=1).astype(bfdt)
    w2cat = np.concatenate([W2l, W2r], axis=1).astype(bfdt)

    in_maps = []
    for c in range(NC):
        xkc = np.zeros((NPC, DIN), np.float32)
        sel = pp["node_order"][c]
        real = sel >= 0
        xkc[real] = x[sel[real]]
        # [p, nt, k, q] = xkc[nt*P+q, k*P+p]
        xkT = np.ascontiguousarray(
            xkc.reshape(NT, P, KD, P).transpose(3, 0, 2, 1)
        ).reshape(P, NT * KD * P).astype(bfdt)
        in_maps.append(dict(
            xkT=xkT, w1=w1cat, w2=w2cat, consts=consts, constf=constf,
            gidx_lo=np.ascontiguousarray(pp["gidx_lo"][c]),
            gidx_hi=np.ascontiguousarray(pp["gidx_hi"][c]),
            drel=pp["drel"][c].astype(bfdt),
            ohne=np.ascontiguousarray(pp["ohne"][c]),
        ))

    from concourse.bass_utils import run_bass_kernel_spmd
    res = run_bass_kernel_spmd(nc, in_maps, core_ids=list(range(NC)))

    h = np.empty((N, CO), np.float32)
    ls = np.empty((N, CO), np.float32)
    r_core = pp["core_of"]
    r_loc = pp["local_of"]
    for c in range(NC):
        m = r_core == c
        h[m] = res.results[c]["h2o"][r_loc[m]]
        ls[m] = res.results[c]["lso"][r_loc[m]]
    return h, ls


# revision 31
# speedup vs baseline: 1.8870x; 1.2541x over previous
"""2-layer GATv2 (PyG GATv2Conv semantics) on 8 Trainium2 NeuronCores.

Strategy (v2):
  - Nodes sharded across 8 cores; per-core greedy 2D packing balances each
    destination tile's lo/hi in-edge counts (lo = src owned by cores 0-3).
  - x is shipped pre-transposed; layer-1 projections are 6 accumulating
    matmuls per node tile with a combined [W1l|W1r] moving operand.
  - xl tables AllGathered HBM->HBM in 4 row-chunks overlapped with compute.
  - Per destination tile, incoming-edge source rows are fetched with
    dma_gather (int16 idx; two table halves).  The dst->edge one-hot
    (oh_ne, [dst, edge]) is precomputed on host and streamed from HBM;
    the edge->dst one-hot (oh_en) is one batched DVE is_equal per half.
  - Per half (K edge tiles): K z-matmuls (xr broadcast), one batched DVE
    add (z+xl), 2-op leaky, att mult, reduce, exp, msg mult -- all batched
    over the half's K*128 edges -- then K scatter matmuls accumulate
    numerator+denominator in PSUM.
  - Softmax skips max-subtraction (scores O(1)).
  - Layer 2 (heads=1, 16 ch) repeats the edge structure on a 256B-row table.
  - log_softmax on ACT/DVE per node tile.

kernel(**inputs) takes FULL inputs, returns FULL outputs.
"""

import os
import sys

if "/opt/trn_rl_repo" not in sys.path:
    sys.path.insert(0, "/opt/trn_rl_repo")

import numpy as np
import ml_dtypes

NC = 8          # cores
P = 128         # partitions
NEG_SLOPE = 0.2
NGRP = 4        # AllGather chunks

_plan_cache = {}


# --------------------------------------------------------------------------
# host-side graph preprocessing
# --------------------------------------------------------------------------

def _snake(order, nbins):
    n = len(order)
    ids = np.arange(n)
    round_ = ids // nbins
    pos = ids % nbins
    b = np.where(round_ % 2 == 0, pos, nbins - 1 - pos)
    out = np.empty(n, np.int64)
    out[:] = b
    return out


def _preprocess(N, E, edge_index):
    NPC = ((N + NC - 1) // NC + P - 1) // P * P    # padded nodes per core
    NT = NPC // P
    TBL = NC * NPC
    assert TBL // 2 < 32768, "table half must fit int16 row indices"

    src = np.concatenate([edge_index[0].astype(np.int64), np.arange(N)])
    dst = np.concatenate([edge_index[1].astype(np.int64), np.arange(N)])
    deg = np.bincount(dst, minlength=N)

    # --- group assignment (fixes each edge's table half), then cores within
    # each group balanced by OWN-group in-degree (the heavy, self-loop half)
    order = np.argsort(-deg, kind="stable")
    grp_of = np.empty(N, np.int64)
    grp_of[order] = _snake(order, 2)
    e_own = grp_of[src] == grp_of[dst]
    deg_own = np.bincount(dst[e_own], minlength=N)
    core_of = np.empty(N, np.int64)
    half_nc = NC // 2
    for g in range(2):
        nodes_g = np.where(grp_of == g)[0]
        og = nodes_g[np.argsort(-deg_own[nodes_g], kind="stable")]
        core_of[og] = g * half_nc + _snake(og, half_nc)

    lo_src = core_of[src] < NC // 2                # which table half each edge reads
    deg_lo = np.bincount(dst[lo_src], minlength=N)
    deg_hi = deg - deg_lo

    # --- per-core greedy 2D packing: balance (lo, hi) in-edge sums per tile
    slot_of = np.empty(N, np.int64)
    tile_of = np.empty(N, np.int64)
    nlo = np.zeros((NC, NT), np.int64)   # per-tile lo in-edges (incl. pad fakes)
    nhi = np.zeros((NC, NT), np.int64)
    cnt_ct = np.zeros((NC, NT), np.int64)
    for c in range(NC):
        heavy_is_lo = c < NC // 2        # own-group half carries the self-loops
        dh = deg_lo if heavy_is_lo else deg_hi
        dl = deg_hi if heavy_is_lo else deg_lo
        nodes = np.where(core_of == c)[0]
        nodes = nodes[np.argsort(-(dh[nodes] * 64 + dl[nodes]), kind="stable")]
        hv = np.zeros(NT, np.int64)
        lt = np.zeros(NT, np.int64)
        cnt = np.zeros(NT, np.int64)
        for v in nodes:
            cost = (hv + dh[v]).astype(np.float64) \
                + 0.02 * (lt + dl[v]) + 1e-4 * cnt \
                + 1e6 * np.maximum(lt + dl[v] - 5 * P, 0)
            cost[cnt >= P] = 1e18
            t = int(np.argmin(cost))
            tile_of[v] = t
            slot_of[v] = cnt[t]
            cnt[t] += 1
            hv[t] += dh[v]
            lt[t] += dl[v]
        lt += P - cnt                    # pad slots: one fake edge, light half
        if heavy_is_lo:
            nlo[c], nhi[c] = hv, lt
        else:
            nlo[c], nhi[c] = lt, hv
        cnt_ct[c] = cnt

    # --- per-core rank permutation so heavy tiles align across cores
    kl = (nlo + P - 1) // P
    kh = (nhi + P - 1) // P
    perm = np.empty((NC, NT), np.int64)     # rank -> old tile
    for c in range(NC):
        key = (kl[c] + kh[c]) + 1e-3 * kl[c] + 1e-9 * (nlo[c] + nhi[c])
        perm[c] = np.argsort(-key, kind="stable")

    Klo = [int(max(kl[c, perm[c, r]] for c in range(NC))) for r in range(NT)]
    Khi = [int(max(kh[c, perm[c, r]] for c in range(NC))) for r in range(NT)]
    T = [Klo[r] + Khi[r] for r in range(NT)]
    KM = max(max(Klo), max(Khi))

    rank_of = np.empty((NC, NT), np.int64)  # old tile -> rank
    for c in range(NC):
        rank_of[c, perm[c]] = np.arange(NT)
    local_of = rank_of[core_of, tile_of] * P + slot_of
    r_of = core_of * NPC + local_of         # final global table row of each node

    # --- per (core, old-tile) edge lists split by half (final r_of values)
    e_core = core_of[dst]
    e_tile = tile_of[dst]
    e_slot = slot_of[dst]
    lists_lo = {}
    lists_hi = {}
    for c in range(NC):
        m_c = e_core == c
        for tl in range(NT):
            m = m_c & (e_tile == tl)
            ml = m & lo_src
            mh = m & ~lo_src
            lists_lo[(c, tl)] = (r_of[src[ml]], e_slot[ml])
            lists_hi[(c, tl)] = (r_of[src[mh]] - TBL // 2, e_slot[mh])
            # fake self-edges for empty (padding) node slots so denominators
            # stay nonzero (their outputs are discarded by the host); they
            # live in the light half (matches the greedy's accounting)
            npad = P - cnt_ct[c, tl]
            if npad:
                pads = np.arange(P - npad, P)
                key = (c, tl)
                tgt = lists_hi if c < NC // 2 else lists_lo
                a, b = tgt[key]
                tgt[key] = (np.concatenate([a, np.zeros(npad, np.int64)]),
                            np.concatenate([b, pads]))

    # offsets
    od = np.concatenate([[0], np.cumsum(T)]).astype(np.int64)       # drel/ohne cols
    olo = np.concatenate([[0], np.cumsum(Klo)]).astype(np.int64)    # gidx_lo tiles
    ohi = np.concatenate([[0], np.cumsum(Khi)]).astype(np.int64)
    OD = int(od[-1]); OLO = int(olo[-1]); OHI = int(ohi[-1])

    def pack_idx(flat):
        n = len(flat)
        s = (n + 15) // 16
        arr = np.zeros(s * 16, np.int16)
        arr[:n] = flat
        block = arr.reshape(s, 16).T
        return np.tile(block, (8, 1))

    gidx_lo = np.zeros((NC, P, OLO * 8), np.int16)
    gidx_hi = np.zeros((NC, P, OHI * 8), np.int16)
    drel = np.full((NC, P, OD), -1.0, np.float32)
    for c in range(NC):
        for r in range(NT):
            tl = perm[c, r]
            for half, (K, off8, dcol0, lst) in enumerate([
                    (Klo[r], olo[r], od[r], lists_lo[(c, tl)]),
                    (Khi[r], ohi[r], od[r] + Klo[r], lists_hi[(c, tl)])]):
                rows, slots = lst
                # re-slot: dst slots were computed pre-permutation; slot within
                # tile is unchanged (rank remap keeps slot % P)
                n = len(rows)
                flat = np.zeros(K * P, np.int64)
                flat[:n] = rows
                g = gidx_hi if half else gidx_lo
                g[c, :, off8 * 8:(off8 + K) * 8] = pack_idx(flat)
                dr = np.full(K * P, -1.0, np.float32)
                dr[:n] = slots
                drel[c, :, dcol0:dcol0 + K] = dr.reshape(K, P).T

    node_order = np.full((NC, NPC), -1, np.int64)  # local row -> global node id
    for c in range(NC):
        nodes = np.where(core_of == c)[0]
        node_order[c, local_of[nodes]] = nodes

    # per tile col: [ohne (dst-major [d, e]) | ohen (edge-major [e, d])]
    ar = np.arange(P, dtype=np.float32)
    ohne = (ar[None, :, None, None]
            == drel.transpose(0, 2, 1)[:, None, :, :])   # [NC, P(d), OD, P(e)]
    ohen = (drel[:, :, :, None] == ar[None, None, None, :])  # [NC, P(e), OD, P(d)]
    ohb = np.empty((NC, P, 2 * OD * P), ml_dtypes.bfloat16)
    for r in range(NT):
        o0, o1 = int(od[r]), int(od[r + 1])
        t_ = o1 - o0
        ohb[:, :, 2 * o0 * P:(2 * o0 + t_) * P] = \
            ohne[:, :, o0:o1, :].reshape(NC, P, t_ * P)
        ohb[:, :, (2 * o0 + t_) * P:2 * o1 * P] = \
            ohen[:, :, o0:o1, :].reshape(NC, P, t_ * P)

    # AllGather groups (tile ranks)
    gsz = (NT + NGRP - 1) // NGRP
    groups = [(g * gsz, min((g + 1) * gsz, NT)) for g in range(NGRP)]
    groups = [(a, b) for a, b in groups if b > a]

    return dict(NPC=NPC, NT=NT, TBL=TBL, Klo=Klo, Khi=Khi, T=T, KM=KM,
                od=od.tolist(), olo=olo.tolist(), ohi=ohi.tolist(),
                OD=OD, OLO=OLO, OHI=OHI, groups=groups,
                gidx_lo=gidx_lo, gidx_hi=gidx_hi, ohb=ohb,
                node_order=node_order, core_of=core_of, local_of=local_of)


# --------------------------------------------------------------------------
# bass program
# --------------------------------------------------------------------------

def _build_program(dims, post_passes=True):
    PHASES = int(os.environ.get("GAT_PHASES", "3"))
    SHARED = os.environ.get("GAT_SHARED", "0") == "1"
    GB = int(os.environ.get("GAT_GB", "1"))        # gather batch (node tiles)
    import concourse.bass as bass
    import concourse.mybir as mybir
    import concourse.tile as tile
    from concourse import library_config
    from concourse.bass import _add_dep_helper
    import bass_rust as _br

    fp32 = mybir.dt.float32
    bf = mybir.dt.bfloat16
    i16 = mybir.dt.int16
    AX = mybir.AxisListType
    OP = mybir.AluOpType
    AF = mybir.ActivationFunctionType

    DIN = dims["DIN"]; HC = dims["HC"]; H = dims["H"]; CH = dims["CH"]
    CO = dims["CO"]
    NPC = dims["NPC"]; NT = dims["NT"]; TBL = dims["TBL"]
    Klo = dims["Klo"]; Khi = dims["Khi"]; T = dims["T"]; KM = dims["KM"]
    od = dims["od"]; olo = dims["olo"]; ohi = dims["ohi"]
    OD = dims["OD"]; OLO = dims["OLO"]; OHI = dims["OHI"]
    groups = dims["groups"]
    KD = DIN // P
    KH = HC // P
    CO_PAD = 128
    HALF = TBL // 2
    TM = max(T)
    addr_space = "Shared" if SHARED else "Local"

    # gather batches: consecutive ranks within each AG group, <= GB tiles
    batches = []        # (nt0, nt1)
    for g0, g1 in groups:
        nt = g0
        while nt < g1:
            batches.append((nt, min(nt + GB, g1)))
            nt = batches[-1][1]
    BKM = max(max(olo[b1] - olo[b0], ohi[b1] - ohi[b0]) for b0, b1 in batches)

    nc = bass.Bass(num_devices=NC, num_swdge_queues=4,
                   dynamic_dma_scratch_size=int(os.environ.get("GAT_DDS", "16384")))

    xkT_d = nc.dram_tensor("xkT", [P, NT * KD * P], bf, kind="ExternalInput")
    w1_d = nc.dram_tensor("w1", [DIN, 2 * HC], bf, kind="ExternalInput")
    w2_d = nc.dram_tensor("w2", [HC, 2 * CO], bf, kind="ExternalInput")
    CCOLS = KM * HC + HC + KM * CO + CO + P
    consts = nc.dram_tensor("consts", [P, CCOLS], bf, kind="ExternalInput")
    constf = nc.dram_tensor("constf", [P, 1], fp32, kind="ExternalInput")
    gidx_lo_d = nc.dram_tensor("gidx_lo", [P, OLO * 8], i16, kind="ExternalInput")
    gidx_hi_d = nc.dram_tensor("gidx_hi", [P, OHI * 8], i16, kind="ExternalInput")
    ohb_d = nc.dram_tensor("ohb", [P, 2 * OD * P], bf, kind="ExternalInput")
    h2_out = nc.dram_tensor("h2o", [NPC, CO], fp32, kind="ExternalOutput")
    ls_out = nc.dram_tensor("lso", [NPC, CO], fp32, kind="ExternalOutput")

    with tile.TileContext(nc) as tc:
        with (
            tc.tile_pool(name="dram", bufs=1, space="DRAM") as dram,
            tc.tile_pool(name="cst", bufs=1) as cst,
        ):
            lib = nc.gpsimd.load_library(library_config.mlp)
            regs = {}
            for b0, b1 in batches:
                for n in (olo[b1] - olo[b0], ohi[b1] - ohi[b0]):
                    if n not in regs:
                        regs[n] = nc.gpsimd.to_reg(n * P)

            ctile = cst.tile([P, CCOLS], bf)
            nc.sync.dma_start(out=ctile[:], in_=consts[:])
            cftile = cst.tile([P, 1], fp32)
            nc.sync.dma_start(out=cftile[:], in_=constf[:])
            o = 0
            ident = ctile[:, o:o + P]; o += P
            attBK = ctile[:, o:o + KM * HC]; o += KM * HC
            b1B = ctile[:, o:o + HC]; o += HC
            att2BK = ctile[:, o:o + KM * CO]; o += KM * CO
            b2B = ctile[:, o:o + CO]; o += CO
            alpha = cftile[:, 0:1]

            w1_sb = cst.tile([P, KD, 2 * HC], bf)
            nc.sync.dma_start(out=w1_sb[:], in_=w1_d.rearrange("(k p) c -> p k c", p=P))
            w2_sb = cst.tile([P, KH, 2 * CO], bf)
            nc.sync.dma_start(out=w2_sb[:], in_=w2_d.rearrange("(k p) c -> p k c", p=P))

            gidx_lo_sb = cst.tile([P, OLO * 8], i16)
            nc.sync.dma_start(out=gidx_lo_sb[:], in_=gidx_lo_d[:])
            gidx_hi_sb = cst.tile([P, OHI * 8], i16)
            nc.sync.dma_start(out=gidx_hi_sb[:], in_=gidx_hi_d[:])

            xr1_all = cst.tile([P, NT, HC], bf)
            xr2_all = cst.tile([P, NT, CO], bf)
            h2_all = cst.tile([P, NT, CO], fp32)
            ls_all = cst.tile([P, NT, CO], fp32)
            nc.vector.memset(h2_all[:], 0.0)
            nc.vector.memset(ls_all[:], 0.0)
            nc.vector.memset(xr2_all[:], 0.0)

            tbl1 = dram.tile([TBL, HC], bf)
            tbl2 = dram.tile([TBL, CO_PAD], bf)
            ag1_in = {}
            ag2_in = {}
            stg1 = {}
            stg2 = {}
            for gi, (g0, g1) in enumerate(groups):
                rows = (g1 - g0) * P
                ag1_in[gi] = dram.tile([rows, HC], bf, name=f"ag1i_{gi}")
                ag2_in[gi] = dram.tile([rows, CO_PAD], bf, name=f"ag2i_{gi}")
                stg1[gi] = dram.tile([NC * rows, HC], bf,
                                     addr_space=addr_space, name=f"stg1_{gi}")
                stg2[gi] = dram.tile([NC * rows, CO_PAD], bf,
                                     addr_space=addr_space, name=f"stg2_{gi}")

            def ag_chunk(src, stage, dst, g0, g1):
                nc.gpsimd.collective_compute(
                    "AllGather", mybir.AluOpType.bypass,
                    replica_groups=[list(range(NC))],
                    ins=[src[:].opt()],
                    outs=[stage[:].opt()],
                )
                nc.sync.dma_start(
                    out=dst[:].rearrange("(c n) h -> c n h", c=NC)
                        [:, g0 * P:g1 * P, :],
                    in_=stage[:].rearrange("(c n) h -> c n h", c=NC))

            # ============ phase A: layer-1 projections ============
            with (tc.tile_pool(name="sbA", bufs=3) as sb,
                  tc.tile_pool(name="psA", bufs=2, space="PSUM") as ps):
                for gi, (g0, g1) in enumerate(groups):
                    for nt in range(g0, g1):
                        xt = sb.tile([P, KD, P], bf, tag="xt")
                        nc.sync.dma_start(
                            out=xt[:],
                            in_=xkT_d[:, nt * KD * P:(nt + 1) * KD * P])
                        xlr_ps = ps.tile([P, 2 * HC], fp32, tag="mm", space="PSUM")
                        for k in range(KD):
                            nc.tensor.matmul(out=xlr_ps[:], lhsT=xt[:, k, :],
                                             rhs=w1_sb[:, k, :],
                                             start=(k == 0), stop=(k == KD - 1))
                        xl_sb = sb.tile([P, HC], bf, tag="xls")
                        nc.vector.tensor_copy(out=xl_sb[:], in_=xlr_ps[:, 0:HC])
                        nc.vector.tensor_copy(out=xr1_all[:, nt, :],
                                              in_=xlr_ps[:, HC:2 * HC])
                        nc.sync.dma_start(
                            out=ag1_in[gi][(nt - g0) * P:(nt - g0 + 1) * P, :],
                            in_=xl_sb[:])
                    if PHASES >= 2:
                        ag_chunk(ag1_in[gi], stg1[gi], tbl1, g0, g1)

            # ============ phase B: layer-1 edges ============
            if PHASES >= 2:
                grp_of_nt = {}
                for gi, (g0, g1) in enumerate(groups):
                    for nt in range(g0, g1):
                        grp_of_nt[nt] = gi
                with (tc.tile_pool(name="sbB", bufs=2) as sb,
                      tc.tile_pool(name="gbB", bufs=2) as gb,
                      tc.tile_pool(name="psB", bufs=2, space="PSUM") as ps):
                    for b0, b1 in batches:
                        nlo_b = olo[b1] - olo[b0]
                        nhi_b = ohi[b1] - ohi[b0]
                        glo = gb.tile([P, BKM, HC], bf, tag="glo")
                        ghi = gb.tile([P, BKM, HC], bf, tag="ghi")
                        qb = 0 if True else (2 * (b0 % 2)) % 4
                        g1i = nc.gpsimd.dma_gather(
                            glo[:, 0:nlo_b, :], tbl1[0:HALF, :],
                            gidx_lo_sb[:, olo[b0] * 8:olo[b1] * 8],
                            nlo_b * P, regs[nlo_b], HC, queue_num=qb)
                        g2i = nc.gpsimd.dma_gather(
                            ghi[:, 0:nhi_b, :], tbl1[HALF:TBL, :],
                            gidx_hi_sb[:, ohi[b0] * 8:ohi[b1] * 8],
                            nhi_b * P, regs[nhi_b], HC, queue_num=qb)
                        _add_dep_helper(g1i.ins, lib.ins, sync=False, reason="lib")
                        _add_dep_helper(g2i.ins, lib.ins, sync=False, reason="lib")

                        for nt in range(b0, b1):
                            T_ = T[nt]; Klo_ = Klo[nt]; Khi_ = Khi[nt]
                            blo = olo[nt] - olo[b0]
                            bhi = ohi[nt] - ohi[b0]
                            ohb_t = sb.tile([P, 2 * TM * P], bf, tag="ohb")
                            nc.sync.dma_start(
                                out=ohb_t[:, 0:2 * T_ * P],
                                in_=ohb_d[:, 2 * od[nt] * P:2 * (od[nt] + T_) * P])
                            ohne = ohb_t[:, 0:T_ * P]
                            ohen = ohb_t[:, T_ * P:2 * T_ * P]

                            acc = ps.tile([P, HC + H], fp32, tag="acc",
                                          space="PSUM", bufs=1)
                            msg = sb.tile([P, TM, HC + H], bf, tag="msg")
                            for t0, K, gx, gb0 in [(0, Klo_, glo, blo),
                                                   (Klo_, Khi_, ghi, bhi)]:
                                t_sb = sb.tile([P, KM, HC], bf, tag="t")
                                for j in range(K):
                                    zj = ps.tile([P, HC], fp32, tag="z",
                                                 space="PSUM", bufs=4)
                                    nc.tensor.matmul(
                                        out=zj[:],
                                        lhsT=ohne[:, (t0 + j) * P:(t0 + j + 1) * P],
                                        rhs=xr1_all[:, nt, :],
                                        start=True, stop=False)
                                    nc.tensor.matmul(
                                        out=zj[:], lhsT=ident,
                                        rhs=gx[:, gb0 + j, :],
                                        start=False, stop=True)
                                    nc.scalar.activation(out=t_sb[:, j, :],
                                                         in_=zj[:],
                                                         func=AF.Prelu, alpha=alpha)
                                ta = sb.tile([P, KM, HC], bf, tag="ta")
                                nc.vector.tensor_tensor(
                                    out=ta[:, 0:K, :], in0=t_sb[:, 0:K, :],
                                    in1=attBK[:, 0:K * HC].rearrange(
                                        "p (k c) -> p k c", k=K), op=OP.mult)
                                sc = sb.tile([P, KM * H], fp32, tag="sc")
                                nc.vector.tensor_reduce(
                                    out=sc[:, 0:K * H],
                                    in_=ta[:, 0:K, :].rearrange(
                                        "p k (h c) -> p (k h) c", h=H),
                                    axis=AX.X, op=OP.add)
                                ex = sb.tile([P, KM * H], bf, tag="ex")
                                nc.scalar.activation(out=ex[:, 0:K * H],
                                                     in_=sc[:, 0:K * H], func=AF.Exp)
                                nc.vector.tensor_tensor(
                                    out=msg[:, t0:t0 + K, 0:HC].rearrange(
                                        "p k (h c) -> p k h c", h=H),
                                    in0=gx[:, gb0:gb0 + K, :].rearrange(
                                        "p k (h c) -> p k h c", h=H),
                                    in1=ex[:, 0:K * H].rearrange(
                                        "p (k h) -> p k h", k=K)[:, :, :, None]
                                        .to_broadcast([P, K, H, CH]),
                                    op=OP.mult)
                                nc.vector.tensor_copy(
                                    out=msg[:, t0:t0 + K, HC:HC + H],
                                    in_=ex[:, 0:K * H].rearrange(
                                        "p (k h) -> p k h", k=K))
                                for j in range(K):
                                    nc.tensor.matmul(
                                        out=acc[:],
                                        lhsT=ohen[:, (t0 + j) * P:(t0 + j + 1) * P],
                                        rhs=msg[:, t0 + j, :],
                                        start=(t0 + j == 0), stop=(t0 + j == T_ - 1))

                            rec = sb.tile([P, H], fp32, tag="rec")
                            nc.vector.reciprocal(out=rec[:], in_=acc[:, HC:HC + H])
                            h1 = sb.tile([P, HC], fp32, tag="h1")
                            nc.vector.tensor_tensor(
                                out=h1[:].rearrange("p (h c) -> p h c", h=H),
                                in0=acc[:, 0:HC].rearrange("p (h c) -> p h c", h=H),
                                in1=rec[:, :, None].to_broadcast([P, H, CH]),
                                op=OP.mult)
                            if dims["add_b1"]:
                                nc.vector.tensor_tensor(out=h1[:], in0=h1[:],
                                                        in1=b1B, op=OP.add)
                            eh = sb.tile([P, HC], fp32, tag="eh")
                            nc.scalar.activation(out=eh[:], in_=h1[:], func=AF.Exp)
                            em = sb.tile([P, HC], fp32, tag="em")
                            nc.vector.tensor_scalar(
                                out=em[:], in0=eh[:], scalar1=1.0, scalar2=0.0,
                                op0=OP.subtract, op1=OP.min)
                            elu = sb.tile([P, HC], bf, tag="elu")
                            nc.vector.tensor_scalar(out=elu[:], in0=h1[:],
                                                    scalar1=0.0, scalar2=None,
                                                    op0=OP.max)
                            nc.vector.tensor_tensor(out=elu[:], in0=elu[:],
                                                    in1=em[:], op=OP.add)

                            tail_ps = ps.tile([P, KH * P + 2 * CO], fp32,
                                              tag="tail", space="PSUM", bufs=1)
                            for k in range(KH):
                                nc.tensor.matmul(
                                    out=tail_ps[:, k * P:(k + 1) * P],
                                    lhsT=elu[:, k * P:(k + 1) * P],
                                    rhs=ident, start=True, stop=True)
                            hT_sb = sb.tile([P, KH, P], bf, tag="hTs")
                            nc.vector.tensor_copy(
                                out=hT_sb[:],
                                in_=tail_ps[:, 0:KH * P].rearrange(
                                    "p (k q) -> p k q", k=KH))
                            x2_ps = tail_ps[:, KH * P:KH * P + 2 * CO]
                            for k in range(KH):
                                nc.tensor.matmul(out=x2_ps, lhsT=hT_sb[:, k, :],
                                                 rhs=w2_sb[:, k, :],
                                                 start=(k == 0), stop=(k == KH - 1))
                            xl2_sb = sb.tile([P, CO], bf, tag="xl2s")
                            nc.vector.tensor_copy(out=xl2_sb[:], in_=x2_ps[:, 0:CO])
                            nc.vector.tensor_copy(out=xr2_all[:, nt, :],
                                                  in_=x2_ps[:, CO:2 * CO])
                            gi = grp_of_nt[nt]
                            g0_, _ = groups[gi]
                            nc.sync.dma_start(
                                out=ag2_in[gi][(nt - g0_) * P:(nt - g0_ + 1) * P,
                                               0:CO],
                                in_=xl2_sb[:])
                        if PHASES >= 3 and b1 == groups[grp_of_nt[b0]][1]:
                            gi = grp_of_nt[b0]
                            g0_, g1_ = groups[gi]
                            ag_chunk(ag2_in[gi], stg2[gi], tbl2, g0_, g1_)

            # ============ phase C: layer-2 edges ============
            if PHASES >= 3:
                with (tc.tile_pool(name="sbC", bufs=2) as sb,
                      tc.tile_pool(name="gbC", bufs=2) as gb,
                      tc.tile_pool(name="psC", bufs=2, space="PSUM") as ps):
                    for b0, b1 in batches:
                        nlo_b = olo[b1] - olo[b0]
                        nhi_b = ohi[b1] - ohi[b0]
                        g2lo = gb.tile([P, BKM, CO_PAD], bf, tag="g2lo")
                        g2hi = gb.tile([P, BKM, CO_PAD], bf, tag="g2hi")
                        qb = 0 if True else (2 * (b0 % 2)) % 4
                        g1i = nc.gpsimd.dma_gather(
                            g2lo[:, 0:nlo_b, :], tbl2[0:HALF, :],
                            gidx_lo_sb[:, olo[b0] * 8:olo[b1] * 8],
                            nlo_b * P, regs[nlo_b], CO_PAD, queue_num=qb)
                        g2i = nc.gpsimd.dma_gather(
                            g2hi[:, 0:nhi_b, :], tbl2[HALF:TBL, :],
                            gidx_hi_sb[:, ohi[b0] * 8:ohi[b1] * 8],
                            nhi_b * P, regs[nhi_b], CO_PAD, queue_num=qb)
                        _add_dep_helper(g1i.ins, lib.ins, sync=False, reason="lib")
                        _add_dep_helper(g2i.ins, lib.ins, sync=False, reason="lib")

                        for nt in range(b0, b1):
                            T_ = T[nt]; Klo_ = Klo[nt]; Khi_ = Khi[nt]
                            blo = olo[nt] - olo[b0]
                            bhi = ohi[nt] - ohi[b0]
                            ohb_t = sb.tile([P, 2 * TM * P], bf, tag="ohb2")
                            nc.sync.dma_start(
                                out=ohb_t[:, 0:2 * T_ * P],
                                in_=ohb_d[:, 2 * od[nt] * P:2 * (od[nt] + T_) * P])
                            ohne = ohb_t[:, 0:T_ * P]
                            ohen = ohb_t[:, T_ * P:2 * T_ * P]

                            acc2 = ps.tile([P, CO + 1], fp32, tag="acc2",
                                           space="PSUM", bufs=1)
                            msg2 = sb.tile([P, TM, CO + 1], bf, tag="msg2")
                            for t0, K, gx, gb0 in [(0, Klo_, g2lo, blo),
                                                   (Klo_, Khi_, g2hi, bhi)]:
                                t2 = sb.tile([P, KM, CO], bf, tag="t2")
                                for j in range(K):
                                    zj = ps.tile([P, CO], fp32, tag="z2",
                                                 space="PSUM", bufs=4)
                                    nc.tensor.matmul(
                                        out=zj[:],
                                        lhsT=ohne[:, (t0 + j) * P:(t0 + j + 1) * P],
                                        rhs=xr2_all[:, nt, :],
                                        start=True, stop=False)
                                    nc.tensor.matmul(
                                        out=zj[:], lhsT=ident,
                                        rhs=gx[:, gb0 + j, 0:CO],
                                        start=False, stop=True)
                                    nc.scalar.activation(out=t2[:, j, :],
                                                         in_=zj[:],
                                                         func=AF.Prelu, alpha=alpha)
                                ta2 = sb.tile([P, KM, CO], bf, tag="ta2")
                                nc.vector.tensor_tensor(
                                    out=ta2[:, 0:K, :], in0=t2[:, 0:K, :],
                                    in1=att2BK[:, 0:K * CO].rearrange(
                                        "p (k c) -> p k c", k=K), op=OP.mult)
                                sc2 = sb.tile([P, KM], fp32, tag="sc2")
                                nc.vector.tensor_reduce(
                                    out=sc2[:, 0:K], in_=ta2[:, 0:K, :],
                                    axis=AX.X, op=OP.add)
                                ex2 = sb.tile([P, KM], bf, tag="ex2")
                                nc.scalar.activation(out=ex2[:, 0:K],
                                                     in_=sc2[:, 0:K], func=AF.Exp)
                                nc.vector.tensor_tensor(
                                    out=msg2[:, t0:t0 + K, 0:CO],
                                    in0=gx[:, gb0:gb0 + K, 0:CO],
                                    in1=ex2[:, 0:K][:, :, None]
                                        .to_broadcast([P, K, CO]),
                                    op=OP.mult)
                                nc.vector.tensor_copy(
                                    out=msg2[:, t0:t0 + K, CO:CO + 1],
                                    in_=ex2[:, 0:K][:, :, None])
                                for j in range(K):
                                    nc.tensor.matmul(
                                        out=acc2[:],
                                        lhsT=ohen[:, (t0 + j) * P:(t0 + j + 1) * P],
                                        rhs=msg2[:, t0 + j, :],
                                        start=(t0 + j == 0), stop=(t0 + j == T_ - 1))

                            rec2 = sb.tile([P, 1], fp32, tag="rec2")
                            nc.vector.reciprocal(out=rec2[:], in_=acc2[:, CO:CO + 1])
                            h2 = sb.tile([P, CO], fp32, tag="h2")
                            nc.vector.tensor_scalar(out=h2[:], in0=acc2[:, 0:CO],
                                                    scalar1=rec2[:, 0:1],
                                                    scalar2=None, op0=OP.mult)
                            if dims["add_b2"]:
                                nc.vector.tensor_tensor(out=h2[:], in0=h2[:],
                                                        in1=b2B, op=OP.add)
                            nc.vector.tensor_copy(out=h2_all[:, nt, :], in_=h2[:])
                            nm = sb.tile([P, 1], fp32, tag="nm")
                            nc.vector.tensor_reduce(out=nm[:], in_=h2[:], axis=AX.X,
                                                    op=OP.max, negate=True)
                            esc = sb.tile([P, CO], fp32, tag="esc")
                            ssum = sb.tile([P, 1], fp32, tag="ssum")
                            nc.scalar.activation(out=esc[:], in_=h2[:], func=AF.Exp,
                                                 bias=nm[:, 0:1],
                                                 accum_out=ssum[:, 0:1])
                            lns = sb.tile([P, 1], fp32, tag="lns")
                            nc.scalar.activation(out=lns[:], in_=ssum[:], func=AF.Ln)
                            nc.vector.tensor_scalar(
                                out=ls_all[:, nt, :], in0=h2[:], scalar1=nm[:, 0:1],
                                scalar2=lns[:, 0:1], op0=OP.add, op1=OP.subtract)

            nc.sync.dma_start(out=h2_out.rearrange("(a p) d -> p a d", p=P),
                              in_=h2_all[:])
            nc.sync.dma_start(out=ls_out.rearrange("(a p) d -> p a d", p=P),
                              in_=ls_all[:])

    if post_passes:
        _br.generate_event_semaphores(nc)
        _br.codegen_inst_isa_subclasses(nc)
    return nc


# --------------------------------------------------------------------------
# entry point
# --------------------------------------------------------------------------

def kernel(x, edge_index, W1l, W1r, att1, b1, W2l, W2r, att2, b2):
    x = np.asarray(x, np.float32)
    edge_index = np.asarray(edge_index)
    W1l = np.asarray(W1l, np.float32); W1r = np.asarray(W1r, np.float32)
    att1 = np.asarray(att1, np.float32); b1 = np.asarray(b1, np.float32)
    W2l = np.asarray(W2l, np.float32); W2r = np.asarray(W2r, np.float32)
    att2 = np.asarray(att2, np.float32); b2 = np.asarray(b2, np.float32)

    N, DIN = x.shape
    E = edge_index.shape[1]
    H, CH = att1.shape
    HC = W1l.shape[1]
    CO = W2l.shape[1]

    key = (N, E, DIN, H, CH, HC, CO,
           int(np.abs(b1).max() > 0), int(np.abs(b2).max() > 0),
           hash(edge_index.tobytes()))
    if key in _plan_cache:
        pp, nc, dims = _plan_cache[key]
    else:
        pp = _preprocess(N, E, edge_index)
        dims = dict(DIN=DIN, HC=HC, H=H, CH=CH, CO=CO,
                    NPC=pp["NPC"], NT=pp["NT"], TBL=pp["TBL"],
                    Klo=pp["Klo"], Khi=pp["Khi"], T=pp["T"], KM=pp["KM"],
                    od=pp["od"], olo=pp["olo"], ohi=pp["ohi"],
                    OD=pp["OD"], OLO=pp["OLO"], OHI=pp["OHI"],
                    groups=pp["groups"],
                    add_b1=bool(np.abs(b1).max() > 0),
                    add_b2=bool(np.abs(b2).max() > 0))
        nc = _build_program(dims)
        _plan_cache[key] = (pp, nc, dims)

    NPC = pp["NPC"]; NT = pp["NT"]; KM = pp["KM"]
    KD = DIN // P
    bfdt = ml_dtypes.bfloat16

    # consts blob: ident | attBK | b1B | att2BK | b2B
    ident = np.eye(P, dtype=np.float32)
    attBK = np.broadcast_to(
        np.tile(att1.reshape(HC), KM)[None, :], (P, KM * HC))
    b1B = np.broadcast_to(b1.reshape(1, HC), (P, HC))
    att2BK = np.broadcast_to(
        np.tile(att2.reshape(CO), KM)[None, :], (P, KM * CO))
    b2B = np.broadcast_to(b2.reshape(1, CO), (P, CO))
    consts = np.concatenate([ident, attBK, b1B, att2BK, b2B],
                            axis=1).astype(bfdt)
    constf = np.full((P, 1), NEG_SLOPE, np.float32)
    w1cat = np.concatenate([W1l, W1r], axis=1).astype(bfdt)
    w2cat = np.concatenate([W2l, W2r], axis=1).astype(bfdt)

    in_maps = []
    for c in range(NC):
        xkc = np.zeros((NPC, DIN), np.float32)
        sel = pp["node_order"][c]
        real = sel >= 0
        xkc[real] = x[sel[real]]
        # [p, nt, k, q] = xkc[nt*P+q, k*P+p]
        xkT = np.ascontiguousarray(
            xkc.reshape(NT, P, KD, P).transpose(3, 0, 2, 1)
        ).reshape(P, NT * KD * P).astype(bfdt)
        in_maps.append(dict(
            xkT=xkT, w1=w1cat, w2=w2cat, consts=consts, constf=constf,
            gidx_lo=np.ascontiguousarray(pp["gidx_lo"][c]),
            gidx_hi=np.ascontiguousarray(pp["gidx_hi"][c]),
            ohb=np.ascontiguousarray(pp["ohb"][c]),
        ))

    from concourse.bass_utils import run_bass_kernel_spmd
    res = run_bass_kernel_spmd(nc, in_maps, core_ids=list(range(NC)))

    h = np.empty((N, CO), np.float32)
    ls = np.empty((N, CO), np.float32)
    r_core = pp["core_of"]
    r_loc = pp["local_of"]
    for c in range(NC):
        m = r_core == c
        h[m] = res.results[c]["h2o"][r_loc[m]]
        ls[m] = res.results[c]["lso"][r_loc[m]]
    return h, ls


# revision 32
# speedup vs baseline: 1.9240x; 1.0196x over previous
"""2-layer GATv2 (PyG GATv2Conv semantics) on 8 Trainium2 NeuronCores.

Strategy (v2):
  - Nodes sharded across 8 cores; per-core greedy 2D packing balances each
    destination tile's lo/hi in-edge counts (lo = src owned by cores 0-3).
  - x is shipped pre-transposed; layer-1 projections are 6 accumulating
    matmuls per node tile with a combined [W1l|W1r] moving operand.
  - xl tables AllGathered HBM->HBM in 4 row-chunks overlapped with compute.
  - Per destination tile, incoming-edge source rows are fetched with
    dma_gather (int16 idx; two table halves).  The dst->edge one-hot
    (oh_ne, [dst, edge]) is precomputed on host and streamed from HBM;
    the edge->dst one-hot (oh_en) is one batched DVE is_equal per half.
  - Per half (K edge tiles): K z-matmuls (xr broadcast), one batched DVE
    add (z+xl), 2-op leaky, att mult, reduce, exp, msg mult -- all batched
    over the half's K*128 edges -- then K scatter matmuls accumulate
    numerator+denominator in PSUM.
  - Softmax skips max-subtraction (scores O(1)).
  - Layer 2 (heads=1, 16 ch) repeats the edge structure on a 256B-row table.
  - log_softmax on ACT/DVE per node tile.

kernel(**inputs) takes FULL inputs, returns FULL outputs.
"""

import os
import sys

if "/opt/trn_rl_repo" not in sys.path:
    sys.path.insert(0, "/opt/trn_rl_repo")

import numpy as np
import ml_dtypes

NC = 8          # cores
P = 128         # partitions
NEG_SLOPE = 0.2
NGRP = 4        # AllGather chunks

_plan_cache = {}


# --------------------------------------------------------------------------
# host-side graph preprocessing
# --------------------------------------------------------------------------

def _snake(order, nbins):
    n = len(order)
    ids = np.arange(n)
    round_ = ids // nbins
    pos = ids % nbins
    b = np.where(round_ % 2 == 0, pos, nbins - 1 - pos)
    out = np.empty(n, np.int64)
    out[:] = b
    return out


def _preprocess(N, E, edge_index):
    NPC = ((N + NC - 1) // NC + P - 1) // P * P    # padded nodes per core
    NT = NPC // P
    TBL = NC * NPC
    assert TBL // 2 < 32768, "table half must fit int16 row indices"

    src = np.concatenate([edge_index[0].astype(np.int64), np.arange(N)])
    dst = np.concatenate([edge_index[1].astype(np.int64), np.arange(N)])
    deg = np.bincount(dst, minlength=N)

    # --- group assignment (fixes each edge's table half), then cores within
    # each group balanced by OWN-group in-degree (the heavy, self-loop half)
    order = np.argsort(-deg, kind="stable")
    grp_of = np.empty(N, np.int64)
    grp_of[order] = _snake(order, 2)
    e_own = grp_of[src] == grp_of[dst]
    deg_own = np.bincount(dst[e_own], minlength=N)
    core_of = np.empty(N, np.int64)
    half_nc = NC // 2
    for g in range(2):
        nodes_g = np.where(grp_of == g)[0]
        og = nodes_g[np.argsort(-deg_own[nodes_g], kind="stable")]
        core_of[og] = g * half_nc + _snake(og, half_nc)

    lo_src = core_of[src] < NC // 2                # which table half each edge reads
    deg_lo = np.bincount(dst[lo_src], minlength=N)
    deg_hi = deg - deg_lo

    # --- per-core greedy 2D packing: balance (lo, hi) in-edge sums per tile
    slot_of = np.empty(N, np.int64)
    tile_of = np.empty(N, np.int64)
    nlo = np.zeros((NC, NT), np.int64)   # per-tile lo in-edges (incl. pad fakes)
    nhi = np.zeros((NC, NT), np.int64)
    cnt_ct = np.zeros((NC, NT), np.int64)
    for c in range(NC):
        heavy_is_lo = c < NC // 2        # own-group half carries the self-loops
        dh = deg_lo if heavy_is_lo else deg_hi
        dl = deg_hi if heavy_is_lo else deg_lo
        nodes = np.where(core_of == c)[0]
        nodes = nodes[np.argsort(-(dh[nodes] * 64 + dl[nodes]), kind="stable")]
        hv = np.zeros(NT, np.int64)
        lt = np.zeros(NT, np.int64)
        cnt = np.zeros(NT, np.int64)
        for v in nodes:
            cost = (hv + dh[v]).astype(np.float64) \
                + 0.02 * (lt + dl[v]) + 1e-4 * cnt \
                + 1e6 * np.maximum(lt + dl[v] - 5 * P, 0)
            cost[cnt >= P] = 1e18
            t = int(np.argmin(cost))
            tile_of[v] = t
            slot_of[v] = cnt[t]
            cnt[t] += 1
            hv[t] += dh[v]
            lt[t] += dl[v]
        lt += P - cnt                    # pad slots: one fake edge, light half
        if heavy_is_lo:
            nlo[c], nhi[c] = hv, lt
        else:
            nlo[c], nhi[c] = lt, hv
        cnt_ct[c] = cnt

    # --- per-core rank permutation so heavy tiles align across cores
    kl = (nlo + P - 1) // P
    kh = (nhi + P - 1) // P
    perm = np.empty((NC, NT), np.int64)     # rank -> old tile
    for c in range(NC):
        key = (kl[c] + kh[c]) + 1e-3 * kl[c] + 1e-9 * (nlo[c] + nhi[c])
        perm[c] = np.argsort(-key, kind="stable")

    Klo = [int(max(kl[c, perm[c, r]] for c in range(NC))) for r in range(NT)]
    Khi = [int(max(kh[c, perm[c, r]] for c in range(NC))) for r in range(NT)]
    T = [Klo[r] + Khi[r] for r in range(NT)]
    KM = max(max(Klo), max(Khi))

    rank_of = np.empty((NC, NT), np.int64)  # old tile -> rank
    for c in range(NC):
        rank_of[c, perm[c]] = np.arange(NT)
    local_of = rank_of[core_of, tile_of] * P + slot_of
    r_of = core_of * NPC + local_of         # final global table row of each node

    # --- per (core, old-tile) edge lists split by half (final r_of values)
    e_core = core_of[dst]
    e_tile = tile_of[dst]
    e_slot = slot_of[dst]
    lists_lo = {}
    lists_hi = {}
    for c in range(NC):
        m_c = e_core == c
        for tl in range(NT):
            m = m_c & (e_tile == tl)
            ml = m & lo_src
            mh = m & ~lo_src
            lists_lo[(c, tl)] = (r_of[src[ml]], e_slot[ml])
            lists_hi[(c, tl)] = (r_of[src[mh]] - TBL // 2, e_slot[mh])
            # fake self-edges for empty (padding) node slots so denominators
            # stay nonzero (their outputs are discarded by the host); they
            # live in the light half (matches the greedy's accounting)
            npad = P - cnt_ct[c, tl]
            if npad:
                pads = np.arange(P - npad, P)
                key = (c, tl)
                tgt = lists_hi if c < NC // 2 else lists_lo
                a, b = tgt[key]
                tgt[key] = (np.concatenate([a, np.zeros(npad, np.int64)]),
                            np.concatenate([b, pads]))

    # offsets
    od = np.concatenate([[0], np.cumsum(T)]).astype(np.int64)       # drel/ohne cols
    olo = np.concatenate([[0], np.cumsum(Klo)]).astype(np.int64)    # gidx_lo tiles
    ohi = np.concatenate([[0], np.cumsum(Khi)]).astype(np.int64)
    OD = int(od[-1]); OLO = int(olo[-1]); OHI = int(ohi[-1])

    def pack_idx(flat):
        n = len(flat)
        s = (n + 15) // 16
        arr = np.zeros(s * 16, np.int16)
        arr[:n] = flat
        block = arr.reshape(s, 16).T
        return np.tile(block, (8, 1))

    gidx_lo = np.zeros((NC, P, OLO * 8), np.int16)
    gidx_hi = np.zeros((NC, P, OHI * 8), np.int16)
    drel = np.full((NC, P, OD), -1.0, np.float32)
    for c in range(NC):
        for r in range(NT):
            tl = perm[c, r]
            for half, (K, off8, dcol0, lst) in enumerate([
                    (Klo[r], olo[r], od[r], lists_lo[(c, tl)]),
                    (Khi[r], ohi[r], od[r] + Klo[r], lists_hi[(c, tl)])]):
                rows, slots = lst
                # re-slot: dst slots were computed pre-permutation; slot within
                # tile is unchanged (rank remap keeps slot % P)
                n = len(rows)
                flat = np.zeros(K * P, np.int64)
                flat[:n] = rows
                g = gidx_hi if half else gidx_lo
                g[c, :, off8 * 8:(off8 + K) * 8] = pack_idx(flat)
                dr = np.full(K * P, -1.0, np.float32)
                dr[:n] = slots
                drel[c, :, dcol0:dcol0 + K] = dr.reshape(K, P).T

    node_order = np.full((NC, NPC), -1, np.int64)  # local row -> global node id
    for c in range(NC):
        nodes = np.where(core_of == c)[0]
        node_order[c, local_of[nodes]] = nodes

    # per tile col: [ohne (dst-major [d, e]) | ohen (edge-major [e, d])]
    ar = np.arange(P, dtype=np.float32)
    ohne = (ar[None, :, None, None]
            == drel.transpose(0, 2, 1)[:, None, :, :])   # [NC, P(d), OD, P(e)]
    ohen = (drel[:, :, :, None] == ar[None, None, None, :])  # [NC, P(e), OD, P(d)]
    ohb = np.empty((NC, P, 2 * OD * P), ml_dtypes.bfloat16)
    for r in range(NT):
        o0, o1 = int(od[r]), int(od[r + 1])
        t_ = o1 - o0
        ohb[:, :, 2 * o0 * P:(2 * o0 + t_) * P] = \
            ohne[:, :, o0:o1, :].reshape(NC, P, t_ * P)
        ohb[:, :, (2 * o0 + t_) * P:2 * o1 * P] = \
            ohen[:, :, o0:o1, :].reshape(NC, P, t_ * P)

    # AllGather groups (tile ranks)
    gsz = (NT + NGRP - 1) // NGRP
    groups = [(g * gsz, min((g + 1) * gsz, NT)) for g in range(NGRP)]
    groups = [(a, b) for a, b in groups if b > a]

    return dict(NPC=NPC, NT=NT, TBL=TBL, Klo=Klo, Khi=Khi, T=T, KM=KM,
                od=od.tolist(), olo=olo.tolist(), ohi=ohi.tolist(),
                OD=OD, OLO=OLO, OHI=OHI, groups=groups,
                gidx_lo=gidx_lo, gidx_hi=gidx_hi, ohb=ohb,
                node_order=node_order, core_of=core_of, local_of=local_of)


# --------------------------------------------------------------------------
# bass program
# --------------------------------------------------------------------------

def _build_program(dims, post_passes=True):
    PHASES = int(os.environ.get("GAT_PHASES", "3"))
    SHARED = os.environ.get("GAT_SHARED", "0") == "1"
    GB = int(os.environ.get("GAT_GB", "1"))        # gather batch (node tiles)
    import concourse.bass as bass
    import concourse.mybir as mybir
    import concourse.tile as tile
    from concourse import library_config
    from concourse.bass import _add_dep_helper
    import bass_rust as _br

    fp32 = mybir.dt.float32
    bf = mybir.dt.bfloat16
    i16 = mybir.dt.int16
    AX = mybir.AxisListType
    OP = mybir.AluOpType
    AF = mybir.ActivationFunctionType

    DIN = dims["DIN"]; HC = dims["HC"]; H = dims["H"]; CH = dims["CH"]
    CO = dims["CO"]
    NPC = dims["NPC"]; NT = dims["NT"]; TBL = dims["TBL"]
    Klo = dims["Klo"]; Khi = dims["Khi"]; T = dims["T"]; KM = dims["KM"]
    od = dims["od"]; olo = dims["olo"]; ohi = dims["ohi"]
    OD = dims["OD"]; OLO = dims["OLO"]; OHI = dims["OHI"]
    groups = dims["groups"]
    KD = DIN // P
    KH = HC // P
    CO_PAD = 128
    HALF = TBL // 2
    TM = max(T)
    addr_space = "Shared" if SHARED else "Local"

    # gather batches: consecutive ranks within each AG group, <= GB tiles
    batches = []        # (nt0, nt1)
    for g0, g1 in groups:
        nt = g0
        while nt < g1:
            batches.append((nt, min(nt + GB, g1)))
            nt = batches[-1][1]
    BKM = max(max(olo[b1] - olo[b0], ohi[b1] - ohi[b0]) for b0, b1 in batches)

    nc = bass.Bass(num_devices=NC, num_swdge_queues=4,
                   dynamic_dma_scratch_size=int(os.environ.get("GAT_DDS", "16384")))

    xkT_d = nc.dram_tensor("xkT", [P, NT * KD * P], bf, kind="ExternalInput")
    w1_d = nc.dram_tensor("w1", [DIN, 2 * HC], bf, kind="ExternalInput")
    w2_d = nc.dram_tensor("w2", [HC, 2 * CO], bf, kind="ExternalInput")
    CCOLS = KM * HC + HC + KM * CO + CO + P
    consts = nc.dram_tensor("consts", [P, CCOLS], bf, kind="ExternalInput")
    constf = nc.dram_tensor("constf", [P, 1], fp32, kind="ExternalInput")
    gidx_lo_d = nc.dram_tensor("gidx_lo", [P, OLO * 8], i16, kind="ExternalInput")
    gidx_hi_d = nc.dram_tensor("gidx_hi", [P, OHI * 8], i16, kind="ExternalInput")
    ohb_d = nc.dram_tensor("ohb", [P, 2 * OD * P], bf, kind="ExternalInput")
    h2_out = nc.dram_tensor("h2o", [NPC, CO], fp32, kind="ExternalOutput")
    ls_out = nc.dram_tensor("lso", [NPC, CO], fp32, kind="ExternalOutput")

    with tile.TileContext(nc) as tc:
        with (
            tc.tile_pool(name="dram", bufs=1, space="DRAM") as dram,
            tc.tile_pool(name="cst", bufs=1) as cst,
        ):
            lib = nc.gpsimd.load_library(library_config.mlp)
            regs = {}
            for b0, b1 in batches:
                for n in (olo[b1] - olo[b0], ohi[b1] - ohi[b0]):
                    if n not in regs:
                        regs[n] = nc.gpsimd.to_reg(n * P)

            ctile = cst.tile([P, CCOLS], bf)
            nc.sync.dma_start(out=ctile[:], in_=consts[:])
            cftile = cst.tile([P, 1], fp32)
            nc.sync.dma_start(out=cftile[:], in_=constf[:])
            o = 0
            ident = ctile[:, o:o + P]; o += P
            attBK = ctile[:, o:o + KM * HC]; o += KM * HC
            b1B = ctile[:, o:o + HC]; o += HC
            att2BK = ctile[:, o:o + KM * CO]; o += KM * CO
            b2B = ctile[:, o:o + CO]; o += CO
            alpha = cftile[:, 0:1]

            w1_sb = cst.tile([P, KD, 2 * HC], bf)
            nc.sync.dma_start(out=w1_sb[:], in_=w1_d.rearrange("(k p) c -> p k c", p=P))
            w2_sb = cst.tile([P, KH, 2 * CO], bf)
            nc.sync.dma_start(out=w2_sb[:], in_=w2_d.rearrange("(k p) c -> p k c", p=P))

            gidx_lo_sb = cst.tile([P, OLO * 8], i16)
            nc.sync.dma_start(out=gidx_lo_sb[:], in_=gidx_lo_d[:])
            gidx_hi_sb = cst.tile([P, OHI * 8], i16)
            nc.sync.dma_start(out=gidx_hi_sb[:], in_=gidx_hi_d[:])

            xr1_all = cst.tile([P, NT, HC], bf)
            xr2_all = cst.tile([P, NT, CO], bf)
            h2_all = cst.tile([P, NT, CO], fp32)
            ls_all = cst.tile([P, NT, CO], fp32)
            nc.vector.memset(h2_all[:], 0.0)
            nc.vector.memset(ls_all[:], 0.0)
            nc.vector.memset(xr2_all[:], 0.0)

            tbl1 = dram.tile([TBL, HC], bf)
            tbl2 = dram.tile([TBL, CO_PAD], bf)
            ag1_in = {}
            ag2_in = {}
            stg1 = {}
            stg2 = {}
            for gi, (g0, g1) in enumerate(groups):
                rows = (g1 - g0) * P
                ag1_in[gi] = dram.tile([rows, HC], bf, name=f"ag1i_{gi}")
                ag2_in[gi] = dram.tile([rows, CO_PAD], bf, name=f"ag2i_{gi}")
                stg1[gi] = dram.tile([NC * rows, HC], bf,
                                     addr_space=addr_space, name=f"stg1_{gi}")
                stg2[gi] = dram.tile([NC * rows, CO_PAD], bf,
                                     addr_space=addr_space, name=f"stg2_{gi}")

            def ag_chunk(src, stage, dst, g0, g1):
                nc.gpsimd.collective_compute(
                    "AllGather", mybir.AluOpType.bypass,
                    replica_groups=[list(range(NC))],
                    ins=[src[:].opt()],
                    outs=[stage[:].opt()],
                )
                nc.scalar.dma_start(
                    out=dst[:].rearrange("(c n) h -> c n h", c=NC)
                        [:, g0 * P:g1 * P, :],
                    in_=stage[:].rearrange("(c n) h -> c n h", c=NC))

            # ============ phase A: layer-1 projections ============
            with (tc.tile_pool(name="sbA", bufs=3) as sb,
                  tc.tile_pool(name="psA", bufs=2, space="PSUM") as ps):
                for gi, (g0, g1) in enumerate(groups):
                    for nt in range(g0, g1):
                        xt = sb.tile([P, KD, P], bf, tag="xt")
                        nc.sync.dma_start(
                            out=xt[:],
                            in_=xkT_d[:, nt * KD * P:(nt + 1) * KD * P])
                        xlr_ps = ps.tile([P, 2 * HC], fp32, tag="mm", space="PSUM")
                        for k in range(KD):
                            nc.tensor.matmul(out=xlr_ps[:], lhsT=xt[:, k, :],
                                             rhs=w1_sb[:, k, :],
                                             start=(k == 0), stop=(k == KD - 1))
                        xl_sb = sb.tile([P, HC], bf, tag="xls")
                        nc.vector.tensor_copy(out=xl_sb[:], in_=xlr_ps[:, 0:HC])
                        nc.vector.tensor_copy(out=xr1_all[:, nt, :],
                                              in_=xlr_ps[:, HC:2 * HC])
                        nc.sync.dma_start(
                            out=ag1_in[gi][(nt - g0) * P:(nt - g0 + 1) * P, :],
                            in_=xl_sb[:])
                    if PHASES >= 2:
                        ag_chunk(ag1_in[gi], stg1[gi], tbl1, g0, g1)

            # ============ phase B: layer-1 edges ============
            if PHASES >= 2:
                grp_of_nt = {}
                for gi, (g0, g1) in enumerate(groups):
                    for nt in range(g0, g1):
                        grp_of_nt[nt] = gi
                with (tc.tile_pool(name="sbB", bufs=2) as sb,
                      tc.tile_pool(name="gbB", bufs=2) as gb,
                      tc.tile_pool(name="psB", bufs=2, space="PSUM") as ps):
                    for b0, b1 in batches:
                        nlo_b = olo[b1] - olo[b0]
                        nhi_b = ohi[b1] - ohi[b0]
                        glo = gb.tile([P, BKM, HC], bf, tag="glo")
                        ghi = gb.tile([P, BKM, HC], bf, tag="ghi")
                        qb = 0 if True else (2 * (b0 % 2)) % 4
                        g1i = nc.gpsimd.dma_gather(
                            glo[:, 0:nlo_b, :], tbl1[0:HALF, :],
                            gidx_lo_sb[:, olo[b0] * 8:olo[b1] * 8],
                            nlo_b * P, regs[nlo_b], HC, queue_num=qb)
                        g2i = nc.gpsimd.dma_gather(
                            ghi[:, 0:nhi_b, :], tbl1[HALF:TBL, :],
                            gidx_hi_sb[:, ohi[b0] * 8:ohi[b1] * 8],
                            nhi_b * P, regs[nhi_b], HC, queue_num=qb)
                        _add_dep_helper(g1i.ins, lib.ins, sync=False, reason="lib")
                        _add_dep_helper(g2i.ins, lib.ins, sync=False, reason="lib")

                        for nt in range(b0, b1):
                            T_ = T[nt]; Klo_ = Klo[nt]; Khi_ = Khi[nt]
                            blo = olo[nt] - olo[b0]
                            bhi = ohi[nt] - ohi[b0]
                            ohb_t = sb.tile([P, 2 * TM * P], bf, tag="ohb")
                            nc.sync.dma_start(
                                out=ohb_t[:, 0:2 * T_ * P],
                                in_=ohb_d[:, 2 * od[nt] * P:2 * (od[nt] + T_) * P])
                            ohne = ohb_t[:, 0:T_ * P]
                            ohen = ohb_t[:, T_ * P:2 * T_ * P]

                            acc = ps.tile([P, HC + H], fp32, tag="acc",
                                          space="PSUM", bufs=1)
                            msg = sb.tile([P, TM, HC + H], bf, tag="msg")
                            for t0, K, gx, gb0 in [(0, Klo_, glo, blo),
                                                   (Klo_, Khi_, ghi, bhi)]:
                                t_sb = sb.tile([P, KM, HC], bf, tag="t")
                                for j in range(K):
                                    zj = ps.tile([P, HC], fp32, tag="z",
                                                 space="PSUM", bufs=4)
                                    nc.tensor.matmul(
                                        out=zj[:],
                                        lhsT=ohne[:, (t0 + j) * P:(t0 + j + 1) * P],
                                        rhs=xr1_all[:, nt, :],
                                        start=True, stop=False)
                                    nc.tensor.matmul(
                                        out=zj[:], lhsT=ident,
                                        rhs=gx[:, gb0 + j, :],
                                        start=False, stop=True)
                                    nc.scalar.activation(out=t_sb[:, j, :],
                                                         in_=zj[:],
                                                         func=AF.Prelu, alpha=alpha)
                                ta = sb.tile([P, KM, HC], bf, tag="ta")
                                nc.vector.tensor_tensor(
                                    out=ta[:, 0:K, :], in0=t_sb[:, 0:K, :],
                                    in1=attBK[:, 0:K * HC].rearrange(
                                        "p (k c) -> p k c", k=K), op=OP.mult)
                                sc = sb.tile([P, KM * H], fp32, tag="sc")
                                nc.vector.tensor_reduce(
                                    out=sc[:, 0:K * H],
                                    in_=ta[:, 0:K, :].rearrange(
                                        "p k (h c) -> p (k h) c", h=H),
                                    axis=AX.X, op=OP.add)
                                ex = sb.tile([P, KM * H], bf, tag="ex")
                                nc.scalar.activation(out=ex[:, 0:K * H],
                                                     in_=sc[:, 0:K * H], func=AF.Exp)
                                nc.vector.tensor_tensor(
                                    out=msg[:, t0:t0 + K, 0:HC].rearrange(
                                        "p k (h c) -> p k h c", h=H),
                                    in0=gx[:, gb0:gb0 + K, :].rearrange(
                                        "p k (h c) -> p k h c", h=H),
                                    in1=ex[:, 0:K * H].rearrange(
                                        "p (k h) -> p k h", k=K)[:, :, :, None]
                                        .to_broadcast([P, K, H, CH]),
                                    op=OP.mult)
                                nc.vector.tensor_copy(
                                    out=msg[:, t0:t0 + K, HC:HC + H],
                                    in_=ex[:, 0:K * H].rearrange(
                                        "p (k h) -> p k h", k=K))
                                for j in range(K):
                                    nc.tensor.matmul(
                                        out=acc[:],
                                        lhsT=ohen[:, (t0 + j) * P:(t0 + j + 1) * P],
                                        rhs=msg[:, t0 + j, :],
                                        start=(t0 + j == 0), stop=(t0 + j == T_ - 1))

                            rec = sb.tile([P, H], fp32, tag="rec")
                            nc.vector.reciprocal(out=rec[:], in_=acc[:, HC:HC + H])
                            h1 = sb.tile([P, HC], fp32, tag="h1")
                            nc.vector.tensor_tensor(
                                out=h1[:].rearrange("p (h c) -> p h c", h=H),
                                in0=acc[:, 0:HC].rearrange("p (h c) -> p h c", h=H),
                                in1=rec[:, :, None].to_broadcast([P, H, CH]),
                                op=OP.mult)
                            if dims["add_b1"]:
                                nc.vector.tensor_tensor(out=h1[:], in0=h1[:],
                                                        in1=b1B, op=OP.add)
                            eh = sb.tile([P, HC], fp32, tag="eh")
                            nc.scalar.activation(out=eh[:], in_=h1[:], func=AF.Exp)
                            em = sb.tile([P, HC], fp32, tag="em")
                            nc.vector.tensor_scalar(
                                out=em[:], in0=eh[:], scalar1=1.0, scalar2=0.0,
                                op0=OP.subtract, op1=OP.min)
                            elu = sb.tile([P, HC], bf, tag="elu")
                            nc.vector.tensor_scalar(out=elu[:], in0=h1[:],
                                                    scalar1=0.0, scalar2=None,
                                                    op0=OP.max)
                            nc.vector.tensor_tensor(out=elu[:], in0=elu[:],
                                                    in1=em[:], op=OP.add)

                            tail_ps = ps.tile([P, KH * P + 2 * CO], fp32,
                                              tag="tail", space="PSUM", bufs=1)
                            for k in range(KH):
                                nc.tensor.matmul(
                                    out=tail_ps[:, k * P:(k + 1) * P],
                                    lhsT=elu[:, k * P:(k + 1) * P],
                                    rhs=ident, start=True, stop=True)
                            hT_sb = sb.tile([P, KH, P], bf, tag="hTs")
                            nc.vector.tensor_copy(
                                out=hT_sb[:],
                                in_=tail_ps[:, 0:KH * P].rearrange(
                                    "p (k q) -> p k q", k=KH))
                            x2_ps = tail_ps[:, KH * P:KH * P + 2 * CO]
                            for k in range(KH):
                                nc.tensor.matmul(out=x2_ps, lhsT=hT_sb[:, k, :],
                                                 rhs=w2_sb[:, k, :],
                                                 start=(k == 0), stop=(k == KH - 1))
                            xl2_sb = sb.tile([P, CO], bf, tag="xl2s")
                            nc.vector.tensor_copy(out=xl2_sb[:], in_=x2_ps[:, 0:CO])
                            nc.vector.tensor_copy(out=xr2_all[:, nt, :],
                                                  in_=x2_ps[:, CO:2 * CO])
                            gi = grp_of_nt[nt]
                            g0_, _ = groups[gi]
                            nc.sync.dma_start(
                                out=ag2_in[gi][(nt - g0_) * P:(nt - g0_ + 1) * P,
                                               0:CO],
                                in_=xl2_sb[:])
                        if PHASES >= 3 and b1 == groups[grp_of_nt[b0]][1]:
                            gi = grp_of_nt[b0]
                            g0_, g1_ = groups[gi]
                            ag_chunk(ag2_in[gi], stg2[gi], tbl2, g0_, g1_)

            # ============ phase C: layer-2 edges ============
            if PHASES >= 3:
                with (tc.tile_pool(name="sbC", bufs=2) as sb,
                      tc.tile_pool(name="gbC", bufs=2) as gb,
                      tc.tile_pool(name="psC", bufs=2, space="PSUM") as ps):
                    for b0, b1 in batches:
                        nlo_b = olo[b1] - olo[b0]
                        nhi_b = ohi[b1] - ohi[b0]
                        g2lo = gb.tile([P, BKM, CO_PAD], bf, tag="g2lo")
                        g2hi = gb.tile([P, BKM, CO_PAD], bf, tag="g2hi")
                        qb = 0 if True else (2 * (b0 % 2)) % 4
                        g1i = nc.gpsimd.dma_gather(
                            g2lo[:, 0:nlo_b, :], tbl2[0:HALF, :],
                            gidx_lo_sb[:, olo[b0] * 8:olo[b1] * 8],
                            nlo_b * P, regs[nlo_b], CO_PAD, queue_num=qb)
                        g2i = nc.gpsimd.dma_gather(
                            g2hi[:, 0:nhi_b, :], tbl2[HALF:TBL, :],
                            gidx_hi_sb[:, ohi[b0] * 8:ohi[b1] * 8],
                            nhi_b * P, regs[nhi_b], CO_PAD, queue_num=qb)
                        _add_dep_helper(g1i.ins, lib.ins, sync=False, reason="lib")
                        _add_dep_helper(g2i.ins, lib.ins, sync=False, reason="lib")

                        for nt in range(b0, b1):
                            T_ = T[nt]; Klo_ = Klo[nt]; Khi_ = Khi[nt]
                            blo = olo[nt] - olo[b0]
                            bhi = ohi[nt] - ohi[b0]
                            ohb_t = sb.tile([P, 2 * TM * P], bf, tag="ohb2")
                            nc.sync.dma_start(
                                out=ohb_t[:, 0:2 * T_ * P],
                                in_=ohb_d[:, 2 * od[nt] * P:2 * (od[nt] + T_) * P])
                            ohne = ohb_t[:, 0:T_ * P]
                            ohen = ohb_t[:, T_ * P:2 * T_ * P]

                            acc2 = ps.tile([P, CO + 1], fp32, tag="acc2",
                                           space="PSUM", bufs=1)
                            msg2 = sb.tile([P, TM, CO + 1], bf, tag="msg2")
                            for t0, K, gx, gb0 in [(0, Klo_, g2lo, blo),
                                                   (Klo_, Khi_, g2hi, bhi)]:
                                t2 = sb.tile([P, KM, CO], bf, tag="t2")
                                for j in range(K):
                                    zj = ps.tile([P, CO], fp32, tag="z2",
                                                 space="PSUM", bufs=4)
                                    nc.tensor.matmul(
                                        out=zj[:],
                                        lhsT=ohne[:, (t0 + j) * P:(t0 + j + 1) * P],
                                        rhs=xr2_all[:, nt, :],
                                        start=True, stop=False)
                                    nc.tensor.matmul(
                                        out=zj[:], lhsT=ident,
                                        rhs=gx[:, gb0 + j, 0:CO],
                                        start=False, stop=True)
                                    nc.scalar.activation(out=t2[:, j, :],
                                                         in_=zj[:],
                                                         func=AF.Prelu, alpha=alpha)
                                ta2 = sb.tile([P, KM, CO], bf, tag="ta2")
                                nc.vector.tensor_tensor(
                                    out=ta2[:, 0:K, :], in0=t2[:, 0:K, :],
                                    in1=att2BK[:, 0:K * CO].rearrange(
                                        "p (k c) -> p k c", k=K), op=OP.mult)
                                sc2 = sb.tile([P, KM], fp32, tag="sc2")
                                nc.vector.tensor_reduce(
                                    out=sc2[:, 0:K], in_=ta2[:, 0:K, :],
                                    axis=AX.X, op=OP.add)
                                ex2 = sb.tile([P, KM], bf, tag="ex2")
                                nc.scalar.activation(out=ex2[:, 0:K],
                                                     in_=sc2[:, 0:K], func=AF.Exp)
                                nc.vector.tensor_tensor(
                                    out=msg2[:, t0:t0 + K, 0:CO],
                                    in0=gx[:, gb0:gb0 + K, 0:CO],
                                    in1=ex2[:, 0:K][:, :, None]
                                        .to_broadcast([P, K, CO]),
                                    op=OP.mult)
                                nc.vector.tensor_copy(
                                    out=msg2[:, t0:t0 + K, CO:CO + 1],
                                    in_=ex2[:, 0:K][:, :, None])
                                for j in range(K):
                                    nc.tensor.matmul(
                                        out=acc2[:],
                                        lhsT=ohen[:, (t0 + j) * P:(t0 + j + 1) * P],
                                        rhs=msg2[:, t0 + j, :],
                                        start=(t0 + j == 0), stop=(t0 + j == T_ - 1))

                            rec2 = sb.tile([P, 1], fp32, tag="rec2")
                            nc.vector.reciprocal(out=rec2[:], in_=acc2[:, CO:CO + 1])
                            h2 = sb.tile([P, CO], fp32, tag="h2")
                            nc.vector.tensor_scalar(out=h2[:], in0=acc2[:, 0:CO],
                                                    scalar1=rec2[:, 0:1],
                                                    scalar2=None, op0=OP.mult)
                            if dims["add_b2"]:
                                nc.vector.tensor_tensor(out=h2[:], in0=h2[:],
                                                        in1=b2B, op=OP.add)
                            nc.vector.tensor_copy(out=h2_all[:, nt, :], in_=h2[:])
                            nm = sb.tile([P, 1], fp32, tag="nm")
                            nc.vector.tensor_reduce(out=nm[:], in_=h2[:], axis=AX.X,
                                                    op=OP.max, negate=True)
                            esc = sb.tile([P, CO], fp32, tag="esc")
                            ssum = sb.tile([P, 1], fp32, tag="ssum")
                            nc.scalar.activation(out=esc[:], in_=h2[:], func=AF.Exp,
                                                 bias=nm[:, 0:1],
                                                 accum_out=ssum[:, 0:1])
                            lns = sb.tile([P, 1], fp32, tag="lns")
                            nc.scalar.activation(out=lns[:], in_=ssum[:], func=AF.Ln)
                            nc.vector.tensor_scalar(
                                out=ls_all[:, nt, :], in0=h2[:], scalar1=nm[:, 0:1],
                                scalar2=lns[:, 0:1], op0=OP.add, op1=OP.subtract)

            nc.sync.dma_start(out=h2_out.rearrange("(a p) d -> p a d", p=P),
                              in_=h2_all[:])
            nc.sync.dma_start(out=ls_out.rearrange("(a p) d -> p a d", p=P),
                              in_=ls_all[:])

    if post_passes:
        _br.generate_event_semaphores(nc)
        _br.codegen_inst_isa_subclasses(nc)
    return nc


# --------------------------------------------------------------------------
# entry point
# --------------------------------------------------------------------------

def kernel(x, edge_index, W1l, W1r, att1, b1, W2l, W2r, att2, b2):
    x = np.asarray(x, np.float32)
    edge_index = np.asarray(edge_index)
    W1l = np.asarray(W1l, np.float32); W1r = np.asarray(W1r, np.float32)
    att1 = np.asarray(att1, np.float32); b1 = np.asarray(b1, np.float32)
    W2l = np.asarray(W2l, np.float32); W2r = np.asarray(W2r, np.float32)
    att2 = np.asarray(att2, np.float32); b2 = np.asarray(b2, np.float32)

    N, DIN = x.shape
    E = edge_index.shape[1]
    H, CH = att1.shape
    HC = W1l.shape[1]
    CO = W2l.shape[1]

    key = (N, E, DIN, H, CH, HC, CO,
           int(np.abs(b1).max() > 0), int(np.abs(b2).max() > 0),
           hash(edge_index.tobytes()))
    if key in _plan_cache:
        pp, nc, dims = _plan_cache[key]
    else:
        pp = _preprocess(N, E, edge_index)
        dims = dict(DIN=DIN, HC=HC, H=H, CH=CH, CO=CO,
                    NPC=pp["NPC"], NT=pp["NT"], TBL=pp["TBL"],
                    Klo=pp["Klo"], Khi=pp["Khi"], T=pp["T"], KM=pp["KM"],
                    od=pp["od"], olo=pp["olo"], ohi=pp["ohi"],
                    OD=pp["OD"], OLO=pp["OLO"], OHI=pp["OHI"],
                    groups=pp["groups"],
                    add_b1=bool(np.abs(b1).max() > 0),
                    add_b2=bool(np.abs(b2).max() > 0))
        nc = _build_program(dims)
        _plan_cache[key] = (pp, nc, dims)

    NPC = pp["NPC"]; NT = pp["NT"]; KM = pp["KM"]
    KD = DIN // P
    bfdt = ml_dtypes.bfloat16

    # consts blob: ident | attBK | b1B | att2BK | b2B
    ident = np.eye(P, dtype=np.float32)
    attBK = np.broadcast_to(
        np.tile(att1.reshape(HC), KM)[None, :], (P, KM * HC))
    b1B = np.broadcast_to(b1.reshape(1, HC), (P, HC))
    att2BK = np.broadcast_to(
        np.tile(att2.reshape(CO), KM)[None, :], (P, KM * CO))
    b2B = np.broadcast_to(b2.reshape(1, CO), (P, CO))
    consts = np.concatenate([ident, attBK, b1B, att2BK, b2B],
                            axis=1).astype(bfdt)
    constf = np.full((P, 1), NEG_SLOPE, np.float32)
    w1cat = np.concatenate([W1l, W1r], axis=1).astype(bfdt)
    w2cat = np.concatenate([W2l, W2r], axis=1).astype(bfdt)

    in_maps = []
    for c in range(NC):
        xkc = np.zeros((NPC, DIN), np.float32)
        sel = pp["node_order"][c]
        real = sel >= 0
        xkc[real] = x[sel[real]]
        # [p, nt, k, q] = xkc[nt*P+q, k*P+p]
        xkT = np.ascontiguousarray(
            xkc.reshape(NT, P, KD, P).transpose(3, 0, 2, 1)
        ).reshape(P, NT * KD * P).astype(bfdt)
        in_maps.append(dict(
            xkT=xkT, w1=w1cat, w2=w2cat, consts=consts, constf=constf,
            gidx_lo=np.ascontiguousarray(pp["gidx_lo"][c]),
            gidx_hi=np.ascontiguousarray(pp["gidx_hi"][c]),
            ohb=np.ascontiguousarray(pp["ohb"][c]),
        ))

    from concourse.bass_utils import run_bass_kernel_spmd
    res = run_bass_kernel_spmd(nc, in_maps, core_ids=list(range(NC)))

    h = np.empty((N, CO), np.float32)
    ls = np.empty((N, CO), np.float32)
    r_core = pp["core_of"]
    r_loc = pp["local_of"]
    for c in range(NC):
        m = r_core == c
        h[m] = res.results[c]["h2o"][r_loc[m]]
        ls[m] = res.results[c]["lso"][r_loc[m]]
    return h, ls


# revision 33
# speedup vs baseline: 1.9425x; 1.0096x over previous
"""2-layer GATv2 (PyG GATv2Conv semantics) on 8 Trainium2 NeuronCores.

Strategy (v2):
  - Nodes sharded across 8 cores; per-core greedy 2D packing balances each
    destination tile's lo/hi in-edge counts (lo = src owned by cores 0-3).
  - x is shipped pre-transposed; layer-1 projections are 6 accumulating
    matmuls per node tile with a combined [W1l|W1r] moving operand.
  - xl tables AllGathered HBM->HBM in 4 row-chunks overlapped with compute.
  - Per destination tile, incoming-edge source rows are fetched with
    dma_gather (int16 idx; two table halves).  The dst->edge one-hot
    (oh_ne, [dst, edge]) is precomputed on host and streamed from HBM;
    the edge->dst one-hot (oh_en) is one batched DVE is_equal per half.
  - Per half (K edge tiles): K z-matmuls (xr broadcast), one batched DVE
    add (z+xl), 2-op leaky, att mult, reduce, exp, msg mult -- all batched
    over the half's K*128 edges -- then K scatter matmuls accumulate
    numerator+denominator in PSUM.
  - Softmax skips max-subtraction (scores O(1)).
  - Layer 2 (heads=1, 16 ch) repeats the edge structure on a 256B-row table.
  - log_softmax on ACT/DVE per node tile.

kernel(**inputs) takes FULL inputs, returns FULL outputs.
"""

import os
import sys

if "/opt/trn_rl_repo" not in sys.path:
    sys.path.insert(0, "/opt/trn_rl_repo")

import numpy as np
import ml_dtypes

NC = 8          # cores
P = 128         # partitions
NEG_SLOPE = 0.2
NGRP = 2        # AllGather chunks

_plan_cache = {}


# --------------------------------------------------------------------------
# host-side graph preprocessing
# --------------------------------------------------------------------------

def _snake(order, nbins):
    n = len(order)
    ids = np.arange(n)
    round_ = ids // nbins
    pos = ids % nbins
    b = np.where(round_ % 2 == 0, pos, nbins - 1 - pos)
    out = np.empty(n, np.int64)
    out[:] = b
    return out


def _preprocess(N, E, edge_index):
    NPC = ((N + NC - 1) // NC + P - 1) // P * P    # padded nodes per core
    NT = NPC // P
    TBL = NC * NPC
    assert TBL // 2 < 32768, "table half must fit int16 row indices"

    src = np.concatenate([edge_index[0].astype(np.int64), np.arange(N)])
    dst = np.concatenate([edge_index[1].astype(np.int64), np.arange(N)])
    deg = np.bincount(dst, minlength=N)

    # --- group assignment (fixes each edge's table half), then cores within
    # each group balanced by OWN-group in-degree (the heavy, self-loop half)
    order = np.argsort(-deg, kind="stable")
    grp_of = np.empty(N, np.int64)
    grp_of[order] = _snake(order, 2)
    e_own = grp_of[src] == grp_of[dst]
    deg_own = np.bincount(dst[e_own], minlength=N)
    core_of = np.empty(N, np.int64)
    half_nc = NC // 2
    for g in range(2):
        nodes_g = np.where(grp_of == g)[0]
        og = nodes_g[np.argsort(-deg_own[nodes_g], kind="stable")]
        core_of[og] = g * half_nc + _snake(og, half_nc)

    lo_src = core_of[src] < NC // 2                # which table half each edge reads
    deg_lo = np.bincount(dst[lo_src], minlength=N)
    deg_hi = deg - deg_lo

    # --- per-core greedy 2D packing: balance (lo, hi) in-edge sums per tile
    slot_of = np.empty(N, np.int64)
    tile_of = np.empty(N, np.int64)
    nlo = np.zeros((NC, NT), np.int64)   # per-tile lo in-edges (incl. pad fakes)
    nhi = np.zeros((NC, NT), np.int64)
    cnt_ct = np.zeros((NC, NT), np.int64)
    for c in range(NC):
        heavy_is_lo = c < NC // 2        # own-group half carries the self-loops
        dh = deg_lo if heavy_is_lo else deg_hi
        dl = deg_hi if heavy_is_lo else deg_lo
        nodes = np.where(core_of == c)[0]
        nodes = nodes[np.argsort(-(dh[nodes] * 64 + dl[nodes]), kind="stable")]
        hv = np.zeros(NT, np.int64)
        lt = np.zeros(NT, np.int64)
        cnt = np.zeros(NT, np.int64)
        for v in nodes:
            cost = (hv + dh[v]).astype(np.float64) \
                + 0.02 * (lt + dl[v]) + 1e-4 * cnt \
                + 1e6 * np.maximum(lt + dl[v] - 5 * P, 0)
            cost[cnt >= P] = 1e18
            t = int(np.argmin(cost))
            tile_of[v] = t
            slot_of[v] = cnt[t]
            cnt[t] += 1
            hv[t] += dh[v]
            lt[t] += dl[v]
        lt += P - cnt                    # pad slots: one fake edge, light half
        if heavy_is_lo:
            nlo[c], nhi[c] = hv, lt
        else:
            nlo[c], nhi[c] = lt, hv
        cnt_ct[c] = cnt

    # --- per-core rank permutation so heavy tiles align across cores
    kl = (nlo + P - 1) // P
    kh = (nhi + P - 1) // P
    perm = np.empty((NC, NT), np.int64)     # rank -> old tile
    for c in range(NC):
        key = (kl[c] + kh[c]) + 1e-3 * kl[c] + 1e-9 * (nlo[c] + nhi[c])
        perm[c] = np.argsort(-key, kind="stable")

    Klo = [int(max(kl[c, perm[c, r]] for c in range(NC))) for r in range(NT)]
    Khi = [int(max(kh[c, perm[c, r]] for c in range(NC))) for r in range(NT)]
    T = [Klo[r] + Khi[r] for r in range(NT)]
    KM = max(max(Klo), max(Khi))

    rank_of = np.empty((NC, NT), np.int64)  # old tile -> rank
    for c in range(NC):
        rank_of[c, perm[c]] = np.arange(NT)
    local_of = rank_of[core_of, tile_of] * P + slot_of
    r_of = core_of * NPC + local_of         # final global table row of each node

    # --- per (core, old-tile) edge lists split by half (final r_of values)
    e_core = core_of[dst]
    e_tile = tile_of[dst]
    e_slot = slot_of[dst]
    lists_lo = {}
    lists_hi = {}
    for c in range(NC):
        m_c = e_core == c
        for tl in range(NT):
            m = m_c & (e_tile == tl)
            ml = m & lo_src
            mh = m & ~lo_src
            lists_lo[(c, tl)] = (r_of[src[ml]], e_slot[ml])
            lists_hi[(c, tl)] = (r_of[src[mh]] - TBL // 2, e_slot[mh])
            # fake self-edges for empty (padding) node slots so denominators
            # stay nonzero (their outputs are discarded by the host); they
            # live in the light half (matches the greedy's accounting)
            npad = P - cnt_ct[c, tl]
            if npad:
                pads = np.arange(P - npad, P)
                key = (c, tl)
                tgt = lists_hi if c < NC // 2 else lists_lo
                a, b = tgt[key]
                tgt[key] = (np.concatenate([a, np.zeros(npad, np.int64)]),
                            np.concatenate([b, pads]))

    # offsets
    od = np.concatenate([[0], np.cumsum(T)]).astype(np.int64)       # drel/ohne cols
    olo = np.concatenate([[0], np.cumsum(Klo)]).astype(np.int64)    # gidx_lo tiles
    ohi = np.concatenate([[0], np.cumsum(Khi)]).astype(np.int64)
    OD = int(od[-1]); OLO = int(olo[-1]); OHI = int(ohi[-1])

    def pack_idx(flat):
        n = len(flat)
        s = (n + 15) // 16
        arr = np.zeros(s * 16, np.int16)
        arr[:n] = flat
        block = arr.reshape(s, 16).T
        return np.tile(block, (8, 1))

    gidx_lo = np.zeros((NC, P, OLO * 8), np.int16)
    gidx_hi = np.zeros((NC, P, OHI * 8), np.int16)
    drel = np.full((NC, P, OD), -1.0, np.float32)
    for c in range(NC):
        for r in range(NT):
            tl = perm[c, r]
            for half, (K, off8, dcol0, lst) in enumerate([
                    (Klo[r], olo[r], od[r], lists_lo[(c, tl)]),
                    (Khi[r], ohi[r], od[r] + Klo[r], lists_hi[(c, tl)])]):
                rows, slots = lst
                # re-slot: dst slots were computed pre-permutation; slot within
                # tile is unchanged (rank remap keeps slot % P)
                n = len(rows)
                flat = np.zeros(K * P, np.int64)
                flat[:n] = rows
                g = gidx_hi if half else gidx_lo
                g[c, :, off8 * 8:(off8 + K) * 8] = pack_idx(flat)
                dr = np.full(K * P, -1.0, np.float32)
                dr[:n] = slots
                drel[c, :, dcol0:dcol0 + K] = dr.reshape(K, P).T

    node_order = np.full((NC, NPC), -1, np.int64)  # local row -> global node id
    for c in range(NC):
        nodes = np.where(core_of == c)[0]
        node_order[c, local_of[nodes]] = nodes

    # per tile col: [ohne (dst-major [d, e]) | ohen (edge-major [e, d])]
    ar = np.arange(P, dtype=np.float32)
    ohne = (ar[None, :, None, None]
            == drel.transpose(0, 2, 1)[:, None, :, :])   # [NC, P(d), OD, P(e)]
    ohen = (drel[:, :, :, None] == ar[None, None, None, :])  # [NC, P(e), OD, P(d)]
    ohb = np.empty((NC, P, 2 * OD * P), ml_dtypes.bfloat16)
    for r in range(NT):
        o0, o1 = int(od[r]), int(od[r + 1])
        t_ = o1 - o0
        ohb[:, :, 2 * o0 * P:(2 * o0 + t_) * P] = \
            ohne[:, :, o0:o1, :].reshape(NC, P, t_ * P)
        ohb[:, :, (2 * o0 + t_) * P:2 * o1 * P] = \
            ohen[:, :, o0:o1, :].reshape(NC, P, t_ * P)

    # AllGather groups (tile ranks)
    gsz = (NT + NGRP - 1) // NGRP
    groups = [(g * gsz, min((g + 1) * gsz, NT)) for g in range(NGRP)]
    groups = [(a, b) for a, b in groups if b > a]

    return dict(NPC=NPC, NT=NT, TBL=TBL, Klo=Klo, Khi=Khi, T=T, KM=KM,
                od=od.tolist(), olo=olo.tolist(), ohi=ohi.tolist(),
                OD=OD, OLO=OLO, OHI=OHI, groups=groups,
                gidx_lo=gidx_lo, gidx_hi=gidx_hi, ohb=ohb,
                node_order=node_order, core_of=core_of, local_of=local_of)


# --------------------------------------------------------------------------
# bass program
# --------------------------------------------------------------------------

def _build_program(dims, post_passes=True):
    PHASES = int(os.environ.get("GAT_PHASES", "3"))
    SHARED = os.environ.get("GAT_SHARED", "0") == "1"
    GB = int(os.environ.get("GAT_GB", "1"))        # gather batch (node tiles)
    import concourse.bass as bass
    import concourse.mybir as mybir
    import concourse.tile as tile
    from concourse import library_config
    from concourse.bass import _add_dep_helper
    import bass_rust as _br

    fp32 = mybir.dt.float32
    bf = mybir.dt.bfloat16
    i16 = mybir.dt.int16
    AX = mybir.AxisListType
    OP = mybir.AluOpType
    AF = mybir.ActivationFunctionType

    DIN = dims["DIN"]; HC = dims["HC"]; H = dims["H"]; CH = dims["CH"]
    CO = dims["CO"]
    NPC = dims["NPC"]; NT = dims["NT"]; TBL = dims["TBL"]
    Klo = dims["Klo"]; Khi = dims["Khi"]; T = dims["T"]; KM = dims["KM"]
    od = dims["od"]; olo = dims["olo"]; ohi = dims["ohi"]
    OD = dims["OD"]; OLO = dims["OLO"]; OHI = dims["OHI"]
    groups = dims["groups"]
    KD = DIN // P
    KH = HC // P
    CO_PAD = 128
    HALF = TBL // 2
    TM = max(T)
    addr_space = "Shared" if SHARED else "Local"

    # gather batches: consecutive ranks within each AG group, <= GB tiles
    batches = []        # (nt0, nt1)
    for g0, g1 in groups:
        nt = g0
        while nt < g1:
            batches.append((nt, min(nt + GB, g1)))
            nt = batches[-1][1]
    BKM = max(max(olo[b1] - olo[b0], ohi[b1] - ohi[b0]) for b0, b1 in batches)

    nc = bass.Bass(num_devices=NC, num_swdge_queues=4,
                   dynamic_dma_scratch_size=int(os.environ.get("GAT_DDS", "16384")))

    xkT_d = nc.dram_tensor("xkT", [P, NT * KD * P], bf, kind="ExternalInput")
    w1_d = nc.dram_tensor("w1", [DIN, 2 * HC], bf, kind="ExternalInput")
    w2_d = nc.dram_tensor("w2", [HC, 2 * CO], bf, kind="ExternalInput")
    CCOLS = KM * HC + HC + KM * CO + CO + P
    consts = nc.dram_tensor("consts", [P, CCOLS], bf, kind="ExternalInput")
    constf = nc.dram_tensor("constf", [P, 1], fp32, kind="ExternalInput")
    gidx_lo_d = nc.dram_tensor("gidx_lo", [P, OLO * 8], i16, kind="ExternalInput")
    gidx_hi_d = nc.dram_tensor("gidx_hi", [P, OHI * 8], i16, kind="ExternalInput")
    ohb_d = nc.dram_tensor("ohb", [P, 2 * OD * P], bf, kind="ExternalInput")
    h2_out = nc.dram_tensor("h2o", [NPC, CO], fp32, kind="ExternalOutput")
    ls_out = nc.dram_tensor("lso", [NPC, CO], fp32, kind="ExternalOutput")

    with tile.TileContext(nc) as tc:
        with (
            tc.tile_pool(name="dram", bufs=1, space="DRAM") as dram,
            tc.tile_pool(name="cst", bufs=1) as cst,
        ):
            lib = nc.gpsimd.load_library(library_config.mlp)
            regs = {}
            for b0, b1 in batches:
                for n in (olo[b1] - olo[b0], ohi[b1] - ohi[b0]):
                    if n not in regs:
                        regs[n] = nc.gpsimd.to_reg(n * P)

            ctile = cst.tile([P, CCOLS], bf)
            nc.sync.dma_start(out=ctile[:], in_=consts[:])
            cftile = cst.tile([P, 1], fp32)
            nc.sync.dma_start(out=cftile[:], in_=constf[:])
            o = 0
            ident = ctile[:, o:o + P]; o += P
            attBK = ctile[:, o:o + KM * HC]; o += KM * HC
            b1B = ctile[:, o:o + HC]; o += HC
            att2BK = ctile[:, o:o + KM * CO]; o += KM * CO
            b2B = ctile[:, o:o + CO]; o += CO
            alpha = cftile[:, 0:1]

            w1_sb = cst.tile([P, KD, 2 * HC], bf)
            nc.sync.dma_start(out=w1_sb[:], in_=w1_d.rearrange("(k p) c -> p k c", p=P))
            w2_sb = cst.tile([P, KH, 2 * CO], bf)
            nc.sync.dma_start(out=w2_sb[:], in_=w2_d.rearrange("(k p) c -> p k c", p=P))

            gidx_lo_sb = cst.tile([P, OLO * 8], i16)
            nc.sync.dma_start(out=gidx_lo_sb[:], in_=gidx_lo_d[:])
            gidx_hi_sb = cst.tile([P, OHI * 8], i16)
            nc.sync.dma_start(out=gidx_hi_sb[:], in_=gidx_hi_d[:])

            xr1_all = cst.tile([P, NT, HC], bf)
            xr2_all = cst.tile([P, NT, CO], bf)
            nc.vector.memset(xr2_all[:], 0.0)

            tbl1 = dram.tile([TBL, HC], bf)
            tbl2 = dram.tile([TBL, CO_PAD], bf)
            ag1_in = {}
            ag2_in = {}
            stg1 = {}
            stg2 = {}
            for gi, (g0, g1) in enumerate(groups):
                rows = (g1 - g0) * P
                ag1_in[gi] = dram.tile([rows, HC], bf, name=f"ag1i_{gi}")
                ag2_in[gi] = dram.tile([rows, CO_PAD], bf, name=f"ag2i_{gi}")
                stg1[gi] = dram.tile([NC * rows, HC], bf,
                                     addr_space=addr_space, name=f"stg1_{gi}")
                stg2[gi] = dram.tile([NC * rows, CO_PAD], bf,
                                     addr_space=addr_space, name=f"stg2_{gi}")

            def ag_chunk(src, stage, dst, g0, g1):
                nc.gpsimd.collective_compute(
                    "AllGather", mybir.AluOpType.bypass,
                    replica_groups=[list(range(NC))],
                    ins=[src[:].opt()],
                    outs=[stage[:].opt()],
                )
                nc.scalar.dma_start(
                    out=dst[:].rearrange("(c n) h -> c n h", c=NC)
                        [:, g0 * P:g1 * P, :],
                    in_=stage[:].rearrange("(c n) h -> c n h", c=NC))

            # ============ phase A: layer-1 projections ============
            with (tc.tile_pool(name="sbA", bufs=3) as sb,
                  tc.tile_pool(name="psA", bufs=2, space="PSUM") as ps):
                for gi, (g0, g1) in enumerate(groups):
                    for nt in range(g0, g1):
                        xt = sb.tile([P, KD, P], bf, tag="xt")
                        nc.sync.dma_start(
                            out=xt[:],
                            in_=xkT_d[:, nt * KD * P:(nt + 1) * KD * P])
                        xlr_ps = ps.tile([P, 2 * HC], fp32, tag="mm", space="PSUM")
                        for k in range(KD):
                            nc.tensor.matmul(out=xlr_ps[:], lhsT=xt[:, k, :],
                                             rhs=w1_sb[:, k, :],
                                             start=(k == 0), stop=(k == KD - 1))
                        xl_sb = sb.tile([P, HC], bf, tag="xls")
                        nc.vector.tensor_copy(out=xl_sb[:], in_=xlr_ps[:, 0:HC])
                        nc.vector.tensor_copy(out=xr1_all[:, nt, :],
                                              in_=xlr_ps[:, HC:2 * HC])
                        nc.sync.dma_start(
                            out=ag1_in[gi][(nt - g0) * P:(nt - g0 + 1) * P, :],
                            in_=xl_sb[:])
                    if PHASES >= 2:
                        ag_chunk(ag1_in[gi], stg1[gi], tbl1, g0, g1)

            # ============ phase B: layer-1 edges ============
            if PHASES >= 2:
                grp_of_nt = {}
                for gi, (g0, g1) in enumerate(groups):
                    for nt in range(g0, g1):
                        grp_of_nt[nt] = gi
                with (tc.tile_pool(name="sbB", bufs=2) as sb,
                      tc.tile_pool(name="gbB", bufs=3) as gb,
                      tc.tile_pool(name="psB", bufs=2, space="PSUM") as ps):
                    for b0, b1 in batches:
                        nlo_b = olo[b1] - olo[b0]
                        nhi_b = ohi[b1] - ohi[b0]
                        glo = gb.tile([P, BKM, HC], bf, tag="glo")
                        ghi = gb.tile([P, BKM, HC], bf, tag="ghi")
                        qb = 0 if True else (2 * (b0 % 2)) % 4
                        g1i = nc.gpsimd.dma_gather(
                            glo[:, 0:nlo_b, :], tbl1[0:HALF, :],
                            gidx_lo_sb[:, olo[b0] * 8:olo[b1] * 8],
                            nlo_b * P, regs[nlo_b], HC, queue_num=qb)
                        g2i = nc.gpsimd.dma_gather(
                            ghi[:, 0:nhi_b, :], tbl1[HALF:TBL, :],
                            gidx_hi_sb[:, ohi[b0] * 8:ohi[b1] * 8],
                            nhi_b * P, regs[nhi_b], HC, queue_num=qb)
                        _add_dep_helper(g1i.ins, lib.ins, sync=False, reason="lib")
                        _add_dep_helper(g2i.ins, lib.ins, sync=False, reason="lib")

                        for nt in range(b0, b1):
                            T_ = T[nt]; Klo_ = Klo[nt]; Khi_ = Khi[nt]
                            blo = olo[nt] - olo[b0]
                            bhi = ohi[nt] - ohi[b0]
                            ohb_t = sb.tile([P, 2 * TM * P], bf, tag="ohb")
                            nc.sync.dma_start(
                                out=ohb_t[:, 0:2 * T_ * P],
                                in_=ohb_d[:, 2 * od[nt] * P:2 * (od[nt] + T_) * P])
                            ohne = ohb_t[:, 0:T_ * P]
                            ohen = ohb_t[:, T_ * P:2 * T_ * P]

                            acc = ps.tile([P, HC + H], fp32, tag="acc",
                                          space="PSUM", bufs=2)
                            msg = sb.tile([P, TM, HC + H], bf, tag="msg")
                            for t0, K, gx, gb0 in [(0, Klo_, glo, blo),
                                                   (Klo_, Khi_, ghi, bhi)]:
                                t_sb = sb.tile([P, KM, HC], bf, tag="t")
                                for j in range(K):
                                    zj = ps.tile([P, HC], fp32, tag="z",
                                                 space="PSUM", bufs=4)
                                    nc.tensor.matmul(
                                        out=zj[:],
                                        lhsT=ohne[:, (t0 + j) * P:(t0 + j + 1) * P],
                                        rhs=xr1_all[:, nt, :],
                                        start=True, stop=False)
                                    nc.tensor.matmul(
                                        out=zj[:], lhsT=ident,
                                        rhs=gx[:, gb0 + j, :],
                                        start=False, stop=True)
                                    nc.scalar.activation(out=t_sb[:, j, :],
                                                         in_=zj[:],
                                                         func=AF.Prelu, alpha=alpha)
                                ta = sb.tile([P, KM, HC], bf, tag="ta")
                                nc.vector.tensor_tensor(
                                    out=ta[:, 0:K, :], in0=t_sb[:, 0:K, :],
                                    in1=attBK[:, 0:K * HC].rearrange(
                                        "p (k c) -> p k c", k=K), op=OP.mult)
                                sc = sb.tile([P, KM * H], fp32, tag="sc")
                                nc.vector.tensor_reduce(
                                    out=sc[:, 0:K * H],
                                    in_=ta[:, 0:K, :].rearrange(
                                        "p k (h c) -> p (k h) c", h=H),
                                    axis=AX.X, op=OP.add)
                                nc.scalar.activation(
                                    out=msg[:, t0:t0 + K, HC:HC + H],
                                    in_=sc[:, 0:K * H], func=AF.Exp)
                                nc.vector.tensor_tensor(
                                    out=msg[:, t0:t0 + K, 0:HC].rearrange(
                                        "p k (h c) -> p k h c", h=H),
                                    in0=gx[:, gb0:gb0 + K, :].rearrange(
                                        "p k (h c) -> p k h c", h=H),
                                    in1=msg[:, t0:t0 + K, HC:HC + H]
                                        [:, :, :, None]
                                        .to_broadcast([P, K, H, CH]),
                                    op=OP.mult)
                                for j in range(K):
                                    nc.tensor.matmul(
                                        out=acc[:],
                                        lhsT=ohen[:, (t0 + j) * P:(t0 + j + 1) * P],
                                        rhs=msg[:, t0 + j, :],
                                        start=(t0 + j == 0), stop=(t0 + j == T_ - 1))

                            rec = sb.tile([P, H], fp32, tag="rec")
                            nc.vector.reciprocal(out=rec[:], in_=acc[:, HC:HC + H])
                            h1 = sb.tile([P, HC], fp32, tag="h1")
                            nc.vector.tensor_tensor(
                                out=h1[:].rearrange("p (h c) -> p h c", h=H),
                                in0=acc[:, 0:HC].rearrange("p (h c) -> p h c", h=H),
                                in1=rec[:, :, None].to_broadcast([P, H, CH]),
                                op=OP.mult)
                            if dims["add_b1"]:
                                nc.vector.tensor_tensor(out=h1[:], in0=h1[:],
                                                        in1=b1B, op=OP.add)
                            eh = sb.tile([P, HC], fp32, tag="eh")
                            nc.scalar.activation(out=eh[:], in_=h1[:], func=AF.Exp)
                            em = sb.tile([P, HC], fp32, tag="em")
                            nc.vector.tensor_scalar(
                                out=em[:], in0=eh[:], scalar1=1.0, scalar2=0.0,
                                op0=OP.subtract, op1=OP.min)
                            elu = sb.tile([P, HC], bf, tag="elu")
                            nc.vector.tensor_scalar(out=elu[:], in0=h1[:],
                                                    scalar1=0.0, scalar2=None,
                                                    op0=OP.max)
                            nc.vector.tensor_tensor(out=elu[:], in0=elu[:],
                                                    in1=em[:], op=OP.add)

                            tail_ps = ps.tile([P, KH * P + 2 * CO], fp32,
                                              tag="tail", space="PSUM", bufs=2)
                            for k in range(KH):
                                nc.tensor.matmul(
                                    out=tail_ps[:, k * P:(k + 1) * P],
                                    lhsT=elu[:, k * P:(k + 1) * P],
                                    rhs=ident, start=True, stop=True)
                            hT_sb = sb.tile([P, KH, P], bf, tag="hTs")
                            nc.vector.tensor_copy(
                                out=hT_sb[:],
                                in_=tail_ps[:, 0:KH * P].rearrange(
                                    "p (k q) -> p k q", k=KH))
                            x2_ps = tail_ps[:, KH * P:KH * P + 2 * CO]
                            for k in range(KH):
                                nc.tensor.matmul(out=x2_ps, lhsT=hT_sb[:, k, :],
                                                 rhs=w2_sb[:, k, :],
                                                 start=(k == 0), stop=(k == KH - 1))
                            xl2_sb = sb.tile([P, CO], bf, tag="xl2s")
                            nc.vector.tensor_copy(out=xl2_sb[:], in_=x2_ps[:, 0:CO])
                            nc.vector.tensor_copy(out=xr2_all[:, nt, :],
                                                  in_=x2_ps[:, CO:2 * CO])
                            gi = grp_of_nt[nt]
                            g0_, _ = groups[gi]
                            nc.sync.dma_start(
                                out=ag2_in[gi][(nt - g0_) * P:(nt - g0_ + 1) * P,
                                               0:CO],
                                in_=xl2_sb[:])
                        if PHASES >= 3 and b1 == groups[grp_of_nt[b0]][1]:
                            gi = grp_of_nt[b0]
                            g0_, g1_ = groups[gi]
                            ag_chunk(ag2_in[gi], stg2[gi], tbl2, g0_, g1_)

            # ============ phase C: layer-2 edges ============
            if PHASES >= 3:
                with (tc.tile_pool(name="sbC", bufs=2) as sb,
                      tc.tile_pool(name="gbC", bufs=3) as gb,
                      tc.tile_pool(name="psC", bufs=2, space="PSUM") as ps):
                    for b0, b1 in batches:
                        nlo_b = olo[b1] - olo[b0]
                        nhi_b = ohi[b1] - ohi[b0]
                        g2lo = gb.tile([P, BKM, CO_PAD], bf, tag="g2lo")
                        g2hi = gb.tile([P, BKM, CO_PAD], bf, tag="g2hi")
                        qb = 0 if True else (2 * (b0 % 2)) % 4
                        g1i = nc.gpsimd.dma_gather(
                            g2lo[:, 0:nlo_b, :], tbl2[0:HALF, :],
                            gidx_lo_sb[:, olo[b0] * 8:olo[b1] * 8],
                            nlo_b * P, regs[nlo_b], CO_PAD, queue_num=qb)
                        g2i = nc.gpsimd.dma_gather(
                            g2hi[:, 0:nhi_b, :], tbl2[HALF:TBL, :],
                            gidx_hi_sb[:, ohi[b0] * 8:ohi[b1] * 8],
                            nhi_b * P, regs[nhi_b], CO_PAD, queue_num=qb)
                        _add_dep_helper(g1i.ins, lib.ins, sync=False, reason="lib")
                        _add_dep_helper(g2i.ins, lib.ins, sync=False, reason="lib")

                        for nt in range(b0, b1):
                            T_ = T[nt]; Klo_ = Klo[nt]; Khi_ = Khi[nt]
                            blo = olo[nt] - olo[b0]
                            bhi = ohi[nt] - ohi[b0]
                            ohb_t = sb.tile([P, 2 * TM * P], bf, tag="ohb2")
                            nc.sync.dma_start(
                                out=ohb_t[:, 0:2 * T_ * P],
                                in_=ohb_d[:, 2 * od[nt] * P:2 * (od[nt] + T_) * P])
                            ohne = ohb_t[:, 0:T_ * P]
                            ohen = ohb_t[:, T_ * P:2 * T_ * P]

                            acc2 = ps.tile([P, CO + 1], fp32, tag="acc2",
                                           space="PSUM", bufs=2)
                            msg2 = sb.tile([P, TM, CO + 1], bf, tag="msg2")
                            for t0, K, gx, gb0 in [(0, Klo_, g2lo, blo),
                                                   (Klo_, Khi_, g2hi, bhi)]:
                                t2 = sb.tile([P, KM, CO], bf, tag="t2")
                                for j in range(K):
                                    zj = ps.tile([P, CO], fp32, tag="z2",
                                                 space="PSUM", bufs=4)
                                    nc.tensor.matmul(
                                        out=zj[:],
                                        lhsT=ohne[:, (t0 + j) * P:(t0 + j + 1) * P],
                                        rhs=xr2_all[:, nt, :],
                                        start=True, stop=False)
                                    nc.tensor.matmul(
                                        out=zj[:], lhsT=ident,
                                        rhs=gx[:, gb0 + j, 0:CO],
                                        start=False, stop=True)
                                    nc.scalar.activation(out=t2[:, j, :],
                                                         in_=zj[:],
                                                         func=AF.Prelu, alpha=alpha)
                                ta2 = sb.tile([P, KM, CO], bf, tag="ta2")
                                nc.vector.tensor_tensor(
                                    out=ta2[:, 0:K, :], in0=t2[:, 0:K, :],
                                    in1=att2BK[:, 0:K * CO].rearrange(
                                        "p (k c) -> p k c", k=K), op=OP.mult)
                                sc2 = sb.tile([P, KM], fp32, tag="sc2")
                                nc.vector.tensor_reduce(
                                    out=sc2[:, 0:K], in_=ta2[:, 0:K, :],
                                    axis=AX.X, op=OP.add)
                                nc.scalar.activation(
                                    out=msg2[:, t0:t0 + K, CO:CO + 1],
                                    in_=sc2[:, 0:K], func=AF.Exp)
                                nc.vector.tensor_tensor(
                                    out=msg2[:, t0:t0 + K, 0:CO],
                                    in0=gx[:, gb0:gb0 + K, 0:CO],
                                    in1=msg2[:, t0:t0 + K, CO:CO + 1]
                                        .to_broadcast([P, K, CO]),
                                    op=OP.mult)
                                for j in range(K):
                                    nc.tensor.matmul(
                                        out=acc2[:],
                                        lhsT=ohen[:, (t0 + j) * P:(t0 + j + 1) * P],
                                        rhs=msg2[:, t0 + j, :],
                                        start=(t0 + j == 0), stop=(t0 + j == T_ - 1))

                            rec2 = sb.tile([P, 1], fp32, tag="rec2")
                            nc.vector.reciprocal(out=rec2[:], in_=acc2[:, CO:CO + 1])
                            h2 = sb.tile([P, CO], fp32, tag="h2")
                            nc.vector.tensor_scalar(out=h2[:], in0=acc2[:, 0:CO],
                                                    scalar1=rec2[:, 0:1],
                                                    scalar2=None, op0=OP.mult)
                            if dims["add_b2"]:
                                nc.vector.tensor_tensor(out=h2[:], in0=h2[:],
                                                        in1=b2B, op=OP.add)
                            nc.sync.dma_start(
                                out=h2_out.rearrange("(a p) d -> p a d", p=P)
                                    [:, nt, :],
                                in_=h2[:])
                            nm = sb.tile([P, 1], fp32, tag="nm")
                            nc.vector.tensor_reduce(out=nm[:], in_=h2[:], axis=AX.X,
                                                    op=OP.max, negate=True)
                            esc = sb.tile([P, CO], fp32, tag="esc")
                            ssum = sb.tile([P, 1], fp32, tag="ssum")
                            nc.scalar.activation(out=esc[:], in_=h2[:], func=AF.Exp,
                                                 bias=nm[:, 0:1],
                                                 accum_out=ssum[:, 0:1])
                            lns = sb.tile([P, 1], fp32, tag="lns")
                            nc.scalar.activation(out=lns[:], in_=ssum[:], func=AF.Ln)
                            ls = sb.tile([P, CO], fp32, tag="ls")
                            nc.vector.tensor_scalar(
                                out=ls[:], in0=h2[:], scalar1=nm[:, 0:1],
                                scalar2=lns[:, 0:1], op0=OP.add, op1=OP.subtract)
                            nc.sync.dma_start(
                                out=ls_out.rearrange("(a p) d -> p a d", p=P)
                                    [:, nt, :],
                                in_=ls[:])



    if post_passes:
        _br.generate_event_semaphores(nc)
        _br.codegen_inst_isa_subclasses(nc)
    return nc


# --------------------------------------------------------------------------
# entry point
# --------------------------------------------------------------------------

def kernel(x, edge_index, W1l, W1r, att1, b1, W2l, W2r, att2, b2):
    x = np.asarray(x, np.float32)
    edge_index = np.asarray(edge_index)
    W1l = np.asarray(W1l, np.float32); W1r = np.asarray(W1r, np.float32)
    att1 = np.asarray(att1, np.float32); b1 = np.asarray(b1, np.float32)
    W2l = np.asarray(W2l, np.float32); W2r = np.asarray(W2r, np.float32)
    att2 = np.asarray(att2, np.float32); b2 = np.asarray(b2, np.float32)

    N, DIN = x.shape
    E = edge_index.shape[1]
    H, CH = att1.shape
    HC = W1l.shape[1]
    CO = W2l.shape[1]

    key = (N, E, DIN, H, CH, HC, CO,
           int(np.abs(b1).max() > 0), int(np.abs(b2).max() > 0),
           hash(edge_index.tobytes()))
    if key in _plan_cache:
        pp, nc, dims = _plan_cache[key]
    else:
        pp = _preprocess(N, E, edge_index)
        dims = dict(DIN=DIN, HC=HC, H=H, CH=CH, CO=CO,
                    NPC=pp["NPC"], NT=pp["NT"], TBL=pp["TBL"],
                    Klo=pp["Klo"], Khi=pp["Khi"], T=pp["T"], KM=pp["KM"],
                    od=pp["od"], olo=pp["olo"], ohi=pp["ohi"],
                    OD=pp["OD"], OLO=pp["OLO"], OHI=pp["OHI"],
                    groups=pp["groups"],
                    add_b1=bool(np.abs(b1).max() > 0),
                    add_b2=bool(np.abs(b2).max() > 0))
        nc = _build_program(dims)
        _plan_cache[key] = (pp, nc, dims)

    NPC = pp["NPC"]; NT = pp["NT"]; KM = pp["KM"]
    KD = DIN // P
    bfdt = ml_dtypes.bfloat16

    # consts blob: ident | attBK | b1B | att2BK | b2B
    ident = np.eye(P, dtype=np.float32)
    attBK = np.broadcast_to(
        np.tile(att1.reshape(HC), KM)[None, :], (P, KM * HC))
    b1B = np.broadcast_to(b1.reshape(1, HC), (P, HC))
    att2BK = np.broadcast_to(
        np.tile(att2.reshape(CO), KM)[None, :], (P, KM * CO))
    b2B = np.broadcast_to(b2.reshape(1, CO), (P, CO))
    consts = np.concatenate([ident, attBK, b1B, att2BK, b2B],
                            axis=1).astype(bfdt)
    constf = np.full((P, 1), NEG_SLOPE, np.float32)
    w1cat = np.concatenate([W1l, W1r], axis=1).astype(bfdt)
    w2cat = np.concatenate([W2l, W2r], axis=1).astype(bfdt)

    in_maps = []
    for c in range(NC):
        xkc = np.zeros((NPC, DIN), np.float32)
        sel = pp["node_order"][c]
        real = sel >= 0
        xkc[real] = x[sel[real]]
        # [p, nt, k, q] = xkc[nt*P+q, k*P+p]
        xkT = np.ascontiguousarray(
            xkc.reshape(NT, P, KD, P).transpose(3, 0, 2, 1)
        ).reshape(P, NT * KD * P).astype(bfdt)
        in_maps.append(dict(
            xkT=xkT, w1=w1cat, w2=w2cat, consts=consts, constf=constf,
            gidx_lo=np.ascontiguousarray(pp["gidx_lo"][c]),
            gidx_hi=np.ascontiguousarray(pp["gidx_hi"][c]),
            ohb=np.ascontiguousarray(pp["ohb"][c]),
        ))

    from concourse.bass_utils import run_bass_kernel_spmd
    res = run_bass_kernel_spmd(nc, in_maps, core_ids=list(range(NC)))

    h = np.empty((N, CO), np.float32)
    ls = np.empty((N, CO), np.float32)
    r_core = pp["core_of"]
    r_loc = pp["local_of"]
    for c in range(NC):
        m = r_core == c
        h[m] = res.results[c]["h2o"][r_loc[m]]
        ls[m] = res.results[c]["lso"][r_loc[m]]
    return h, ls


# revision 34
# speedup vs baseline: 2.0554x; 1.0581x over previous
"""2-layer GATv2 (PyG GATv2Conv semantics) on 8 Trainium2 NeuronCores.

Strategy (v2):
  - Nodes sharded across 8 cores; per-core greedy 2D packing balances each
    destination tile's lo/hi in-edge counts (lo = src owned by cores 0-3).
  - x is shipped pre-transposed; layer-1 projections are 6 accumulating
    matmuls per node tile with a combined [W1l|W1r] moving operand.
  - xl tables AllGathered HBM->HBM in 4 row-chunks overlapped with compute.
  - Per destination tile, incoming-edge source rows are fetched with
    dma_gather (int16 idx; two table halves).  The dst->edge one-hot
    (oh_ne, [dst, edge]) is precomputed on host and streamed from HBM;
    the edge->dst one-hot (oh_en) is one batched DVE is_equal per half.
  - Per half (K edge tiles): K z-matmuls (xr broadcast), one batched DVE
    add (z+xl), 2-op leaky, att mult, reduce, exp, msg mult -- all batched
    over the half's K*128 edges -- then K scatter matmuls accumulate
    numerator+denominator in PSUM.
  - Softmax skips max-subtraction (scores O(1)).
  - Layer 2 (heads=1, 16 ch) repeats the edge structure on a 256B-row table.
  - log_softmax on ACT/DVE per node tile.

kernel(**inputs) takes FULL inputs, returns FULL outputs.
"""

import os
import sys

if "/opt/trn_rl_repo" not in sys.path:
    sys.path.insert(0, "/opt/trn_rl_repo")

import numpy as np
import ml_dtypes

NC = 8          # cores
P = 128         # partitions
NEG_SLOPE = 0.2
NGRP = 2        # AllGather chunks

_plan_cache = {}


# --------------------------------------------------------------------------
# host-side graph preprocessing
# --------------------------------------------------------------------------

def _snake(order, nbins):
    n = len(order)
    ids = np.arange(n)
    round_ = ids // nbins
    pos = ids % nbins
    b = np.where(round_ % 2 == 0, pos, nbins - 1 - pos)
    out = np.empty(n, np.int64)
    out[:] = b
    return out


def _preprocess(N, E, edge_index):
    NPC = ((N + NC - 1) // NC + P - 1) // P * P    # padded nodes per core
    NT = NPC // P
    TBL = NC * NPC
    assert TBL // 2 < 32768, "table half must fit int16 row indices"

    src = np.concatenate([edge_index[0].astype(np.int64), np.arange(N)])
    dst = np.concatenate([edge_index[1].astype(np.int64), np.arange(N)])
    deg = np.bincount(dst, minlength=N)

    # --- group assignment (fixes each edge's table half), then cores within
    # each group balanced by OWN-group in-degree (the heavy, self-loop half)
    order = np.argsort(-deg, kind="stable")
    grp_of = np.empty(N, np.int64)
    grp_of[order] = _snake(order, 2)
    e_own = grp_of[src] == grp_of[dst]
    deg_own = np.bincount(dst[e_own], minlength=N)
    core_of = np.empty(N, np.int64)
    half_nc = NC // 2
    for g in range(2):
        nodes_g = np.where(grp_of == g)[0]
        og = nodes_g[np.argsort(-deg_own[nodes_g], kind="stable")]
        core_of[og] = g * half_nc + _snake(og, half_nc)

    lo_src = core_of[src] < NC // 2                # which table half each edge reads
    deg_lo = np.bincount(dst[lo_src], minlength=N)
    deg_hi = deg - deg_lo

    # --- per-core greedy 2D packing: balance (lo, hi) in-edge sums per tile
    slot_of = np.empty(N, np.int64)
    tile_of = np.empty(N, np.int64)
    nlo = np.zeros((NC, NT), np.int64)   # per-tile lo in-edges (incl. pad fakes)
    nhi = np.zeros((NC, NT), np.int64)
    cnt_ct = np.zeros((NC, NT), np.int64)
    for c in range(NC):
        heavy_is_lo = c < NC // 2        # own-group half carries the self-loops
        dh = deg_lo if heavy_is_lo else deg_hi
        dl = deg_hi if heavy_is_lo else deg_lo
        nodes = np.where(core_of == c)[0]
        nodes = nodes[np.argsort(-(dh[nodes] * 64 + dl[nodes]), kind="stable")]
        hv = np.zeros(NT, np.int64)
        lt = np.zeros(NT, np.int64)
        cnt = np.zeros(NT, np.int64)
        for v in nodes:
            cost = (hv + dh[v]).astype(np.float64) \
                + 0.02 * (lt + dl[v]) + 1e-4 * cnt \
                + 1e6 * np.maximum(lt + dl[v] - 5 * P, 0)
            cost[cnt >= P] = 1e18
            t = int(np.argmin(cost))
            tile_of[v] = t
            slot_of[v] = cnt[t]
            cnt[t] += 1
            hv[t] += dh[v]
            lt[t] += dl[v]
        lt += P - cnt                    # pad slots: one fake edge, light half
        if heavy_is_lo:
            nlo[c], nhi[c] = hv, lt
        else:
            nlo[c], nhi[c] = lt, hv
        cnt_ct[c] = cnt

    # --- per-core rank permutation so heavy tiles align across cores
    kl = (nlo + P - 1) // P
    kh = (nhi + P - 1) // P
    perm = np.empty((NC, NT), np.int64)     # rank -> old tile
    for c in range(NC):
        key = (kl[c] + kh[c]) + 1e-3 * kl[c] + 1e-9 * (nlo[c] + nhi[c])
        perm[c] = np.argsort(-key, kind="stable")

    Klo = [int(max(kl[c, perm[c, r]] for c in range(NC))) for r in range(NT)]
    Khi = [int(max(kh[c, perm[c, r]] for c in range(NC))) for r in range(NT)]
    T = [Klo[r] + Khi[r] for r in range(NT)]
    KM = max(max(Klo), max(Khi))

    rank_of = np.empty((NC, NT), np.int64)  # old tile -> rank
    for c in range(NC):
        rank_of[c, perm[c]] = np.arange(NT)
    local_of = rank_of[core_of, tile_of] * P + slot_of
    r_of = core_of * NPC + local_of         # final global table row of each node

    # --- per (core, old-tile) edge lists split by half (final r_of values)
    e_core = core_of[dst]
    e_tile = tile_of[dst]
    e_slot = slot_of[dst]
    lists_lo = {}
    lists_hi = {}
    for c in range(NC):
        m_c = e_core == c
        for tl in range(NT):
            m = m_c & (e_tile == tl)
            ml = m & lo_src
            mh = m & ~lo_src
            lists_lo[(c, tl)] = (r_of[src[ml]], e_slot[ml])
            lists_hi[(c, tl)] = (r_of[src[mh]] - TBL // 2, e_slot[mh])
            # fake self-edges for empty (padding) node slots so denominators
            # stay nonzero (their outputs are discarded by the host); they
            # live in the light half (matches the greedy's accounting)
            npad = P - cnt_ct[c, tl]
            if npad:
                pads = np.arange(P - npad, P)
                key = (c, tl)
                tgt = lists_hi if c < NC // 2 else lists_lo
                a, b = tgt[key]
                tgt[key] = (np.concatenate([a, np.zeros(npad, np.int64)]),
                            np.concatenate([b, pads]))

    # offsets
    od = np.concatenate([[0], np.cumsum(T)]).astype(np.int64)       # drel/ohne cols
    olo = np.concatenate([[0], np.cumsum(Klo)]).astype(np.int64)    # gidx_lo tiles
    ohi = np.concatenate([[0], np.cumsum(Khi)]).astype(np.int64)
    OD = int(od[-1]); OLO = int(olo[-1]); OHI = int(ohi[-1])

    def pack_idx(flat):
        n = len(flat)
        s = (n + 15) // 16
        arr = np.zeros(s * 16, np.int16)
        arr[:n] = flat
        block = arr.reshape(s, 16).T
        return np.tile(block, (8, 1))

    gidx_lo = np.zeros((NC, P, OLO * 8), np.int16)
    gidx_hi = np.zeros((NC, P, OHI * 8), np.int16)
    drel = np.full((NC, P, OD), -1.0, np.float32)
    for c in range(NC):
        for r in range(NT):
            tl = perm[c, r]
            for half, (K, off8, dcol0, lst) in enumerate([
                    (Klo[r], olo[r], od[r], lists_lo[(c, tl)]),
                    (Khi[r], ohi[r], od[r] + Klo[r], lists_hi[(c, tl)])]):
                rows, slots = lst
                # re-slot: dst slots were computed pre-permutation; slot within
                # tile is unchanged (rank remap keeps slot % P)
                n = len(rows)
                flat = np.zeros(K * P, np.int64)
                flat[:n] = rows
                g = gidx_hi if half else gidx_lo
                g[c, :, off8 * 8:(off8 + K) * 8] = pack_idx(flat)
                dr = np.full(K * P, -1.0, np.float32)
                dr[:n] = slots
                drel[c, :, dcol0:dcol0 + K] = dr.reshape(K, P).T

    node_order = np.full((NC, NPC), -1, np.int64)  # local row -> global node id
    for c in range(NC):
        nodes = np.where(core_of == c)[0]
        node_order[c, local_of[nodes]] = nodes

    # per tile col: [ohne (dst-major [d, e]) | ohen (edge-major [e, d])]
    ar = np.arange(P, dtype=np.float32)
    ohne = (ar[None, :, None, None]
            == drel.transpose(0, 2, 1)[:, None, :, :])   # [NC, P(d), OD, P(e)]
    ohen = (drel[:, :, :, None] == ar[None, None, None, :])  # [NC, P(e), OD, P(d)]
    ohb = np.empty((NC, P, 2 * OD * P), ml_dtypes.bfloat16)
    for r in range(NT):
        o0, o1 = int(od[r]), int(od[r + 1])
        t_ = o1 - o0
        ohb[:, :, 2 * o0 * P:(2 * o0 + t_) * P] = \
            ohne[:, :, o0:o1, :].reshape(NC, P, t_ * P)
        ohb[:, :, (2 * o0 + t_) * P:2 * o1 * P] = \
            ohen[:, :, o0:o1, :].reshape(NC, P, t_ * P)

    # AllGather groups (tile ranks)
    gsz = (NT + NGRP - 1) // NGRP
    groups = [(g * gsz, min((g + 1) * gsz, NT)) for g in range(NGRP)]
    groups = [(a, b) for a, b in groups if b > a]

    return dict(NPC=NPC, NT=NT, TBL=TBL, Klo=Klo, Khi=Khi, T=T, KM=KM,
                od=od.tolist(), olo=olo.tolist(), ohi=ohi.tolist(),
                OD=OD, OLO=OLO, OHI=OHI, groups=groups,
                gidx_lo=gidx_lo, gidx_hi=gidx_hi, ohb=ohb,
                node_order=node_order, core_of=core_of, local_of=local_of)


# --------------------------------------------------------------------------
# bass program
# --------------------------------------------------------------------------

def _build_program(dims, post_passes=True):
    PHASES = int(os.environ.get("GAT_PHASES", "3"))
    SHARED = os.environ.get("GAT_SHARED", "1") == "1"
    GB = int(os.environ.get("GAT_GB", "1"))        # gather batch (node tiles)
    import concourse.bass as bass
    import concourse.mybir as mybir
    import concourse.tile as tile
    from concourse import library_config
    from concourse.bass import _add_dep_helper
    import bass_rust as _br

    fp32 = mybir.dt.float32
    bf = mybir.dt.bfloat16
    i16 = mybir.dt.int16
    AX = mybir.AxisListType
    OP = mybir.AluOpType
    AF = mybir.ActivationFunctionType

    DIN = dims["DIN"]; HC = dims["HC"]; H = dims["H"]; CH = dims["CH"]
    CO = dims["CO"]
    NPC = dims["NPC"]; NT = dims["NT"]; TBL = dims["TBL"]
    Klo = dims["Klo"]; Khi = dims["Khi"]; T = dims["T"]; KM = dims["KM"]
    od = dims["od"]; olo = dims["olo"]; ohi = dims["ohi"]
    OD = dims["OD"]; OLO = dims["OLO"]; OHI = dims["OHI"]
    groups = dims["groups"]
    KD = DIN // P
    KH = HC // P
    CO_PAD = 128
    HALF = TBL // 2
    TM = max(T)
    addr_space = "Shared" if SHARED else "Local"

    # gather batches: consecutive ranks within each AG group, <= GB tiles
    batches = []        # (nt0, nt1)
    for g0, g1 in groups:
        nt = g0
        while nt < g1:
            batches.append((nt, min(nt + GB, g1)))
            nt = batches[-1][1]
    BKM = max(max(olo[b1] - olo[b0], ohi[b1] - ohi[b0]) for b0, b1 in batches)

    nc = bass.Bass(num_devices=NC, num_swdge_queues=4,
                   dynamic_dma_scratch_size=int(os.environ.get("GAT_DDS", "16384")))

    xkT_d = nc.dram_tensor("xkT", [P, NT * KD * P], bf, kind="ExternalInput")
    w1_d = nc.dram_tensor("w1", [DIN, 2 * HC], bf, kind="ExternalInput")
    w2_d = nc.dram_tensor("w2", [HC, 2 * CO], bf, kind="ExternalInput")
    CCOLS = KM * HC + HC + KM * CO + CO + P
    consts = nc.dram_tensor("consts", [P, CCOLS], bf, kind="ExternalInput")
    constf = nc.dram_tensor("constf", [P, 1], fp32, kind="ExternalInput")
    gidx_lo_d = nc.dram_tensor("gidx_lo", [P, OLO * 8], i16, kind="ExternalInput")
    gidx_hi_d = nc.dram_tensor("gidx_hi", [P, OHI * 8], i16, kind="ExternalInput")
    ohb_d = nc.dram_tensor("ohb", [P, 2 * OD * P], bf, kind="ExternalInput")
    h2_out = nc.dram_tensor("h2o", [NPC, CO], fp32, kind="ExternalOutput")
    ls_out = nc.dram_tensor("lso", [NPC, CO], fp32, kind="ExternalOutput")

    with tile.TileContext(nc) as tc:
        with (
            tc.tile_pool(name="dram", bufs=1, space="DRAM") as dram,
            tc.tile_pool(name="cst", bufs=1) as cst,
        ):
            lib = nc.gpsimd.load_library(library_config.mlp)
            regs = {}
            for b0, b1 in batches:
                for n in (olo[b1] - olo[b0], ohi[b1] - ohi[b0]):
                    if n not in regs:
                        regs[n] = nc.gpsimd.to_reg(n * P)

            ctile = cst.tile([P, CCOLS], bf)
            nc.sync.dma_start(out=ctile[:], in_=consts[:])
            cftile = cst.tile([P, 1], fp32)
            nc.sync.dma_start(out=cftile[:], in_=constf[:])
            o = 0
            ident = ctile[:, o:o + P]; o += P
            attBK = ctile[:, o:o + KM * HC]; o += KM * HC
            b1B = ctile[:, o:o + HC]; o += HC
            att2BK = ctile[:, o:o + KM * CO]; o += KM * CO
            b2B = ctile[:, o:o + CO]; o += CO
            alpha = cftile[:, 0:1]

            w1_sb = cst.tile([P, KD, 2 * HC], bf)
            nc.sync.dma_start(out=w1_sb[:], in_=w1_d.rearrange("(k p) c -> p k c", p=P))
            w2_sb = cst.tile([P, KH, 2 * CO], bf)
            nc.sync.dma_start(out=w2_sb[:], in_=w2_d.rearrange("(k p) c -> p k c", p=P))

            gidx_lo_sb = cst.tile([P, OLO * 8], i16)
            nc.sync.dma_start(out=gidx_lo_sb[:], in_=gidx_lo_d[:])
            gidx_hi_sb = cst.tile([P, OHI * 8], i16)
            nc.sync.dma_start(out=gidx_hi_sb[:], in_=gidx_hi_d[:])

            xr1_all = cst.tile([P, NT, HC], bf)
            xr2_all = cst.tile([P, NT, CO], bf)
            nc.vector.memset(xr2_all[:], 0.0)

            tbl1 = dram.tile([TBL, HC], bf)
            tbl2 = dram.tile([TBL, CO_PAD], bf)
            ag1_in = {}
            ag2_in = {}
            stg1 = {}
            stg2 = {}
            for gi, (g0, g1) in enumerate(groups):
                rows = (g1 - g0) * P
                ag1_in[gi] = dram.tile([rows, HC], bf, name=f"ag1i_{gi}")
                ag2_in[gi] = dram.tile([rows, CO_PAD], bf, name=f"ag2i_{gi}")
                stg1[gi] = dram.tile([NC * rows, HC], bf,
                                     addr_space=addr_space, name=f"stg1_{gi}")
                stg2[gi] = dram.tile([NC * rows, CO_PAD], bf,
                                     addr_space=addr_space, name=f"stg2_{gi}")

            def ag_chunk(src, stage, dst, g0, g1):
                nc.gpsimd.collective_compute(
                    "AllGather", mybir.AluOpType.bypass,
                    replica_groups=[list(range(NC))],
                    ins=[src[:].opt()],
                    outs=[stage[:].opt()],
                )
                nc.scalar.dma_start(
                    out=dst[:].rearrange("(c n) h -> c n h", c=NC)
                        [:, g0 * P:g1 * P, :],
                    in_=stage[:].rearrange("(c n) h -> c n h", c=NC))

            # ============ phase A: layer-1 projections ============
            with (tc.tile_pool(name="sbA", bufs=3) as sb,
                  tc.tile_pool(name="psA", bufs=2, space="PSUM") as ps):
                for gi, (g0, g1) in enumerate(groups):
                    for nt in range(g0, g1):
                        xt = sb.tile([P, KD, P], bf, tag="xt")
                        nc.sync.dma_start(
                            out=xt[:],
                            in_=xkT_d[:, nt * KD * P:(nt + 1) * KD * P])
                        xlr_ps = ps.tile([P, 2 * HC], fp32, tag="mm", space="PSUM")
                        for k in range(KD):
                            nc.tensor.matmul(out=xlr_ps[:], lhsT=xt[:, k, :],
                                             rhs=w1_sb[:, k, :],
                                             start=(k == 0), stop=(k == KD - 1))
                        xl_sb = sb.tile([P, HC], bf, tag="xls")
                        nc.vector.tensor_copy(out=xl_sb[:], in_=xlr_ps[:, 0:HC])
                        nc.vector.tensor_copy(out=xr1_all[:, nt, :],
                                              in_=xlr_ps[:, HC:2 * HC])
                        nc.sync.dma_start(
                            out=ag1_in[gi][(nt - g0) * P:(nt - g0 + 1) * P, :],
                            in_=xl_sb[:])
                    if PHASES >= 2:
                        ag_chunk(ag1_in[gi], stg1[gi], tbl1, g0, g1)

            # ============ phase B: layer-1 edges ============
            if PHASES >= 2:
                grp_of_nt = {}
                for gi, (g0, g1) in enumerate(groups):
                    for nt in range(g0, g1):
                        grp_of_nt[nt] = gi
                with (tc.tile_pool(name="sbB", bufs=2) as sb,
                      tc.tile_pool(name="gbB", bufs=3) as gb,
                      tc.tile_pool(name="psB", bufs=2, space="PSUM") as ps):
                    for b0, b1 in batches:
                        nlo_b = olo[b1] - olo[b0]
                        nhi_b = ohi[b1] - ohi[b0]
                        glo = gb.tile([P, BKM, HC], bf, tag="glo")
                        ghi = gb.tile([P, BKM, HC], bf, tag="ghi")
                        qb = 0 if True else (2 * (b0 % 2)) % 4
                        g1i = nc.gpsimd.dma_gather(
                            glo[:, 0:nlo_b, :], tbl1[0:HALF, :],
                            gidx_lo_sb[:, olo[b0] * 8:olo[b1] * 8],
                            nlo_b * P, regs[nlo_b], HC, queue_num=qb)
                        g2i = nc.gpsimd.dma_gather(
                            ghi[:, 0:nhi_b, :], tbl1[HALF:TBL, :],
                            gidx_hi_sb[:, ohi[b0] * 8:ohi[b1] * 8],
                            nhi_b * P, regs[nhi_b], HC, queue_num=qb)
                        _add_dep_helper(g1i.ins, lib.ins, sync=False, reason="lib")
                        _add_dep_helper(g2i.ins, lib.ins, sync=False, reason="lib")

                        for nt in range(b0, b1):
                            T_ = T[nt]; Klo_ = Klo[nt]; Khi_ = Khi[nt]
                            blo = olo[nt] - olo[b0]
                            bhi = ohi[nt] - ohi[b0]
                            ohb_t = sb.tile([P, 2 * TM * P], bf, tag="ohb")
                            nc.sync.dma_start(
                                out=ohb_t[:, 0:2 * T_ * P],
                                in_=ohb_d[:, 2 * od[nt] * P:2 * (od[nt] + T_) * P])
                            ohne = ohb_t[:, 0:T_ * P]
                            ohen = ohb_t[:, T_ * P:2 * T_ * P]

                            acc = ps.tile([P, HC + H], fp32, tag="acc",
                                          space="PSUM", bufs=2)
                            msg = sb.tile([P, TM, HC + H], bf, tag="msg")
                            for t0, K, gx, gb0 in [(0, Klo_, glo, blo),
                                                   (Klo_, Khi_, ghi, bhi)]:
                                t_sb = sb.tile([P, KM, HC], bf, tag="t")
                                for j in range(K):
                                    zj = ps.tile([P, HC], fp32, tag="z",
                                                 space="PSUM", bufs=4)
                                    nc.tensor.matmul(
                                        out=zj[:],
                                        lhsT=ohne[:, (t0 + j) * P:(t0 + j + 1) * P],
                                        rhs=xr1_all[:, nt, :],
                                        start=True, stop=False)
                                    nc.tensor.matmul(
                                        out=zj[:], lhsT=ident,
                                        rhs=gx[:, gb0 + j, :],
                                        start=False, stop=True)
                                    nc.scalar.activation(out=t_sb[:, j, :],
                                                         in_=zj[:],
                                                         func=AF.Prelu, alpha=alpha)
                                ta = sb.tile([P, KM, HC], bf, tag="ta")
                                nc.vector.tensor_tensor(
                                    out=ta[:, 0:K, :], in0=t_sb[:, 0:K, :],
                                    in1=attBK[:, 0:K * HC].rearrange(
                                        "p (k c) -> p k c", k=K), op=OP.mult)
                                sc = sb.tile([P, KM * H], fp32, tag="sc")
                                nc.vector.tensor_reduce(
                                    out=sc[:, 0:K * H],
                                    in_=ta[:, 0:K, :].rearrange(
                                        "p k (h c) -> p (k h) c", h=H),
                                    axis=AX.X, op=OP.add)
                                nc.scalar.activation(
                                    out=msg[:, t0:t0 + K, HC:HC + H],
                                    in_=sc[:, 0:K * H], func=AF.Exp)
                                nc.vector.tensor_tensor(
                                    out=msg[:, t0:t0 + K, 0:HC].rearrange(
                                        "p k (h c) -> p k h c", h=H),
                                    in0=gx[:, gb0:gb0 + K, :].rearrange(
                                        "p k (h c) -> p k h c", h=H),
                                    in1=msg[:, t0:t0 + K, HC:HC + H]
                                        [:, :, :, None]
                                        .to_broadcast([P, K, H, CH]),
                                    op=OP.mult)
                                for j in range(K):
                                    nc.tensor.matmul(
                                        out=acc[:],
                                        lhsT=ohen[:, (t0 + j) * P:(t0 + j + 1) * P],
                                        rhs=msg[:, t0 + j, :],
                                        start=(t0 + j == 0), stop=(t0 + j == T_ - 1))

                            rec = sb.tile([P, H], fp32, tag="rec")
                            nc.vector.reciprocal(out=rec[:], in_=acc[:, HC:HC + H])
                            h1 = sb.tile([P, HC], fp32, tag="h1")
                            nc.vector.tensor_tensor(
                                out=h1[:].rearrange("p (h c) -> p h c", h=H),
                                in0=acc[:, 0:HC].rearrange("p (h c) -> p h c", h=H),
                                in1=rec[:, :, None].to_broadcast([P, H, CH]),
                                op=OP.mult)
                            if dims["add_b1"]:
                                nc.vector.tensor_tensor(out=h1[:], in0=h1[:],
                                                        in1=b1B, op=OP.add)
                            eh = sb.tile([P, HC], fp32, tag="eh")
                            nc.scalar.activation(out=eh[:], in_=h1[:], func=AF.Exp)
                            em = sb.tile([P, HC], fp32, tag="em")
                            nc.vector.tensor_scalar(
                                out=em[:], in0=eh[:], scalar1=1.0, scalar2=0.0,
                                op0=OP.subtract, op1=OP.min)
                            elu = sb.tile([P, HC], bf, tag="elu")
                            nc.vector.tensor_scalar(out=elu[:], in0=h1[:],
                                                    scalar1=0.0, scalar2=None,
                                                    op0=OP.max)
                            nc.vector.tensor_tensor(out=elu[:], in0=elu[:],
                                                    in1=em[:], op=OP.add)

                            tail_ps = ps.tile([P, KH * P + 2 * CO], fp32,
                                              tag="tail", space="PSUM", bufs=2)
                            for k in range(KH):
                                nc.tensor.matmul(
                                    out=tail_ps[:, k * P:(k + 1) * P],
                                    lhsT=elu[:, k * P:(k + 1) * P],
                                    rhs=ident, start=True, stop=True)
                            hT_sb = sb.tile([P, KH, P], bf, tag="hTs")
                            nc.vector.tensor_copy(
                                out=hT_sb[:],
                                in_=tail_ps[:, 0:KH * P].rearrange(
                                    "p (k q) -> p k q", k=KH))
                            x2_ps = tail_ps[:, KH * P:KH * P + 2 * CO]
                            for k in range(KH):
                                nc.tensor.matmul(out=x2_ps, lhsT=hT_sb[:, k, :],
                                                 rhs=w2_sb[:, k, :],
                                                 start=(k == 0), stop=(k == KH - 1))
                            xl2_sb = sb.tile([P, CO], bf, tag="xl2s")
                            nc.vector.tensor_copy(out=xl2_sb[:], in_=x2_ps[:, 0:CO])
                            nc.vector.tensor_copy(out=xr2_all[:, nt, :],
                                                  in_=x2_ps[:, CO:2 * CO])
                            gi = grp_of_nt[nt]
                            g0_, _ = groups[gi]
                            nc.sync.dma_start(
                                out=ag2_in[gi][(nt - g0_) * P:(nt - g0_ + 1) * P,
                                               0:CO],
                                in_=xl2_sb[:])
                        if PHASES >= 3 and b1 == groups[grp_of_nt[b0]][1]:
                            gi = grp_of_nt[b0]
                            g0_, g1_ = groups[gi]
                            ag_chunk(ag2_in[gi], stg2[gi], tbl2, g0_, g1_)

            # ============ phase C: layer-2 edges ============
            if PHASES >= 3:
                with (tc.tile_pool(name="sbC", bufs=2) as sb,
                      tc.tile_pool(name="gbC", bufs=3) as gb,
                      tc.tile_pool(name="psC", bufs=2, space="PSUM") as ps):
                    for b0, b1 in batches:
                        nlo_b = olo[b1] - olo[b0]
                        nhi_b = ohi[b1] - ohi[b0]
                        g2lo = gb.tile([P, BKM, CO_PAD], bf, tag="g2lo")
                        g2hi = gb.tile([P, BKM, CO_PAD], bf, tag="g2hi")
                        qb = 0 if True else (2 * (b0 % 2)) % 4
                        g1i = nc.gpsimd.dma_gather(
                            g2lo[:, 0:nlo_b, :], tbl2[0:HALF, :],
                            gidx_lo_sb[:, olo[b0] * 8:olo[b1] * 8],
                            nlo_b * P, regs[nlo_b], CO_PAD, queue_num=qb)
                        g2i = nc.gpsimd.dma_gather(
                            g2hi[:, 0:nhi_b, :], tbl2[HALF:TBL, :],
                            gidx_hi_sb[:, ohi[b0] * 8:ohi[b1] * 8],
                            nhi_b * P, regs[nhi_b], CO_PAD, queue_num=qb)
                        _add_dep_helper(g1i.ins, lib.ins, sync=False, reason="lib")
                        _add_dep_helper(g2i.ins, lib.ins, sync=False, reason="lib")

                        for nt in range(b0, b1):
                            T_ = T[nt]; Klo_ = Klo[nt]; Khi_ = Khi[nt]
                            blo = olo[nt] - olo[b0]
                            bhi = ohi[nt] - ohi[b0]
                            ohb_t = sb.tile([P, 2 * TM * P], bf, tag="ohb2")
                            nc.sync.dma_start(
                                out=ohb_t[:, 0:2 * T_ * P],
                                in_=ohb_d[:, 2 * od[nt] * P:2 * (od[nt] + T_) * P])
                            ohne = ohb_t[:, 0:T_ * P]
                            ohen = ohb_t[:, T_ * P:2 * T_ * P]

                            acc2 = ps.tile([P, CO + 1], fp32, tag="acc2",
                                           space="PSUM", bufs=2)
                            msg2 = sb.tile([P, TM, CO + 1], bf, tag="msg2")
                            for t0, K, gx, gb0 in [(0, Klo_, g2lo, blo),
                                                   (Klo_, Khi_, g2hi, bhi)]:
                                t2 = sb.tile([P, KM, CO], bf, tag="t2")
                                for j in range(K):
                                    zj = ps.tile([P, CO], fp32, tag="z2",
                                                 space="PSUM", bufs=4)
                                    nc.tensor.matmul(
                                        out=zj[:],
                                        lhsT=ohne[:, (t0 + j) * P:(t0 + j + 1) * P],
                                        rhs=xr2_all[:, nt, :],
                                        start=True, stop=False)
                                    nc.tensor.matmul(
                                        out=zj[:], lhsT=ident,
                                        rhs=gx[:, gb0 + j, 0:CO],
                                        start=False, stop=True)
                                    nc.scalar.activation(out=t2[:, j, :],
                                                         in_=zj[:],
                                                         func=AF.Prelu, alpha=alpha)
                                ta2 = sb.tile([P, KM, CO], bf, tag="ta2")
                                nc.vector.tensor_tensor(
                                    out=ta2[:, 0:K, :], in0=t2[:, 0:K, :],
                                    in1=att2BK[:, 0:K * CO].rearrange(
                                        "p (k c) -> p k c", k=K), op=OP.mult)
                                sc2 = sb.tile([P, KM], fp32, tag="sc2")
                                nc.vector.tensor_reduce(
                                    out=sc2[:, 0:K], in_=ta2[:, 0:K, :],
                                    axis=AX.X, op=OP.add)
                                nc.scalar.activation(
                                    out=msg2[:, t0:t0 + K, CO:CO + 1],
                                    in_=sc2[:, 0:K], func=AF.Exp)
                                nc.vector.tensor_tensor(
                                    out=msg2[:, t0:t0 + K, 0:CO],
                                    in0=gx[:, gb0:gb0 + K, 0:CO],
                                    in1=msg2[:, t0:t0 + K, CO:CO + 1]
                                        .to_broadcast([P, K, CO]),
                                    op=OP.mult)
                                for j in range(K):
                                    nc.tensor.matmul(
                                        out=acc2[:],
                                        lhsT=ohen[:, (t0 + j) * P:(t0 + j + 1) * P],
                                        rhs=msg2[:, t0 + j, :],
                                        start=(t0 + j == 0), stop=(t0 + j == T_ - 1))

                            rec2 = sb.tile([P, 1], fp32, tag="rec2")
                            nc.vector.reciprocal(out=rec2[:], in_=acc2[:, CO:CO + 1])
                            h2 = sb.tile([P, CO], fp32, tag="h2")
                            nc.vector.tensor_scalar(out=h2[:], in0=acc2[:, 0:CO],
                                                    scalar1=rec2[:, 0:1],
                                                    scalar2=None, op0=OP.mult)
                            if dims["add_b2"]:
                                nc.vector.tensor_tensor(out=h2[:], in0=h2[:],
                                                        in1=b2B, op=OP.add)
                            nc.sync.dma_start(
                                out=h2_out.rearrange("(a p) d -> p a d", p=P)
                                    [:, nt, :],
                                in_=h2[:])
                            nm = sb.tile([P, 1], fp32, tag="nm")
                            nc.vector.tensor_reduce(out=nm[:], in_=h2[:], axis=AX.X,
                                                    op=OP.max, negate=True)
                            esc = sb.tile([P, CO], fp32, tag="esc")
                            ssum = sb.tile([P, 1], fp32, tag="ssum")
                            nc.scalar.activation(out=esc[:], in_=h2[:], func=AF.Exp,
                                                 bias=nm[:, 0:1],
                                                 accum_out=ssum[:, 0:1])
                            lns = sb.tile([P, 1], fp32, tag="lns")
                            nc.scalar.activation(out=lns[:], in_=ssum[:], func=AF.Ln)
                            ls = sb.tile([P, CO], fp32, tag="ls")
                            nc.vector.tensor_scalar(
                                out=ls[:], in0=h2[:], scalar1=nm[:, 0:1],
                                scalar2=lns[:, 0:1], op0=OP.add, op1=OP.subtract)
                            nc.sync.dma_start(
                                out=ls_out.rearrange("(a p) d -> p a d", p=P)
                                    [:, nt, :],
                                in_=ls[:])



    if post_passes:
        _br.generate_event_semaphores(nc)
        _br.codegen_inst_isa_subclasses(nc)
    return nc


# --------------------------------------------------------------------------
# entry point
# --------------------------------------------------------------------------

def kernel(x, edge_index, W1l, W1r, att1, b1, W2l, W2r, att2, b2):
    x = np.asarray(x, np.float32)
    edge_index = np.asarray(edge_index)
    W1l = np.asarray(W1l, np.float32); W1r = np.asarray(W1r, np.float32)
    att1 = np.asarray(att1, np.float32); b1 = np.asarray(b1, np.float32)
    W2l = np.asarray(W2l, np.float32); W2r = np.asarray(W2r, np.float32)
    att2 = np.asarray(att2, np.float32); b2 = np.asarray(b2, np.float32)

    N, DIN = x.shape
    E = edge_index.shape[1]
    H, CH = att1.shape
    HC = W1l.shape[1]
    CO = W2l.shape[1]

    key = (N, E, DIN, H, CH, HC, CO,
           int(np.abs(b1).max() > 0), int(np.abs(b2).max() > 0),
           hash(edge_index.tobytes()))
    if key in _plan_cache:
        pp, nc, dims = _plan_cache[key]
    else:
        pp = _preprocess(N, E, edge_index)
        dims = dict(DIN=DIN, HC=HC, H=H, CH=CH, CO=CO,
                    NPC=pp["NPC"], NT=pp["NT"], TBL=pp["TBL"],
                    Klo=pp["Klo"], Khi=pp["Khi"], T=pp["T"], KM=pp["KM"],
                    od=pp["od"], olo=pp["olo"], ohi=pp["ohi"],
                    OD=pp["OD"], OLO=pp["OLO"], OHI=pp["OHI"],
                    groups=pp["groups"],
                    add_b1=bool(np.abs(b1).max() > 0),
                    add_b2=bool(np.abs(b2).max() > 0))
        nc = _build_program(dims)
        _plan_cache[key] = (pp, nc, dims)

    NPC = pp["NPC"]; NT = pp["NT"]; KM = pp["KM"]
    KD = DIN // P
    bfdt = ml_dtypes.bfloat16

    # consts blob: ident | attBK | b1B | att2BK | b2B
    ident = np.eye(P, dtype=np.float32)
    attBK = np.broadcast_to(
        np.tile(att1.reshape(HC), KM)[None, :], (P, KM * HC))
    b1B = np.broadcast_to(b1.reshape(1, HC), (P, HC))
    att2BK = np.broadcast_to(
        np.tile(att2.reshape(CO), KM)[None, :], (P, KM * CO))
    b2B = np.broadcast_to(b2.reshape(1, CO), (P, CO))
    consts = np.concatenate([ident, attBK, b1B, att2BK, b2B],
                            axis=1).astype(bfdt)
    constf = np.full((P, 1), NEG_SLOPE, np.float32)
    w1cat = np.concatenate([W1l, W1r], axis=1).astype(bfdt)
    w2cat = np.concatenate([W2l, W2r], axis=1).astype(bfdt)

    in_maps = []
    for c in range(NC):
        xkc = np.zeros((NPC, DIN), np.float32)
        sel = pp["node_order"][c]
        real = sel >= 0
        xkc[real] = x[sel[real]]
        # [p, nt, k, q] = xkc[nt*P+q, k*P+p]
        xkT = np.ascontiguousarray(
            xkc.reshape(NT, P, KD, P).transpose(3, 0, 2, 1)
        ).reshape(P, NT * KD * P).astype(bfdt)
        in_maps.append(dict(
            xkT=xkT, w1=w1cat, w2=w2cat, consts=consts, constf=constf,
            gidx_lo=np.ascontiguousarray(pp["gidx_lo"][c]),
            gidx_hi=np.ascontiguousarray(pp["gidx_hi"][c]),
            ohb=np.ascontiguousarray(pp["ohb"][c]),
        ))

    from concourse.bass_utils import run_bass_kernel_spmd
    res = run_bass_kernel_spmd(nc, in_maps, core_ids=list(range(NC)))

    h = np.empty((N, CO), np.float32)
    ls = np.empty((N, CO), np.float32)
    r_core = pp["core_of"]
    r_loc = pp["local_of"]
    for c in range(NC):
        m = r_core == c
        h[m] = res.results[c]["h2o"][r_loc[m]]
        ls[m] = res.results[c]["lso"][r_loc[m]]
    return h, ls


# revision 35
# speedup vs baseline: 2.1071x; 1.0252x over previous
"""2-layer GATv2 (PyG GATv2Conv semantics) on 8 Trainium2 NeuronCores.

Strategy (v2):
  - Nodes sharded across 8 cores; per-core greedy 2D packing balances each
    destination tile's lo/hi in-edge counts (lo = src owned by cores 0-3).
  - x is shipped pre-transposed; layer-1 projections are 6 accumulating
    matmuls per node tile with a combined [W1l|W1r] moving operand.
  - xl tables AllGathered HBM->HBM in 4 row-chunks overlapped with compute.
  - Per destination tile, incoming-edge source rows are fetched with
    dma_gather (int16 idx; two table halves).  The dst->edge one-hot
    (oh_ne, [dst, edge]) is precomputed on host and streamed from HBM;
    the edge->dst one-hot (oh_en) is one batched DVE is_equal per half.
  - Per half (K edge tiles): K z-matmuls (xr broadcast), one batched DVE
    add (z+xl), 2-op leaky, att mult, reduce, exp, msg mult -- all batched
    over the half's K*128 edges -- then K scatter matmuls accumulate
    numerator+denominator in PSUM.
  - Softmax skips max-subtraction (scores O(1)).
  - Layer 2 (heads=1, 16 ch) repeats the edge structure on a 256B-row table.
  - log_softmax on ACT/DVE per node tile.

kernel(**inputs) takes FULL inputs, returns FULL outputs.
"""

import os
import sys

if "/opt/trn_rl_repo" not in sys.path:
    sys.path.insert(0, "/opt/trn_rl_repo")

import numpy as np
import ml_dtypes

NC = 8          # cores
P = 128         # partitions
NEG_SLOPE = 0.2
NGRP = 2        # AllGather chunks

_plan_cache = {}


# --------------------------------------------------------------------------
# host-side graph preprocessing
# --------------------------------------------------------------------------

def _snake(order, nbins):
    n = len(order)
    ids = np.arange(n)
    round_ = ids // nbins
    pos = ids % nbins
    b = np.where(round_ % 2 == 0, pos, nbins - 1 - pos)
    out = np.empty(n, np.int64)
    out[:] = b
    return out


def _preprocess(N, E, edge_index):
    NPC = ((N + NC - 1) // NC + P - 1) // P * P    # padded nodes per core
    NT = NPC // P
    TBL = NC * NPC
    assert TBL // 2 < 32768, "table half must fit int16 row indices"

    src = np.concatenate([edge_index[0].astype(np.int64), np.arange(N)])
    dst = np.concatenate([edge_index[1].astype(np.int64), np.arange(N)])
    deg = np.bincount(dst, minlength=N)

    # --- group assignment (fixes each edge's table half), then cores within
    # each group balanced by OWN-group in-degree (the heavy, self-loop half)
    order = np.argsort(-deg, kind="stable")
    grp_of = np.empty(N, np.int64)
    grp_of[order] = _snake(order, 2)
    e_own = grp_of[src] == grp_of[dst]
    deg_own = np.bincount(dst[e_own], minlength=N)
    core_of = np.empty(N, np.int64)
    half_nc = NC // 2
    for g in range(2):
        nodes_g = np.where(grp_of == g)[0]
        og = nodes_g[np.argsort(-deg_own[nodes_g], kind="stable")]
        core_of[og] = g * half_nc + _snake(og, half_nc)

    lo_src = core_of[src] < NC // 2                # which table half each edge reads
    deg_lo = np.bincount(dst[lo_src], minlength=N)
    deg_hi = deg - deg_lo

    # --- per-core greedy 2D packing: balance (lo, hi) in-edge sums per tile
    slot_of = np.empty(N, np.int64)
    tile_of = np.empty(N, np.int64)
    nlo = np.zeros((NC, NT), np.int64)   # per-tile lo in-edges (incl. pad fakes)
    nhi = np.zeros((NC, NT), np.int64)
    cnt_ct = np.zeros((NC, NT), np.int64)
    for c in range(NC):
        heavy_is_lo = c < NC // 2        # own-group half carries the self-loops
        dh = deg_lo if heavy_is_lo else deg_hi
        dl = deg_hi if heavy_is_lo else deg_lo
        nodes = np.where(core_of == c)[0]
        nodes = nodes[np.argsort(-(dh[nodes] * 64 + dl[nodes]), kind="stable")]
        hv = np.zeros(NT, np.int64)
        lt = np.zeros(NT, np.int64)
        cnt = np.zeros(NT, np.int64)
        for v in nodes:
            cost = (hv + dh[v]).astype(np.float64) \
                + 0.02 * (lt + dl[v]) + 1e-4 * cnt \
                + 1e6 * np.maximum(lt + dl[v] - 5 * P, 0)
            cost[cnt >= P] = 1e18
            t = int(np.argmin(cost))
            tile_of[v] = t
            slot_of[v] = cnt[t]
            cnt[t] += 1
            hv[t] += dh[v]
            lt[t] += dl[v]
        lt += P - cnt                    # pad slots: one fake edge, light half
        if heavy_is_lo:
            nlo[c], nhi[c] = hv, lt
        else:
            nlo[c], nhi[c] = lt, hv
        cnt_ct[c] = cnt

    # --- per-core rank permutation so heavy tiles align across cores
    kl = (nlo + P - 1) // P
    kh = (nhi + P - 1) // P
    perm = np.empty((NC, NT), np.int64)     # rank -> old tile
    for c in range(NC):
        key = (kl[c] + kh[c]) + 1e-3 * kl[c] + 1e-9 * (nlo[c] + nhi[c])
        perm[c] = np.argsort(-key, kind="stable")

    Klo = [int(max(kl[c, perm[c, r]] for c in range(NC))) for r in range(NT)]
    Khi = [int(max(kh[c, perm[c, r]] for c in range(NC))) for r in range(NT)]
    T = [Klo[r] + Khi[r] for r in range(NT)]
    KM = max(max(Klo), max(Khi))

    rank_of = np.empty((NC, NT), np.int64)  # old tile -> rank
    for c in range(NC):
        rank_of[c, perm[c]] = np.arange(NT)
    local_of = rank_of[core_of, tile_of] * P + slot_of
    r_of = core_of * NPC + local_of         # final global table row of each node

    # --- per (core, old-tile) edge lists split by half (final r_of values)
    e_core = core_of[dst]
    e_tile = tile_of[dst]
    e_slot = slot_of[dst]
    lists_lo = {}
    lists_hi = {}
    for c in range(NC):
        m_c = e_core == c
        for tl in range(NT):
            m = m_c & (e_tile == tl)
            ml = m & lo_src
            mh = m & ~lo_src
            lists_lo[(c, tl)] = (r_of[src[ml]], e_slot[ml])
            lists_hi[(c, tl)] = (r_of[src[mh]] - TBL // 2, e_slot[mh])
            # fake self-edges for empty (padding) node slots so denominators
            # stay nonzero (their outputs are discarded by the host); they
            # live in the light half (matches the greedy's accounting)
            npad = P - cnt_ct[c, tl]
            if npad:
                pads = np.arange(P - npad, P)
                key = (c, tl)
                tgt = lists_hi if c < NC // 2 else lists_lo
                a, b = tgt[key]
                tgt[key] = (np.concatenate([a, np.zeros(npad, np.int64)]),
                            np.concatenate([b, pads]))

    # offsets
    od = np.concatenate([[0], np.cumsum(T)]).astype(np.int64)       # drel/ohne cols
    olo = np.concatenate([[0], np.cumsum(Klo)]).astype(np.int64)    # gidx_lo tiles
    ohi = np.concatenate([[0], np.cumsum(Khi)]).astype(np.int64)
    OD = int(od[-1]); OLO = int(olo[-1]); OHI = int(ohi[-1])

    def pack_idx(flat):
        n = len(flat)
        s = (n + 15) // 16
        arr = np.zeros(s * 16, np.int16)
        arr[:n] = flat
        block = arr.reshape(s, 16).T
        return np.tile(block, (8, 1))

    gidx_lo = np.zeros((NC, P, OLO * 8), np.int16)
    gidx_hi = np.zeros((NC, P, OHI * 8), np.int16)
    drel = np.full((NC, P, OD), -1.0, np.float32)
    for c in range(NC):
        for r in range(NT):
            tl = perm[c, r]
            for half, (K, off8, dcol0, lst) in enumerate([
                    (Klo[r], olo[r], od[r], lists_lo[(c, tl)]),
                    (Khi[r], ohi[r], od[r] + Klo[r], lists_hi[(c, tl)])]):
                rows, slots = lst
                # re-slot: dst slots were computed pre-permutation; slot within
                # tile is unchanged (rank remap keeps slot % P)
                n = len(rows)
                flat = np.zeros(K * P, np.int64)
                flat[:n] = rows
                g = gidx_hi if half else gidx_lo
                g[c, :, off8 * 8:(off8 + K) * 8] = pack_idx(flat)
                dr = np.full(K * P, -1.0, np.float32)
                dr[:n] = slots
                drel[c, :, dcol0:dcol0 + K] = dr.reshape(K, P).T

    node_order = np.full((NC, NPC), -1, np.int64)  # local row -> global node id
    for c in range(NC):
        nodes = np.where(core_of == c)[0]
        node_order[c, local_of[nodes]] = nodes

    # per tile col: [ohne (dst-major [d, e]) | ohen (edge-major [e, d])]
    ar = np.arange(P, dtype=np.float32)
    ohne = (ar[None, :, None, None]
            == drel.transpose(0, 2, 1)[:, None, :, :])   # [NC, P(d), OD, P(e)]
    ohen = (drel[:, :, :, None] == ar[None, None, None, :])  # [NC, P(e), OD, P(d)]
    ohb = np.empty((NC, P, 2 * OD * P), ml_dtypes.bfloat16)
    for r in range(NT):
        o0, o1 = int(od[r]), int(od[r + 1])
        t_ = o1 - o0
        ohb[:, :, 2 * o0 * P:(2 * o0 + t_) * P] = \
            ohne[:, :, o0:o1, :].reshape(NC, P, t_ * P)
        ohb[:, :, (2 * o0 + t_) * P:2 * o1 * P] = \
            ohen[:, :, o0:o1, :].reshape(NC, P, t_ * P)

    # AllGather groups (tile ranks)
    gsz = (NT + NGRP - 1) // NGRP
    groups = [(g * gsz, min((g + 1) * gsz, NT)) for g in range(NGRP)]
    groups = [(a, b) for a, b in groups if b > a]

    return dict(NPC=NPC, NT=NT, TBL=TBL, Klo=Klo, Khi=Khi, T=T, KM=KM,
                od=od.tolist(), olo=olo.tolist(), ohi=ohi.tolist(),
                OD=OD, OLO=OLO, OHI=OHI, groups=groups,
                gidx_lo=gidx_lo, gidx_hi=gidx_hi, ohb=ohb,
                node_order=node_order, core_of=core_of, local_of=local_of)


# --------------------------------------------------------------------------
# bass program
# --------------------------------------------------------------------------

def _build_program(dims, post_passes=True):
    PHASES = int(os.environ.get("GAT_PHASES", "3"))
    SHARED = os.environ.get("GAT_SHARED", "1") == "1"
    GB = int(os.environ.get("GAT_GB", "1"))        # gather batch (node tiles)
    import concourse.bass as bass
    import concourse.mybir as mybir
    import concourse.tile as tile
    from concourse import library_config
    from concourse.bass import _add_dep_helper
    import bass_rust as _br

    fp32 = mybir.dt.float32
    bf = mybir.dt.bfloat16
    i16 = mybir.dt.int16
    AX = mybir.AxisListType
    OP = mybir.AluOpType
    AF = mybir.ActivationFunctionType

    DIN = dims["DIN"]; HC = dims["HC"]; H = dims["H"]; CH = dims["CH"]
    CO = dims["CO"]
    NPC = dims["NPC"]; NT = dims["NT"]; TBL = dims["TBL"]
    Klo = dims["Klo"]; Khi = dims["Khi"]; T = dims["T"]; KM = dims["KM"]
    od = dims["od"]; olo = dims["olo"]; ohi = dims["ohi"]
    OD = dims["OD"]; OLO = dims["OLO"]; OHI = dims["OHI"]
    groups = dims["groups"]
    KD = DIN // P
    KH = HC // P
    CO_PAD = 128
    HALF = TBL // 2
    TM = max(T)
    addr_space = "Shared" if SHARED else "Local"

    # gather batches: consecutive ranks within each AG group, <= GB tiles
    batches = []        # (nt0, nt1)
    for g0, g1 in groups:
        nt = g0
        while nt < g1:
            batches.append((nt, min(nt + GB, g1)))
            nt = batches[-1][1]
    BKM = max(max(olo[b1] - olo[b0], ohi[b1] - ohi[b0]) for b0, b1 in batches)

    nc = bass.Bass(num_devices=NC, num_swdge_queues=4,
                   dynamic_dma_scratch_size=int(os.environ.get("GAT_DDS", "16384")))

    xkT_d = nc.dram_tensor("xkT", [P, NT * KD * P], bf, kind="ExternalInput")
    w1_d = nc.dram_tensor("w1", [DIN, 2 * HC], bf, kind="ExternalInput")
    w2_d = nc.dram_tensor("w2", [HC, 2 * CO], bf, kind="ExternalInput")
    CCOLS = KM * HC + HC + KM * CO + CO + P
    consts = nc.dram_tensor("consts", [P, CCOLS], bf, kind="ExternalInput")
    constf = nc.dram_tensor("constf", [P, 1], fp32, kind="ExternalInput")
    gidx_lo_d = nc.dram_tensor("gidx_lo", [P, OLO * 8], i16, kind="ExternalInput")
    gidx_hi_d = nc.dram_tensor("gidx_hi", [P, OHI * 8], i16, kind="ExternalInput")
    ohb_d = nc.dram_tensor("ohb", [P, 2 * OD * P], bf, kind="ExternalInput")
    h2_out = nc.dram_tensor("h2o", [NPC, CO], fp32, kind="ExternalOutput")
    ls_out = nc.dram_tensor("lso", [NPC, CO], fp32, kind="ExternalOutput")

    with tile.TileContext(nc) as tc:
        with (
            tc.tile_pool(name="dram", bufs=1, space="DRAM") as dram,
            tc.tile_pool(name="cst", bufs=1) as cst,
        ):
            lib = nc.gpsimd.load_library(library_config.mlp)
            regs = {}
            for b0, b1 in batches:
                for n in (olo[b1] - olo[b0], ohi[b1] - ohi[b0]):
                    if n not in regs:
                        regs[n] = nc.gpsimd.to_reg(n * P)

            ctile = cst.tile([P, CCOLS], bf)
            nc.sync.dma_start(out=ctile[:], in_=consts[:])
            cftile = cst.tile([P, 1], fp32)
            nc.sync.dma_start(out=cftile[:], in_=constf[:])
            o = 0
            ident = ctile[:, o:o + P]; o += P
            attBK = ctile[:, o:o + KM * HC]; o += KM * HC
            b1B = ctile[:, o:o + HC]; o += HC
            att2BK = ctile[:, o:o + KM * CO]; o += KM * CO
            b2B = ctile[:, o:o + CO]; o += CO
            alpha = cftile[:, 0:1]

            w1_sb = cst.tile([P, KD, 2 * HC], bf)
            nc.sync.dma_start(out=w1_sb[:], in_=w1_d.rearrange("(k p) c -> p k c", p=P))
            w2_sb = cst.tile([P, KH, 2 * CO], bf)
            nc.sync.dma_start(out=w2_sb[:], in_=w2_d.rearrange("(k p) c -> p k c", p=P))

            gidx_lo_sb = cst.tile([P, OLO * 8], i16)
            nc.sync.dma_start(out=gidx_lo_sb[:], in_=gidx_lo_d[:])
            gidx_hi_sb = cst.tile([P, OHI * 8], i16)
            nc.sync.dma_start(out=gidx_hi_sb[:], in_=gidx_hi_d[:])

            xr1_all = cst.tile([P, NT, HC], bf)
            xr2_all = cst.tile([P, NT, CO], bf)
            nc.vector.memset(xr2_all[:], 0.0)

            tbl1 = dram.tile([TBL, HC], bf)
            tbl2 = dram.tile([TBL, CO_PAD], bf)
            ag1_in = {}
            ag2_in = {}
            stg1 = {}
            stg2 = {}
            for gi, (g0, g1) in enumerate(groups):
                rows = (g1 - g0) * P
                ag1_in[gi] = dram.tile([rows, HC], bf, name=f"ag1i_{gi}")
                ag2_in[gi] = dram.tile([rows, CO_PAD], bf, name=f"ag2i_{gi}")
                stg1[gi] = dram.tile([NC * rows, HC], bf,
                                     addr_space=addr_space, name=f"stg1_{gi}")
                stg2[gi] = dram.tile([NC * rows, CO_PAD], bf,
                                     addr_space=addr_space, name=f"stg2_{gi}")

            def ag_chunk(src, stage, dst, g0, g1):
                nc.gpsimd.collective_compute(
                    "AllGather", mybir.AluOpType.bypass,
                    replica_groups=[list(range(NC))],
                    ins=[src[:].opt()],
                    outs=[stage[:].opt()],
                )
                nc.scalar.dma_start(
                    out=dst[:].rearrange("(c n) h -> c n h", c=NC)
                        [:, g0 * P:g1 * P, :],
                    in_=stage[:].rearrange("(c n) h -> c n h", c=NC))

            # ============ phase A: layer-1 projections ============
            with (tc.tile_pool(name="sbA", bufs=3) as sb,
                  tc.tile_pool(name="psA", bufs=2, space="PSUM") as ps):
                XB = 4
                for gi, (g0, g1) in enumerate(groups):
                    for nb in range(g0, g1, XB):
                        ne = min(nb + XB, g1)
                        xt = sb.tile([P, XB, KD, P], bf, tag="xt")
                        nc.sync.dma_start(
                            out=xt[:, 0:ne - nb, :, :],
                            in_=xkT_d[:, nb * KD * P:ne * KD * P])
                        for nt in range(nb, ne):
                            xlr_ps = ps.tile([P, 2 * HC], fp32, tag="mm",
                                             space="PSUM")
                            for k in range(KD):
                                nc.tensor.matmul(out=xlr_ps[:],
                                                 lhsT=xt[:, nt - nb, k, :],
                                                 rhs=w1_sb[:, k, :],
                                                 start=(k == 0),
                                                 stop=(k == KD - 1))
                            xl_sb = sb.tile([P, HC], bf, tag="xls")
                            nc.vector.tensor_copy(out=xl_sb[:],
                                                  in_=xlr_ps[:, 0:HC])
                            nc.vector.tensor_copy(out=xr1_all[:, nt, :],
                                                  in_=xlr_ps[:, HC:2 * HC])
                            nc.sync.dma_start(
                                out=ag1_in[gi][(nt - g0) * P:
                                               (nt - g0 + 1) * P, :],
                                in_=xl_sb[:])
                    if PHASES >= 2:
                        ag_chunk(ag1_in[gi], stg1[gi], tbl1, g0, g1)

            # ============ phase B: layer-1 edges ============
            if PHASES >= 2:
                grp_of_nt = {}
                for gi, (g0, g1) in enumerate(groups):
                    for nt in range(g0, g1):
                        grp_of_nt[nt] = gi
                with (tc.tile_pool(name="sbB", bufs=2) as sb,
                      tc.tile_pool(name="gbB", bufs=3) as gb,
                      tc.tile_pool(name="psB", bufs=2, space="PSUM") as ps):
                    for b0, b1 in batches:
                        nlo_b = olo[b1] - olo[b0]
                        nhi_b = ohi[b1] - ohi[b0]
                        glo = gb.tile([P, BKM, HC], bf, tag="glo")
                        ghi = gb.tile([P, BKM, HC], bf, tag="ghi")
                        qb = 0 if True else (2 * (b0 % 2)) % 4
                        g1i = nc.gpsimd.dma_gather(
                            glo[:, 0:nlo_b, :], tbl1[0:HALF, :],
                            gidx_lo_sb[:, olo[b0] * 8:olo[b1] * 8],
                            nlo_b * P, regs[nlo_b], HC, queue_num=qb)
                        g2i = nc.gpsimd.dma_gather(
                            ghi[:, 0:nhi_b, :], tbl1[HALF:TBL, :],
                            gidx_hi_sb[:, ohi[b0] * 8:ohi[b1] * 8],
                            nhi_b * P, regs[nhi_b], HC, queue_num=qb)
                        _add_dep_helper(g1i.ins, lib.ins, sync=False, reason="lib")
                        _add_dep_helper(g2i.ins, lib.ins, sync=False, reason="lib")

                        for nt in range(b0, b1):
                            T_ = T[nt]; Klo_ = Klo[nt]; Khi_ = Khi[nt]
                            blo = olo[nt] - olo[b0]
                            bhi = ohi[nt] - ohi[b0]
                            ohb_t = sb.tile([P, 2 * TM * P], bf, tag="ohb")
                            nc.sync.dma_start(
                                out=ohb_t[:, 0:2 * T_ * P],
                                in_=ohb_d[:, 2 * od[nt] * P:2 * (od[nt] + T_) * P])
                            ohne = ohb_t[:, 0:T_ * P]
                            ohen = ohb_t[:, T_ * P:2 * T_ * P]

                            acc = ps.tile([P, HC + H], fp32, tag="acc",
                                          space="PSUM", bufs=2)
                            msg = sb.tile([P, TM, HC + H], bf, tag="msg")
                            for t0, K, gx, gb0 in [(0, Klo_, glo, blo),
                                                   (Klo_, Khi_, ghi, bhi)]:
                                t_sb = sb.tile([P, KM, HC], bf, tag="t")
                                for j in range(K):
                                    zj = ps.tile([P, HC], fp32, tag="z",
                                                 space="PSUM", bufs=4)
                                    nc.tensor.matmul(
                                        out=zj[:],
                                        lhsT=ohne[:, (t0 + j) * P:(t0 + j + 1) * P],
                                        rhs=xr1_all[:, nt, :],
                                        start=True, stop=False)
                                    nc.tensor.matmul(
                                        out=zj[:], lhsT=ident,
                                        rhs=gx[:, gb0 + j, :],
                                        start=False, stop=True)
                                    nc.scalar.activation(out=t_sb[:, j, :],
                                                         in_=zj[:],
                                                         func=AF.Prelu, alpha=alpha)
                                ta = sb.tile([P, KM, HC], bf, tag="ta")
                                nc.vector.tensor_tensor(
                                    out=ta[:, 0:K, :], in0=t_sb[:, 0:K, :],
                                    in1=attBK[:, 0:K * HC].rearrange(
                                        "p (k c) -> p k c", k=K), op=OP.mult)
                                sc = sb.tile([P, KM * H], fp32, tag="sc")
                                nc.vector.tensor_reduce(
                                    out=sc[:, 0:K * H],
                                    in_=ta[:, 0:K, :].rearrange(
                                        "p k (h c) -> p (k h) c", h=H),
                                    axis=AX.X, op=OP.add)
                                nc.scalar.activation(
                                    out=msg[:, t0:t0 + K, HC:HC + H],
                                    in_=sc[:, 0:K * H], func=AF.Exp)
                                nc.vector.tensor_tensor(
                                    out=msg[:, t0:t0 + K, 0:HC].rearrange(
                                        "p k (h c) -> p k h c", h=H),
                                    in0=gx[:, gb0:gb0 + K, :].rearrange(
                                        "p k (h c) -> p k h c", h=H),
                                    in1=msg[:, t0:t0 + K, HC:HC + H]
                                        [:, :, :, None]
                                        .to_broadcast([P, K, H, CH]),
                                    op=OP.mult)
                                for j in range(K):
                                    nc.tensor.matmul(
                                        out=acc[:],
                                        lhsT=ohen[:, (t0 + j) * P:(t0 + j + 1) * P],
                                        rhs=msg[:, t0 + j, :],
                                        start=(t0 + j == 0), stop=(t0 + j == T_ - 1))

                            rec = sb.tile([P, H], fp32, tag="rec")
                            nc.vector.reciprocal(out=rec[:], in_=acc[:, HC:HC + H])
                            h1 = sb.tile([P, HC], fp32, tag="h1")
                            nc.vector.tensor_tensor(
                                out=h1[:].rearrange("p (h c) -> p h c", h=H),
                                in0=acc[:, 0:HC].rearrange("p (h c) -> p h c", h=H),
                                in1=rec[:, :, None].to_broadcast([P, H, CH]),
                                op=OP.mult)
                            if dims["add_b1"]:
                                nc.vector.tensor_tensor(out=h1[:], in0=h1[:],
                                                        in1=b1B, op=OP.add)
                            eh = sb.tile([P, HC], fp32, tag="eh")
                            nc.scalar.activation(out=eh[:], in_=h1[:], func=AF.Exp)
                            em = sb.tile([P, HC], fp32, tag="em")
                            nc.vector.tensor_scalar(
                                out=em[:], in0=eh[:], scalar1=1.0, scalar2=0.0,
                                op0=OP.subtract, op1=OP.min)
                            elu = sb.tile([P, HC], bf, tag="elu")
                            nc.vector.tensor_scalar(out=elu[:], in0=h1[:],
                                                    scalar1=0.0, scalar2=None,
                                                    op0=OP.max)
                            nc.vector.tensor_tensor(out=elu[:], in0=elu[:],
                                                    in1=em[:], op=OP.add)

                            tail_ps = ps.tile([P, KH * P + 2 * CO], fp32,
                                              tag="tail", space="PSUM", bufs=2)
                            for k in range(KH):
                                nc.tensor.matmul(
                                    out=tail_ps[:, k * P:(k + 1) * P],
                                    lhsT=elu[:, k * P:(k + 1) * P],
                                    rhs=ident, start=True, stop=True)
                            hT_sb = sb.tile([P, KH, P], bf, tag="hTs")
                            nc.vector.tensor_copy(
                                out=hT_sb[:],
                                in_=tail_ps[:, 0:KH * P].rearrange(
                                    "p (k q) -> p k q", k=KH))
                            x2_ps = tail_ps[:, KH * P:KH * P + 2 * CO]
                            for k in range(KH):
                                nc.tensor.matmul(out=x2_ps, lhsT=hT_sb[:, k, :],
                                                 rhs=w2_sb[:, k, :],
                                                 start=(k == 0), stop=(k == KH - 1))
                            xl2_sb = sb.tile([P, CO], bf, tag="xl2s")
                            nc.vector.tensor_copy(out=xl2_sb[:], in_=x2_ps[:, 0:CO])
                            nc.vector.tensor_copy(out=xr2_all[:, nt, :],
                                                  in_=x2_ps[:, CO:2 * CO])
                            gi = grp_of_nt[nt]
                            g0_, _ = groups[gi]
                            nc.sync.dma_start(
                                out=ag2_in[gi][(nt - g0_) * P:(nt - g0_ + 1) * P,
                                               0:CO],
                                in_=xl2_sb[:])
                        if PHASES >= 3 and b1 == groups[grp_of_nt[b0]][1]:
                            gi = grp_of_nt[b0]
                            g0_, g1_ = groups[gi]
                            ag_chunk(ag2_in[gi], stg2[gi], tbl2, g0_, g1_)

            # ============ phase C: layer-2 edges ============
            if PHASES >= 3:
                with (tc.tile_pool(name="sbC", bufs=2) as sb,
                      tc.tile_pool(name="gbC", bufs=3) as gb,
                      tc.tile_pool(name="psC", bufs=2, space="PSUM") as ps):
                    for b0, b1 in batches:
                        nlo_b = olo[b1] - olo[b0]
                        nhi_b = ohi[b1] - ohi[b0]
                        g2lo = gb.tile([P, BKM, CO_PAD], bf, tag="g2lo")
                        g2hi = gb.tile([P, BKM, CO_PAD], bf, tag="g2hi")
                        qb = 0 if True else (2 * (b0 % 2)) % 4
                        g1i = nc.gpsimd.dma_gather(
                            g2lo[:, 0:nlo_b, :], tbl2[0:HALF, :],
                            gidx_lo_sb[:, olo[b0] * 8:olo[b1] * 8],
                            nlo_b * P, regs[nlo_b], CO_PAD, queue_num=qb)
                        g2i = nc.gpsimd.dma_gather(
                            g2hi[:, 0:nhi_b, :], tbl2[HALF:TBL, :],
                            gidx_hi_sb[:, ohi[b0] * 8:ohi[b1] * 8],
                            nhi_b * P, regs[nhi_b], CO_PAD, queue_num=qb)
                        _add_dep_helper(g1i.ins, lib.ins, sync=False, reason="lib")
                        _add_dep_helper(g2i.ins, lib.ins, sync=False, reason="lib")

                        for nt in range(b0, b1):
                            T_ = T[nt]; Klo_ = Klo[nt]; Khi_ = Khi[nt]
                            blo = olo[nt] - olo[b0]
                            bhi = ohi[nt] - ohi[b0]
                            ohb_t = sb.tile([P, 2 * TM * P], bf, tag="ohb2")
                            nc.sync.dma_start(
                                out=ohb_t[:, 0:2 * T_ * P],
                                in_=ohb_d[:, 2 * od[nt] * P:2 * (od[nt] + T_) * P])
                            ohne = ohb_t[:, 0:T_ * P]
                            ohen = ohb_t[:, T_ * P:2 * T_ * P]

                            acc2 = ps.tile([P, CO + 1], fp32, tag="acc2",
                                           space="PSUM", bufs=2)
                            msg2 = sb.tile([P, TM, CO + 1], bf, tag="msg2")
                            for t0, K, gx, gb0 in [(0, Klo_, g2lo, blo),
                                                   (Klo_, Khi_, g2hi, bhi)]:
                                t2 = sb.tile([P, KM, CO], bf, tag="t2")
                                for j in range(K):
                                    zj = ps.tile([P, CO], fp32, tag="z2",
                                                 space="PSUM", bufs=4)
                                    nc.tensor.matmul(
                                        out=zj[:],
                                        lhsT=ohne[:, (t0 + j) * P:(t0 + j + 1) * P],
                                        rhs=xr2_all[:, nt, :],
                                        start=True, stop=False)
                                    nc.tensor.matmul(
                                        out=zj[:], lhsT=ident,
                                        rhs=gx[:, gb0 + j, 0:CO],
                                        start=False, stop=True)
                                    nc.scalar.activation(out=t2[:, j, :],
                                                         in_=zj[:],
                                                         func=AF.Prelu, alpha=alpha)
                                ta2 = sb.tile([P, KM, CO], bf, tag="ta2")
                                nc.vector.tensor_tensor(
                                    out=ta2[:, 0:K, :], in0=t2[:, 0:K, :],
                                    in1=att2BK[:, 0:K * CO].rearrange(
                                        "p (k c) -> p k c", k=K), op=OP.mult)
                                sc2 = sb.tile([P, KM], fp32, tag="sc2")
                                nc.vector.tensor_reduce(
                                    out=sc2[:, 0:K], in_=ta2[:, 0:K, :],
                                    axis=AX.X, op=OP.add)
                                nc.scalar.activation(
                                    out=msg2[:, t0:t0 + K, CO:CO + 1],
                                    in_=sc2[:, 0:K], func=AF.Exp)
                                nc.vector.tensor_tensor(
                                    out=msg2[:, t0:t0 + K, 0:CO],
                                    in0=gx[:, gb0:gb0 + K, 0:CO],
                                    in1=msg2[:, t0:t0 + K, CO:CO + 1]
                                        .to_broadcast([P, K, CO]),
                                    op=OP.mult)
                                for j in range(K):
                                    nc.tensor.matmul(
                                        out=acc2[:],
                                        lhsT=ohen[:, (t0 + j) * P:(t0 + j + 1) * P],
                                        rhs=msg2[:, t0 + j, :],
                                        start=(t0 + j == 0), stop=(t0 + j == T_ - 1))

                            rec2 = sb.tile([P, 1], fp32, tag="rec2")
                            nc.vector.reciprocal(out=rec2[:], in_=acc2[:, CO:CO + 1])
                            h2 = sb.tile([P, CO], fp32, tag="h2")
                            nc.vector.tensor_scalar(out=h2[:], in0=acc2[:, 0:CO],
                                                    scalar1=rec2[:, 0:1],
                                                    scalar2=None, op0=OP.mult)
                            if dims["add_b2"]:
                                nc.vector.tensor_tensor(out=h2[:], in0=h2[:],
                                                        in1=b2B, op=OP.add)
                            nc.sync.dma_start(
                                out=h2_out.rearrange("(a p) d -> p a d", p=P)
                                    [:, nt, :],
                                in_=h2[:])
                            nm = sb.tile([P, 1], fp32, tag="nm")
                            nc.vector.tensor_reduce(out=nm[:], in_=h2[:], axis=AX.X,
                                                    op=OP.max, negate=True)
                            esc = sb.tile([P, CO], fp32, tag="esc")
                            ssum = sb.tile([P, 1], fp32, tag="ssum")
                            nc.scalar.activation(out=esc[:], in_=h2[:], func=AF.Exp,
                                                 bias=nm[:, 0:1],
                                                 accum_out=ssum[:, 0:1])
                            lns = sb.tile([P, 1], fp32, tag="lns")
                            nc.scalar.activation(out=lns[:], in_=ssum[:], func=AF.Ln)
                            ls = sb.tile([P, CO], fp32, tag="ls")
                            nc.vector.tensor_scalar(
                                out=ls[:], in0=h2[:], scalar1=nm[:, 0:1],
                                scalar2=lns[:, 0:1], op0=OP.add, op1=OP.subtract)
                            nc.sync.dma_start(
                                out=ls_out.rearrange("(a p) d -> p a d", p=P)
                                    [:, nt, :],
                                in_=ls[:])



    if post_passes:
        _br.generate_event_semaphores(nc)
        _br.codegen_inst_isa_subclasses(nc)
    return nc


# --------------------------------------------------------------------------
# entry point
# --------------------------------------------------------------------------

def kernel(x, edge_index, W1l, W1r, att1, b1, W2l, W2r, att2, b2):
    x = np.asarray(x, np.float32)
    edge_index = np.asarray(edge_index)
    W1l = np.asarray(W1l, np.float32); W1r = np.asarray(W1r, np.float32)
    att1 = np.asarray(att1, np.float32); b1 = np.asarray(b1, np.float32)
    W2l = np.asarray(W2l, np.float32); W2r = np.asarray(W2r, np.float32)
    att2 = np.asarray(att2, np.float32); b2 = np.asarray(b2, np.float32)

    N, DIN = x.shape
    E = edge_index.shape[1]
    H, CH = att1.shape
    HC = W1l.shape[1]
    CO = W2l.shape[1]

    key = (N, E, DIN, H, CH, HC, CO,
           int(np.abs(b1).max() > 0), int(np.abs(b2).max() > 0),
           hash(edge_index.tobytes()))
    if key in _plan_cache:
        pp, nc, dims = _plan_cache[key]
    else:
        pp = _preprocess(N, E, edge_index)
        dims = dict(DIN=DIN, HC=HC, H=H, CH=CH, CO=CO,
                    NPC=pp["NPC"], NT=pp["NT"], TBL=pp["TBL"],
                    Klo=pp["Klo"], Khi=pp["Khi"], T=pp["T"], KM=pp["KM"],
                    od=pp["od"], olo=pp["olo"], ohi=pp["ohi"],
                    OD=pp["OD"], OLO=pp["OLO"], OHI=pp["OHI"],
                    groups=pp["groups"],
                    add_b1=bool(np.abs(b1).max() > 0),
                    add_b2=bool(np.abs(b2).max() > 0))
        nc = _build_program(dims)
        _plan_cache[key] = (pp, nc, dims)

    NPC = pp["NPC"]; NT = pp["NT"]; KM = pp["KM"]
    KD = DIN // P
    bfdt = ml_dtypes.bfloat16

    # consts blob: ident | attBK | b1B | att2BK | b2B
    ident = np.eye(P, dtype=np.float32)
    attBK = np.broadcast_to(
        np.tile(att1.reshape(HC), KM)[None, :], (P, KM * HC))
    b1B = np.broadcast_to(b1.reshape(1, HC), (P, HC))
    att2BK = np.broadcast_to(
        np.tile(att2.reshape(CO), KM)[None, :], (P, KM * CO))
    b2B = np.broadcast_to(b2.reshape(1, CO), (P, CO))
    consts = np.concatenate([ident, attBK, b1B, att2BK, b2B],
                            axis=1).astype(bfdt)
    constf = np.full((P, 1), NEG_SLOPE, np.float32)
    w1cat = np.concatenate([W1l, W1r], axis=1).astype(bfdt)
    w2cat = np.concatenate([W2l, W2r], axis=1).astype(bfdt)

    in_maps = []
    for c in range(NC):
        xkc = np.zeros((NPC, DIN), np.float32)
        sel = pp["node_order"][c]
        real = sel >= 0
        xkc[real] = x[sel[real]]
        # [p, nt, k, q] = xkc[nt*P+q, k*P+p]
        xkT = np.ascontiguousarray(
            xkc.reshape(NT, P, KD, P).transpose(3, 0, 2, 1)
        ).reshape(P, NT * KD * P).astype(bfdt)
        in_maps.append(dict(
            xkT=xkT, w1=w1cat, w2=w2cat, consts=consts, constf=constf,
            gidx_lo=np.ascontiguousarray(pp["gidx_lo"][c]),
            gidx_hi=np.ascontiguousarray(pp["gidx_hi"][c]),
            ohb=np.ascontiguousarray(pp["ohb"][c]),
        ))

    from concourse.bass_utils import run_bass_kernel_spmd
    res = run_bass_kernel_spmd(nc, in_maps, core_ids=list(range(NC)))

    h = np.empty((N, CO), np.float32)
    ls = np.empty((N, CO), np.float32)
    r_core = pp["core_of"]
    r_loc = pp["local_of"]
    for c in range(NC):
        m = r_core == c
        h[m] = res.results[c]["h2o"][r_loc[m]]
        ls[m] = res.results[c]["lso"][r_loc[m]]
    return h, ls


# revision 38
# speedup vs baseline: 2.5901x; 1.2292x over previous
"""2-layer GATv2 (PyG GATv2Conv semantics) on 8 Trainium2 NeuronCores.

Strategy (v2):
  - Nodes sharded across 8 cores; per-core greedy 2D packing balances each
    destination tile's lo/hi in-edge counts (lo = src owned by cores 0-3).
  - x is shipped pre-transposed; layer-1 projections are 6 accumulating
    matmuls per node tile with a combined [W1l|W1r] moving operand.
  - xl tables AllGathered HBM->HBM in 4 row-chunks overlapped with compute.
  - Per destination tile, incoming-edge source rows are fetched with
    dma_gather (int16 idx; two table halves).  The dst->edge one-hot
    (oh_ne, [dst, edge]) is precomputed on host and streamed from HBM;
    the edge->dst one-hot (oh_en) is one batched DVE is_equal per half.
  - Per half (K edge tiles): K z-matmuls (xr broadcast), one batched DVE
    add (z+xl), 2-op leaky, att mult, reduce, exp, msg mult -- all batched
    over the half's K*128 edges -- then K scatter matmuls accumulate
    numerator+denominator in PSUM.
  - Softmax skips max-subtraction (scores O(1)).
  - Layer 2 (heads=1, 16 ch) repeats the edge structure on a 256B-row table.
  - log_softmax on ACT/DVE per node tile.

kernel(**inputs) takes FULL inputs, returns FULL outputs.
"""

import os
import sys

if "/opt/trn_rl_repo" not in sys.path:
    sys.path.insert(0, "/opt/trn_rl_repo")

import numpy as np
import ml_dtypes

NC = 8          # cores
P = 128         # partitions
NEG_SLOPE = 0.2
NGRP = 2        # AllGather chunks

_plan_cache = {}


# --------------------------------------------------------------------------
# host-side graph preprocessing
# --------------------------------------------------------------------------

def _snake(order, nbins):
    n = len(order)
    ids = np.arange(n)
    round_ = ids // nbins
    pos = ids % nbins
    b = np.where(round_ % 2 == 0, pos, nbins - 1 - pos)
    out = np.empty(n, np.int64)
    out[:] = b
    return out


def _preprocess(N, E, edge_index):
    # Two rank-groups (= AllGather chunks = table halves), 25 tile-ranks each.
    NTG = ((N + 2 * NC - 1) // (2 * NC) + P - 1) // P      # tiles per group
    NT = 2 * NTG
    NPC = NT * P
    TBL_G = NC * NTG * P                                   # rows per half-table
    assert TBL_G < 32768, "table half must fit int16 row indices"

    src = np.concatenate([edge_index[0].astype(np.int64), np.arange(N)])
    dst = np.concatenate([edge_index[1].astype(np.int64), np.arange(N)])
    deg = np.bincount(dst, minlength=N)

    # --- group assignment (fixes each edge's table half), then cores within
    # each group balanced by OWN-group in-degree (the heavy, self-loop half)
    order = np.argsort(-deg, kind="stable")
    grp_of = np.empty(N, np.int64)
    grp_of[order] = _snake(order, 2)
    e_own = grp_of[src] == grp_of[dst]
    deg_own = np.bincount(dst[e_own], minlength=N)
    deg_oth = deg - deg_own
    core_of = np.empty(N, np.int64)
    for g in range(2):
        nodes_g = np.where(grp_of == g)[0]
        og = nodes_g[np.argsort(-deg_own[nodes_g], kind="stable")]
        core_of[og] = _snake(og, NC)

    lo_src = grp_of[src] == 0              # which table half each edge reads

    # --- per (core, group) greedy packing into NTG tiles: keep the heavy
    # (own-group) sum under 5*P and the light sum under 4*P per tile
    slot_of = np.empty(N, np.int64)
    tile_of = np.empty(N, np.int64)        # tile index within the group
    nheav = np.zeros((NC, 2, NTG), np.int64)
    nlite = np.zeros((NC, 2, NTG), np.int64)
    cnt_ct = np.zeros((NC, 2, NTG), np.int64)
    for c in range(NC):
        for g in range(2):
            nodes = np.where((core_of == c) & (grp_of == g))[0]
            nodes = nodes[np.argsort(
                -(deg_own[nodes] * 64 + deg_oth[nodes]), kind="stable")]
            hv = np.zeros(NTG, np.int64)
            lt = np.zeros(NTG, np.int64)
            cnt = np.zeros(NTG, np.int64)
            for v in nodes:
                cost = (hv + deg_own[v]).astype(np.float64) \
                    + 0.02 * (lt + deg_oth[v]) + 1e-4 * cnt \
                    + 1e6 * np.maximum(lt + deg_oth[v] - 4 * P, 0)
                cost[cnt >= P] = 1e18
                t = int(np.argmin(cost))
                tile_of[v] = t
                slot_of[v] = cnt[t]
                cnt[t] += 1
                hv[t] += deg_own[v]
                lt[t] += deg_oth[v]
            # pad slots: one fake edge each, into whichever half has headroom
            pads = P - cnt
            room_l = np.maximum(4 * P - lt, 0)
            tol = np.minimum(pads, room_l)
            lt += tol
            hv += pads - tol
            nheav[c, g] = hv
            nlite[c, g] = lt
            cnt_ct[c, g] = cnt

    # --- per (core, group) rank permutation to align heavy tiles
    kh_ = (nheav + P - 1) // P
    kl_ = (nlite + P - 1) // P
    perm = np.empty((NC, 2, NTG), np.int64)
    for c in range(NC):
        for g in range(2):
            key = (kh_[c, g] + kl_[c, g]) + 1e-3 * kh_[c, g] \
                + 1e-9 * (nheav[c, g] + nlite[c, g])
            perm[c, g] = np.argsort(-key, kind="stable")

    # rank r in [0, NTG) -> group 0, [NTG, NT) -> group 1
    # group 0 tiles: heavy half = lo;  group 1 tiles: heavy half = hi
    Klo = []
    Khi = []
    for r in range(NT):
        g, rr = (0, r) if r < NTG else (1, r - NTG)
        kh = int(max(kh_[c, g, perm[c, g, rr]] for c in range(NC)))
        kl = int(max(kl_[c, g, perm[c, g, rr]] for c in range(NC)))
        if g == 0:
            Klo.append(kh); Khi.append(kl)
        else:
            Klo.append(kl); Khi.append(kh)
    T = [Klo[r] + Khi[r] for r in range(NT)]
    KM = max(max(Klo), max(Khi))

    rank_of = np.empty((NC, 2, NTG), np.int64)
    for c in range(NC):
        for g in range(2):
            rank_of[c, g, perm[c, g]] = np.arange(NTG)
    rank_glob = rank_of[core_of, grp_of, tile_of] + grp_of * NTG
    local_of = rank_glob * P + slot_of
    # table row within the node's half-table: [core][rank-in-group][slot]
    row_half = core_of * NTG * P + rank_of[core_of, grp_of, tile_of] * P + slot_of

    # --- per (core, group, tile) edge lists split by half
    e_core = core_of[dst]
    e_grp = grp_of[dst]
    e_tile = tile_of[dst]
    e_slot = slot_of[dst]
    lists_lo = {}
    lists_hi = {}
    for c in range(NC):
        for g in range(2):
            m_cg = (e_core == c) & (e_grp == g)
            for tl in range(NTG):
                m = m_cg & (e_tile == tl)
                ml = m & lo_src
                mh = m & ~lo_src
                lists_lo[(c, g, tl)] = (row_half[src[ml]], e_slot[ml])
                lists_hi[(c, g, tl)] = (row_half[src[mh]], e_slot[mh])
                # fake self-edges for empty (padding) node slots; split them
                # to match the greedy's half accounting (light first)
                npad = P - cnt_ct[c, g, tl]
                if npad:
                    pads = np.arange(P - npad, P)
                    lite_is_lo = g == 1
                    n_l = int(min(npad, max(
                        4 * P - len(lists_lo[(c, g, tl)][0]) if lite_is_lo
                        else 4 * P - len(lists_hi[(c, g, tl)][0]), 0)))
                    for tgt, idxs in ((lists_lo if lite_is_lo else lists_hi,
                                       pads[:n_l]),
                                      (lists_hi if lite_is_lo else lists_lo,
                                       pads[n_l:])):
                        if len(idxs):
                            a, b = tgt[(c, g, tl)]
                            tgt[(c, g, tl)] = (
                                np.concatenate([a, np.zeros(len(idxs),
                                                            np.int64)]),
                                np.concatenate([b, idxs]))

    # offsets
    od = np.concatenate([[0], np.cumsum(T)]).astype(np.int64)
    olo = np.concatenate([[0], np.cumsum(Klo)]).astype(np.int64)
    ohi = np.concatenate([[0], np.cumsum(Khi)]).astype(np.int64)
    OD = int(od[-1]); OLO = int(olo[-1]); OHI = int(ohi[-1])

    def pack_idx(flat):
        n = len(flat)
        s = (n + 15) // 16
        arr = np.zeros(s * 16, np.int16)
        arr[:n] = flat
        block = arr.reshape(s, 16).T
        return np.tile(block, (8, 1))

    gidx_lo = np.zeros((NC, P, OLO * 8), np.int16)
    gidx_hi = np.zeros((NC, P, OHI * 8), np.int16)
    drel = np.full((NC, P, OD), -1.0, np.float32)
    for c in range(NC):
        for r in range(NT):
            g, rr = (0, r) if r < NTG else (1, r - NTG)
            tl = perm[c, g, rr]
            for K, off8, dcol0, lst, gax in [
                    (Klo[r], olo[r], od[r], lists_lo[(c, g, tl)], gidx_lo),
                    (Khi[r], ohi[r], od[r] + Klo[r], lists_hi[(c, g, tl)],
                     gidx_hi)]:
                rows, slots = lst
                n = len(rows)
                assert n <= K * P
                flat = np.zeros(K * P, np.int64)
                flat[:n] = rows
                gax[c, :, off8 * 8:(off8 + K) * 8] = pack_idx(flat)
                dr = np.full(K * P, -1.0, np.float32)
                dr[:n] = slots
                drel[c, :, dcol0:dcol0 + K] = dr.reshape(K, P).T

    node_order = np.full((NC, NPC), -1, np.int64)
    for c in range(NC):
        nodes = np.where(core_of == c)[0]
        node_order[c, local_of[nodes]] = nodes

    # per tile col: [ohne (dst-major [d, e]) | ohen (edge-major [e, d])]
    ar = np.arange(P, dtype=np.float32)
    ohne = (ar[None, :, None, None]
            == drel.transpose(0, 2, 1)[:, None, :, :])
    ohen = (drel[:, :, :, None] == ar[None, None, None, :])
    ohb = np.empty((NC, P, 2 * OD * P), ml_dtypes.bfloat16)
    for r in range(NT):
        o0, o1 = int(od[r]), int(od[r + 1])
        t_ = o1 - o0
        ohb[:, :, 2 * o0 * P:(2 * o0 + t_) * P] = \
            ohne[:, :, o0:o1, :].reshape(NC, P, t_ * P)
        ohb[:, :, (2 * o0 + t_) * P:2 * o1 * P] = \
            ohen[:, :, o0:o1, :].reshape(NC, P, t_ * P)

    groups = [(0, NTG), (NTG, NT)]

    return dict(NPC=NPC, NT=NT, NTG=NTG, TBL_G=TBL_G,
                Klo=Klo, Khi=Khi, T=T, KM=KM,
                od=od.tolist(), olo=olo.tolist(), ohi=ohi.tolist(),
                OD=OD, OLO=OLO, OHI=OHI, groups=groups,
                gidx_lo=gidx_lo, gidx_hi=gidx_hi, ohb=ohb,
                node_order=node_order, core_of=core_of, local_of=local_of)


# --------------------------------------------------------------------------
# bass program
# --------------------------------------------------------------------------

def _build_program(dims, post_passes=True):
    PHASES = int(os.environ.get("GAT_PHASES", "3"))
    SHARED = os.environ.get("GAT_SHARED", "1") == "1"
    GB = int(os.environ.get("GAT_GB", "1"))        # gather batch (node tiles)
    import concourse.bass as bass
    import concourse.mybir as mybir
    import concourse.tile as tile
    from concourse import library_config
    from concourse.bass import _add_dep_helper
    import bass_rust as _br

    fp32 = mybir.dt.float32
    bf = mybir.dt.bfloat16
    i16 = mybir.dt.int16
    AX = mybir.AxisListType
    OP = mybir.AluOpType
    AF = mybir.ActivationFunctionType

    DIN = dims["DIN"]; HC = dims["HC"]; H = dims["H"]; CH = dims["CH"]
    CO = dims["CO"]
    NPC = dims["NPC"]; NT = dims["NT"]; NTG = dims["NTG"]
    TBL_G = dims["TBL_G"]
    Klo = dims["Klo"]; Khi = dims["Khi"]; T = dims["T"]; KM = dims["KM"]
    od = dims["od"]; olo = dims["olo"]; ohi = dims["ohi"]
    OD = dims["OD"]; OLO = dims["OLO"]; OHI = dims["OHI"]
    groups = dims["groups"]
    KD = DIN // P
    KH = HC // P
    CO_PAD = 128
    TM = max(T)
    addr_space = "Shared" if SHARED else "Local"

    # gather batches: consecutive ranks within each AG group, <= GB tiles
    batches = []        # (nt0, nt1)
    for g0, g1 in groups:
        nt = g0
        while nt < g1:
            batches.append((nt, min(nt + GB, g1)))
            nt = batches[-1][1]
    BKM = max(max(olo[b1] - olo[b0], ohi[b1] - ohi[b0]) for b0, b1 in batches)

    nc = bass.Bass(num_devices=NC, num_swdge_queues=4,
                   dynamic_dma_scratch_size=int(os.environ.get("GAT_DDS", "16384")))

    xkT_d = nc.dram_tensor("xkT", [P, NT * KD * P], bf, kind="ExternalInput")
    w1_d = nc.dram_tensor("w1", [DIN, 2 * HC], bf, kind="ExternalInput")
    w2_d = nc.dram_tensor("w2", [HC, 2 * CO], bf, kind="ExternalInput")
    CCOLS = KM * HC + HC + KM * CO + CO + P
    consts = nc.dram_tensor("consts", [P, CCOLS], bf, kind="ExternalInput")
    constf = nc.dram_tensor("constf", [P, 1], fp32, kind="ExternalInput")
    gidx_lo_d = nc.dram_tensor("gidx_lo", [P, OLO * 8], i16, kind="ExternalInput")
    gidx_hi_d = nc.dram_tensor("gidx_hi", [P, OHI * 8], i16, kind="ExternalInput")
    ohb_d = nc.dram_tensor("ohb", [P, 2 * OD * P], bf, kind="ExternalInput")
    h2_out = nc.dram_tensor("h2o", [NPC, CO], fp32, kind="ExternalOutput")
    ls_out = nc.dram_tensor("lso", [NPC, CO], fp32, kind="ExternalOutput")

    with tile.TileContext(nc) as tc:
        with (
            tc.tile_pool(name="dram", bufs=1, space="DRAM") as dram,
            tc.tile_pool(name="cst", bufs=1) as cst,
        ):
            lib = nc.gpsimd.load_library(library_config.mlp)
            regs = {}
            for b0, b1 in batches:
                for n in (olo[b1] - olo[b0], ohi[b1] - ohi[b0]):
                    if n not in regs:
                        regs[n] = nc.gpsimd.to_reg(n * P)

            ctile = cst.tile([P, CCOLS], bf)
            nc.sync.dma_start(out=ctile[:], in_=consts[:])
            cftile = cst.tile([P, 1], fp32)
            nc.sync.dma_start(out=cftile[:], in_=constf[:])
            o = 0
            ident = ctile[:, o:o + P]; o += P
            attBK = ctile[:, o:o + KM * HC]; o += KM * HC
            b1B = ctile[:, o:o + HC]; o += HC
            att2BK = ctile[:, o:o + KM * CO]; o += KM * CO
            b2B = ctile[:, o:o + CO]; o += CO
            alpha = cftile[:, 0:1]

            w1_sb = cst.tile([P, KD, 2 * HC], bf)
            nc.sync.dma_start(out=w1_sb[:], in_=w1_d.rearrange("(k p) c -> p k c", p=P))
            w2_sb = cst.tile([P, KH, 2 * CO], bf)
            nc.sync.dma_start(out=w2_sb[:], in_=w2_d.rearrange("(k p) c -> p k c", p=P))

            gidx_lo_sb = cst.tile([P, OLO * 8], i16)
            nc.sync.dma_start(out=gidx_lo_sb[:], in_=gidx_lo_d[:])
            gidx_hi_sb = cst.tile([P, OHI * 8], i16)
            nc.sync.dma_start(out=gidx_hi_sb[:], in_=gidx_hi_d[:])

            xr1_all = cst.tile([P, NT, HC], bf)
            xr2_all = cst.tile([P, NT, CO], bf)
            nc.vector.memset(xr2_all[:], 0.0)

            tbl1 = {}
            tbl2 = {}
            ag1_in = {}
            ag2_in = {}
            for gi, (g0, g1) in enumerate(groups):
                rows = (g1 - g0) * P
                ag1_in[gi] = dram.tile([rows, HC], bf, name=f"ag1i_{gi}")
                ag2_in[gi] = dram.tile([rows, CO_PAD], bf, name=f"ag2i_{gi}")
                tbl1[gi] = dram.tile([NC * rows, HC], bf,
                                     addr_space=addr_space, name=f"tbl1_{gi}")
                tbl2[gi] = dram.tile([NC * rows, CO_PAD], bf,
                                     addr_space=addr_space, name=f"tbl2_{gi}")

            def ag_chunk(src, dst):
                nc.gpsimd.collective_compute(
                    "AllGather", mybir.AluOpType.bypass,
                    replica_groups=[list(range(NC))],
                    ins=[src[:].opt()],
                    outs=[dst[:].opt()],
                )

            # ============ phase A: layer-1 projections ============
            with (tc.tile_pool(name="sbA", bufs=3) as sb,
                  tc.tile_pool(name="psA", bufs=2, space="PSUM") as ps):
                XB = 4
                for gi, (g0, g1) in enumerate(groups):
                    for nb in range(g0, g1, XB):
                        ne = min(nb + XB, g1)
                        xt = sb.tile([P, XB, KD, P], bf, tag="xt")
                        nc.sync.dma_start(
                            out=xt[:, 0:ne - nb, :, :],
                            in_=xkT_d[:, nb * KD * P:ne * KD * P])
                        for nt in range(nb, ne):
                            xlr_ps = ps.tile([P, 2 * HC], fp32, tag="mm",
                                             space="PSUM")
                            for k in range(KD):
                                nc.tensor.matmul(out=xlr_ps[:],
                                                 lhsT=xt[:, nt - nb, k, :],
                                                 rhs=w1_sb[:, k, :],
                                                 start=(k == 0),
                                                 stop=(k == KD - 1))
                            xl_sb = sb.tile([P, HC], bf, tag="xls")
                            nc.vector.tensor_copy(out=xl_sb[:],
                                                  in_=xlr_ps[:, 0:HC])
                            nc.vector.tensor_copy(out=xr1_all[:, nt, :],
                                                  in_=xlr_ps[:, HC:2 * HC])
                            nc.sync.dma_start(
                                out=ag1_in[gi][(nt - g0) * P:
                                               (nt - g0 + 1) * P, :],
                                in_=xl_sb[:])
                    if PHASES >= 2:
                        ag_chunk(ag1_in[gi], tbl1[gi])

            # ============ phase B: layer-1 edges ============
            if PHASES >= 2:
                grp_of_nt = {}
                for gi, (g0, g1) in enumerate(groups):
                    for nt in range(g0, g1):
                        grp_of_nt[nt] = gi
                with (tc.tile_pool(name="sbB", bufs=2) as sb,
                      tc.tile_pool(name="gbB", bufs=3) as gb,
                      tc.tile_pool(name="psB", bufs=2, space="PSUM") as ps):
                    for b0, b1 in batches:
                        nlo_b = olo[b1] - olo[b0]
                        nhi_b = ohi[b1] - ohi[b0]
                        glo = gb.tile([P, BKM, HC], bf, tag="glo")
                        ghi = gb.tile([P, BKM, HC], bf, tag="ghi")
                        qb = 0 if True else (2 * (b0 % 2)) % 4
                        g1i = nc.gpsimd.dma_gather(
                            glo[:, 0:nlo_b, :], tbl1[0][:],
                            gidx_lo_sb[:, olo[b0] * 8:olo[b1] * 8],
                            nlo_b * P, regs[nlo_b], HC, queue_num=qb)
                        g2i = nc.gpsimd.dma_gather(
                            ghi[:, 0:nhi_b, :], tbl1[1][:],
                            gidx_hi_sb[:, ohi[b0] * 8:ohi[b1] * 8],
                            nhi_b * P, regs[nhi_b], HC, queue_num=qb)
                        _add_dep_helper(g1i.ins, lib.ins, sync=False, reason="lib")
                        _add_dep_helper(g2i.ins, lib.ins, sync=False, reason="lib")

                        for nt in range(b0, b1):
                            T_ = T[nt]; Klo_ = Klo[nt]; Khi_ = Khi[nt]
                            blo = olo[nt] - olo[b0]
                            bhi = ohi[nt] - ohi[b0]
                            ohb_t = sb.tile([P, 2 * TM * P], bf, tag="ohb")
                            nc.sync.dma_start(
                                out=ohb_t[:, 0:2 * T_ * P],
                                in_=ohb_d[:, 2 * od[nt] * P:2 * (od[nt] + T_) * P])
                            ohne = ohb_t[:, 0:T_ * P]
                            ohen = ohb_t[:, T_ * P:2 * T_ * P]

                            acc = ps.tile([P, HC + H], fp32, tag="acc",
                                          space="PSUM", bufs=2)
                            msg = sb.tile([P, TM, HC + H], bf, tag="msg")
                            for t0, K, gx, gb0 in [(0, Klo_, glo, blo),
                                                   (Klo_, Khi_, ghi, bhi)]:
                                t_sb = sb.tile([P, KM, HC], bf, tag="t")
                                for j in range(K):
                                    zj = ps.tile([P, HC], fp32, tag="z",
                                                 space="PSUM", bufs=4)
                                    nc.tensor.matmul(
                                        out=zj[:],
                                        lhsT=ohne[:, (t0 + j) * P:(t0 + j + 1) * P],
                                        rhs=xr1_all[:, nt, :],
                                        start=True, stop=False)
                                    nc.tensor.matmul(
                                        out=zj[:], lhsT=ident,
                                        rhs=gx[:, gb0 + j, :],
                                        start=False, stop=True)
                                    nc.scalar.activation(out=t_sb[:, j, :],
                                                         in_=zj[:],
                                                         func=AF.Prelu, alpha=alpha)
                                ta = sb.tile([P, KM, HC], bf, tag="ta")
                                nc.vector.tensor_tensor(
                                    out=ta[:, 0:K, :], in0=t_sb[:, 0:K, :],
                                    in1=attBK[:, 0:K * HC].rearrange(
                                        "p (k c) -> p k c", k=K), op=OP.mult)
                                sc = sb.tile([P, KM * H], fp32, tag="sc")
                                nc.vector.tensor_reduce(
                                    out=sc[:, 0:K * H],
                                    in_=ta[:, 0:K, :].rearrange(
                                        "p k (h c) -> p (k h) c", h=H),
                                    axis=AX.X, op=OP.add)
                                nc.scalar.activation(
                                    out=msg[:, t0:t0 + K, HC:HC + H],
                                    in_=sc[:, 0:K * H], func=AF.Exp)
                                nc.vector.tensor_tensor(
                                    out=msg[:, t0:t0 + K, 0:HC].rearrange(
                                        "p k (h c) -> p k h c", h=H),
                                    in0=gx[:, gb0:gb0 + K, :].rearrange(
                                        "p k (h c) -> p k h c", h=H),
                                    in1=msg[:, t0:t0 + K, HC:HC + H]
                                        [:, :, :, None]
                                        .to_broadcast([P, K, H, CH]),
                                    op=OP.mult)
                                for j in range(K):
                                    nc.tensor.matmul(
                                        out=acc[:],
                                        lhsT=ohen[:, (t0 + j) * P:(t0 + j + 1) * P],
                                        rhs=msg[:, t0 + j, :],
                                        start=(t0 + j == 0), stop=(t0 + j == T_ - 1))

                            rec = sb.tile([P, H], fp32, tag="rec")
                            nc.vector.reciprocal(out=rec[:], in_=acc[:, HC:HC + H])
                            h1 = sb.tile([P, HC], fp32, tag="h1")
                            nc.vector.tensor_tensor(
                                out=h1[:].rearrange("p (h c) -> p h c", h=H),
                                in0=acc[:, 0:HC].rearrange("p (h c) -> p h c", h=H),
                                in1=rec[:, :, None].to_broadcast([P, H, CH]),
                                op=OP.mult)
                            if dims["add_b1"]:
                                nc.vector.tensor_tensor(out=h1[:], in0=h1[:],
                                                        in1=b1B, op=OP.add)
                            eh = sb.tile([P, HC], fp32, tag="eh")
                            nc.scalar.activation(out=eh[:], in_=h1[:], func=AF.Exp)
                            em = sb.tile([P, HC], fp32, tag="em")
                            nc.vector.tensor_scalar(
                                out=em[:], in0=eh[:], scalar1=1.0, scalar2=0.0,
                                op0=OP.subtract, op1=OP.min)
                            elu = sb.tile([P, HC], bf, tag="elu")
                            nc.vector.tensor_scalar(out=elu[:], in0=h1[:],
                                                    scalar1=0.0, scalar2=None,
                                                    op0=OP.max)
                            nc.vector.tensor_tensor(out=elu[:], in0=elu[:],
                                                    in1=em[:], op=OP.add)

                            tail_ps = ps.tile([P, KH * P + 2 * CO], fp32,
                                              tag="tail", space="PSUM", bufs=2)
                            for k in range(KH):
                                nc.tensor.matmul(
                                    out=tail_ps[:, k * P:(k + 1) * P],
                                    lhsT=elu[:, k * P:(k + 1) * P],
                                    rhs=ident, start=True, stop=True)
                            hT_sb = sb.tile([P, KH, P], bf, tag="hTs")
                            nc.vector.tensor_copy(
                                out=hT_sb[:],
                                in_=tail_ps[:, 0:KH * P].rearrange(
                                    "p (k q) -> p k q", k=KH))
                            x2_ps = tail_ps[:, KH * P:KH * P + 2 * CO]
                            for k in range(KH):
                                nc.tensor.matmul(out=x2_ps, lhsT=hT_sb[:, k, :],
                                                 rhs=w2_sb[:, k, :],
                                                 start=(k == 0), stop=(k == KH - 1))
                            xl2_sb = sb.tile([P, CO], bf, tag="xl2s")
                            nc.vector.tensor_copy(out=xl2_sb[:], in_=x2_ps[:, 0:CO])
                            nc.vector.tensor_copy(out=xr2_all[:, nt, :],
                                                  in_=x2_ps[:, CO:2 * CO])
                            gi = grp_of_nt[nt]
                            g0_, _ = groups[gi]
                            nc.sync.dma_start(
                                out=ag2_in[gi][(nt - g0_) * P:(nt - g0_ + 1) * P,
                                               0:CO],
                                in_=xl2_sb[:])
                        if PHASES >= 3 and b1 == groups[grp_of_nt[b0]][1]:
                            gi = grp_of_nt[b0]
                            g0_, g1_ = groups[gi]
                            ag_chunk(ag2_in[gi], tbl2[gi])

            # ============ phase C: layer-2 edges ============
            if PHASES >= 3:
                with (tc.tile_pool(name="sbC", bufs=2) as sb,
                      tc.tile_pool(name="gbC", bufs=3) as gb,
                      tc.tile_pool(name="psC", bufs=2, space="PSUM") as ps):
                    for b0, b1 in batches:
                        nlo_b = olo[b1] - olo[b0]
                        nhi_b = ohi[b1] - ohi[b0]
                        g2lo = gb.tile([P, BKM, CO_PAD], bf, tag="g2lo")
                        g2hi = gb.tile([P, BKM, CO_PAD], bf, tag="g2hi")
                        qb = 0 if True else (2 * (b0 % 2)) % 4
                        g1i = nc.gpsimd.dma_gather(
                            g2lo[:, 0:nlo_b, :], tbl2[0][:],
                            gidx_lo_sb[:, olo[b0] * 8:olo[b1] * 8],
                            nlo_b * P, regs[nlo_b], CO_PAD, queue_num=qb)
                        g2i = nc.gpsimd.dma_gather(
                            g2hi[:, 0:nhi_b, :], tbl2[1][:],
                            gidx_hi_sb[:, ohi[b0] * 8:ohi[b1] * 8],
                            nhi_b * P, regs[nhi_b], CO_PAD, queue_num=qb)
                        _add_dep_helper(g1i.ins, lib.ins, sync=False, reason="lib")
                        _add_dep_helper(g2i.ins, lib.ins, sync=False, reason="lib")

                        for nt in range(b0, b1):
                            T_ = T[nt]; Klo_ = Klo[nt]; Khi_ = Khi[nt]
                            blo = olo[nt] - olo[b0]
                            bhi = ohi[nt] - ohi[b0]
                            ohb_t = sb.tile([P, 2 * TM * P], bf, tag="ohb2")
                            nc.sync.dma_start(
                                out=ohb_t[:, 0:2 * T_ * P],
                                in_=ohb_d[:, 2 * od[nt] * P:2 * (od[nt] + T_) * P])
                            ohne = ohb_t[:, 0:T_ * P]
                            ohen = ohb_t[:, T_ * P:2 * T_ * P]

                            acc2 = ps.tile([P, CO + 1], fp32, tag="acc2",
                                           space="PSUM", bufs=2)
                            msg2 = sb.tile([P, TM, CO + 1], bf, tag="msg2")
                            for t0, K, gx, gb0 in [(0, Klo_, g2lo, blo),
                                                   (Klo_, Khi_, g2hi, bhi)]:
                                t2 = sb.tile([P, KM, CO], bf, tag="t2")
                                for j in range(K):
                                    zj = ps.tile([P, CO], fp32, tag="z2",
                                                 space="PSUM", bufs=4)
                                    nc.tensor.matmul(
                                        out=zj[:],
                                        lhsT=ohne[:, (t0 + j) * P:(t0 + j + 1) * P],
                                        rhs=xr2_all[:, nt, :],
                                        start=True, stop=False)
                                    nc.tensor.matmul(
                                        out=zj[:], lhsT=ident,
                                        rhs=gx[:, gb0 + j, 0:CO],
                                        start=False, stop=True)
                                    nc.scalar.activation(out=t2[:, j, :],
                                                         in_=zj[:],
                                                         func=AF.Prelu, alpha=alpha)
                                ta2 = sb.tile([P, KM, CO], bf, tag="ta2")
                                nc.vector.tensor_tensor(
                                    out=ta2[:, 0:K, :], in0=t2[:, 0:K, :],
                                    in1=att2BK[:, 0:K * CO].rearrange(
                                        "p (k c) -> p k c", k=K), op=OP.mult)
                                sc2 = sb.tile([P, KM], fp32, tag="sc2")
                                nc.vector.tensor_reduce(
                                    out=sc2[:, 0:K], in_=ta2[:, 0:K, :],
                                    axis=AX.X, op=OP.add)
                                nc.scalar.activation(
                                    out=msg2[:, t0:t0 + K, CO:CO + 1],
                                    in_=sc2[:, 0:K], func=AF.Exp)
                                nc.vector.tensor_tensor(
                                    out=msg2[:, t0:t0 + K, 0:CO],
                                    in0=gx[:, gb0:gb0 + K, 0:CO],
                                    in1=msg2[:, t0:t0 + K, CO:CO + 1]
                                        .to_broadcast([P, K, CO]),
                                    op=OP.mult)
                                for j in range(K):
                                    nc.tensor.matmul(
                                        out=acc2[:],
                                        lhsT=ohen[:, (t0 + j) * P:(t0 + j + 1) * P],
                                        rhs=msg2[:, t0 + j, :],
                                        start=(t0 + j == 0), stop=(t0 + j == T_ - 1))

                            rec2 = sb.tile([P, 1], fp32, tag="rec2")
                            nc.vector.reciprocal(out=rec2[:], in_=acc2[:, CO:CO + 1])
                            h2 = sb.tile([P, CO], fp32, tag="h2")
                            nc.vector.tensor_scalar(out=h2[:], in0=acc2[:, 0:CO],
                                                    scalar1=rec2[:, 0:1],
                                                    scalar2=None, op0=OP.mult)
                            if dims["add_b2"]:
                                nc.vector.tensor_tensor(out=h2[:], in0=h2[:],
                                                        in1=b2B, op=OP.add)
                            nc.sync.dma_start(
                                out=h2_out.rearrange("(a p) d -> p a d", p=P)
                                    [:, nt, :],
                                in_=h2[:])
                            nm = sb.tile([P, 1], fp32, tag="nm")
                            nc.vector.tensor_reduce(out=nm[:], in_=h2[:], axis=AX.X,
                                                    op=OP.max, negate=True)
                            esc = sb.tile([P, CO], fp32, tag="esc")
                            ssum = sb.tile([P, 1], fp32, tag="ssum")
                            nc.scalar.activation(out=esc[:], in_=h2[:], func=AF.Exp,
                                                 bias=nm[:, 0:1],
                                                 accum_out=ssum[:, 0:1])
                            lns = sb.tile([P, 1], fp32, tag="lns")
                            nc.scalar.activation(out=lns[:], in_=ssum[:], func=AF.Ln)
                            ls = sb.tile([P, CO], fp32, tag="ls")
                            nc.vector.tensor_scalar(
                                out=ls[:], in0=h2[:], scalar1=nm[:, 0:1],
                                scalar2=lns[:, 0:1], op0=OP.add, op1=OP.subtract)
                            nc.sync.dma_start(
                                out=ls_out.rearrange("(a p) d -> p a d", p=P)
                                    [:, nt, :],
                                in_=ls[:])



    if post_passes:
        _br.generate_event_semaphores(nc)
        _br.codegen_inst_isa_subclasses(nc)
    return nc


# --------------------------------------------------------------------------
# entry point
# --------------------------------------------------------------------------

def kernel(x, edge_index, W1l, W1r, att1, b1, W2l, W2r, att2, b2):
    x = np.asarray(x, np.float32)
    edge_index = np.asarray(edge_index)
    W1l = np.asarray(W1l, np.float32); W1r = np.asarray(W1r, np.float32)
    att1 = np.asarray(att1, np.float32); b1 = np.asarray(b1, np.float32)
    W2l = np.asarray(W2l, np.float32); W2r = np.asarray(W2r, np.float32)
    att2 = np.asarray(att2, np.float32); b2 = np.asarray(b2, np.float32)

    N, DIN = x.shape
    E = edge_index.shape[1]
    H, CH = att1.shape
    HC = W1l.shape[1]
    CO = W2l.shape[1]

    key = (N, E, DIN, H, CH, HC, CO,
           int(np.abs(b1).max() > 0), int(np.abs(b2).max() > 0),
           hash(edge_index.tobytes()))
    if key in _plan_cache:
        pp, nc, dims = _plan_cache[key]
    else:
        pp = _preprocess(N, E, edge_index)
        dims = dict(DIN=DIN, HC=HC, H=H, CH=CH, CO=CO,
                    NPC=pp["NPC"], NT=pp["NT"], NTG=pp["NTG"],
                    TBL_G=pp["TBL_G"],
                    Klo=pp["Klo"], Khi=pp["Khi"], T=pp["T"], KM=pp["KM"],
                    od=pp["od"], olo=pp["olo"], ohi=pp["ohi"],
                    OD=pp["OD"], OLO=pp["OLO"], OHI=pp["OHI"],
                    groups=pp["groups"],
                    add_b1=bool(np.abs(b1).max() > 0),
                    add_b2=bool(np.abs(b2).max() > 0))
        nc = _build_program(dims)
        _plan_cache[key] = (pp, nc, dims)

    NPC = pp["NPC"]; NT = pp["NT"]; KM = pp["KM"]
    KD = DIN // P
    bfdt = ml_dtypes.bfloat16

    # consts blob: ident | attBK | b1B | att2BK | b2B
    ident = np.eye(P, dtype=np.float32)
    attBK = np.broadcast_to(
        np.tile(att1.reshape(HC), KM)[None, :], (P, KM * HC))
    b1B = np.broadcast_to(b1.reshape(1, HC), (P, HC))
    att2BK = np.broadcast_to(
        np.tile(att2.reshape(CO), KM)[None, :], (P, KM * CO))
    b2B = np.broadcast_to(b2.reshape(1, CO), (P, CO))
    consts = np.concatenate([ident, attBK, b1B, att2BK, b2B],
                            axis=1).astype(bfdt)
    constf = np.full((P, 1), NEG_SLOPE, np.float32)
    w1cat = np.concatenate([W1l, W1r], axis=1).astype(bfdt)
    w2cat = np.concatenate([W2l, W2r], axis=1).astype(bfdt)

    in_maps = []
    for c in range(NC):
        xkc = np.zeros((NPC, DIN), np.float32)
        sel = pp["node_order"][c]
        real = sel >= 0
        xkc[real] = x[sel[real]]
        # [p, nt, k, q] = xkc[nt*P+q, k*P+p]
        xkT = np.ascontiguousarray(
            xkc.reshape(NT, P, KD, P).transpose(3, 0, 2, 1)
        ).reshape(P, NT * KD * P).astype(bfdt)
        in_maps.append(dict(
            xkT=xkT, w1=w1cat, w2=w2cat, consts=consts, constf=constf,
            gidx_lo=np.ascontiguousarray(pp["gidx_lo"][c]),
            gidx_hi=np.ascontiguousarray(pp["gidx_hi"][c]),
            ohb=np.ascontiguousarray(pp["ohb"][c]),
        ))

    from concourse.bass_utils import run_bass_kernel_spmd
    res = run_bass_kernel_spmd(nc, in_maps, core_ids=list(range(NC)))

    h = np.empty((N, CO), np.float32)
    ls = np.empty((N, CO), np.float32)
    r_core = pp["core_of"]
    r_loc = pp["local_of"]
    for c in range(NC):
        m = r_core == c
        h[m] = res.results[c]["h2o"][r_loc[m]]
        ls[m] = res.results[c]["lso"][r_loc[m]]
    return h, ls


# revision 40
# speedup vs baseline: 2.5995x; 1.0036x over previous
"""2-layer GATv2 (PyG GATv2Conv semantics) on 8 Trainium2 NeuronCores.

Strategy (v2):
  - Nodes sharded across 8 cores; per-core greedy 2D packing balances each
    destination tile's lo/hi in-edge counts (lo = src owned by cores 0-3).
  - x is shipped pre-transposed; layer-1 projections are 6 accumulating
    matmuls per node tile with a combined [W1l|W1r] moving operand.
  - xl tables AllGathered HBM->HBM in 4 row-chunks overlapped with compute.
  - Per destination tile, incoming-edge source rows are fetched with
    dma_gather (int16 idx; two table halves).  The dst->edge one-hot
    (oh_ne, [dst, edge]) is precomputed on host and streamed from HBM;
    the edge->dst one-hot (oh_en) is one batched DVE is_equal per half.
  - Per half (K edge tiles): K z-matmuls (xr broadcast), one batched DVE
    add (z+xl), 2-op leaky, att mult, reduce, exp, msg mult -- all batched
    over the half's K*128 edges -- then K scatter matmuls accumulate
    numerator+denominator in PSUM.
  - Softmax skips max-subtraction (scores O(1)).
  - Layer 2 (heads=1, 16 ch) repeats the edge structure on a 256B-row table.
  - log_softmax on ACT/DVE per node tile.

kernel(**inputs) takes FULL inputs, returns FULL outputs.
"""

import os
import sys

if "/opt/trn_rl_repo" not in sys.path:
    sys.path.insert(0, "/opt/trn_rl_repo")

import numpy as np
import ml_dtypes

NC = 8          # cores
P = 128         # partitions
NEG_SLOPE = 0.2
NGRP = 2        # AllGather chunks

_plan_cache = {}


# --------------------------------------------------------------------------
# host-side graph preprocessing
# --------------------------------------------------------------------------

def _snake(order, nbins):
    n = len(order)
    ids = np.arange(n)
    round_ = ids // nbins
    pos = ids % nbins
    b = np.where(round_ % 2 == 0, pos, nbins - 1 - pos)
    out = np.empty(n, np.int64)
    out[:] = b
    return out


def _preprocess(N, E, edge_index):
    # Two rank-groups (= AllGather chunks = table halves), 25 tile-ranks each.
    NTG = ((N + 2 * NC - 1) // (2 * NC) + P - 1) // P      # tiles per group
    NT = 2 * NTG
    NPC = NT * P
    TBL_G = NC * NTG * P                                   # rows per half-table
    assert TBL_G < 32768, "table half must fit int16 row indices"

    src = np.concatenate([edge_index[0].astype(np.int64), np.arange(N)])
    dst = np.concatenate([edge_index[1].astype(np.int64), np.arange(N)])
    deg = np.bincount(dst, minlength=N)

    # --- group assignment (fixes each edge's table half), then cores within
    # each group balanced by OWN-group in-degree (the heavy, self-loop half)
    order = np.argsort(-deg, kind="stable")
    grp_of = np.empty(N, np.int64)
    grp_of[order] = _snake(order, 2)
    e_own = grp_of[src] == grp_of[dst]
    deg_own = np.bincount(dst[e_own], minlength=N)
    deg_oth = deg - deg_own
    core_of = np.empty(N, np.int64)
    for g in range(2):
        nodes_g = np.where(grp_of == g)[0]
        og = nodes_g[np.argsort(-deg_own[nodes_g], kind="stable")]
        core_of[og] = _snake(og, NC)

    lo_src = grp_of[src] == 0              # which table half each edge reads

    # --- per (core, group) greedy packing into NTG tiles: keep the heavy
    # (own-group) sum under 5*P and the light sum under 4*P per tile
    slot_of = np.empty(N, np.int64)
    tile_of = np.empty(N, np.int64)        # tile index within the group
    nheav = np.zeros((NC, 2, NTG), np.int64)
    nlite = np.zeros((NC, 2, NTG), np.int64)
    cnt_ct = np.zeros((NC, 2, NTG), np.int64)
    for c in range(NC):
        for g in range(2):
            nodes = np.where((core_of == c) & (grp_of == g))[0]
            nodes = nodes[np.argsort(
                -(deg_own[nodes] * 64 + deg_oth[nodes]), kind="stable")]
            hv = np.zeros(NTG, np.int64)
            lt = np.zeros(NTG, np.int64)
            cnt = np.zeros(NTG, np.int64)
            for v in nodes:
                cost = (hv + deg_own[v]).astype(np.float64) \
                    + 0.02 * (lt + deg_oth[v]) + 1e-4 * cnt \
                    + 1e6 * np.maximum(lt + deg_oth[v] - 4 * P, 0)
                cost[cnt >= P] = 1e18
                t = int(np.argmin(cost))
                tile_of[v] = t
                slot_of[v] = cnt[t]
                cnt[t] += 1
                hv[t] += deg_own[v]
                lt[t] += deg_oth[v]
            # pad slots: one fake edge each, into whichever half has headroom
            pads = P - cnt
            room_l = np.maximum(4 * P - lt, 0)
            tol = np.minimum(pads, room_l)
            lt += tol
            hv += pads - tol
            nheav[c, g] = hv
            nlite[c, g] = lt
            cnt_ct[c, g] = cnt

    # --- per (core, group) rank permutation to align heavy tiles
    kh_ = (nheav + P - 1) // P
    kl_ = (nlite + P - 1) // P
    perm = np.empty((NC, 2, NTG), np.int64)
    for c in range(NC):
        for g in range(2):
            key = (kh_[c, g] + kl_[c, g]) + 1e-3 * kh_[c, g] \
                + 1e-9 * (nheav[c, g] + nlite[c, g])
            perm[c, g] = np.argsort(-key, kind="stable")

    # rank r in [0, NTG) -> group 0, [NTG, NT) -> group 1
    # group 0 tiles: heavy half = lo;  group 1 tiles: heavy half = hi
    Klo = []
    Khi = []
    for r in range(NT):
        g, rr = (0, r) if r < NTG else (1, r - NTG)
        kh = int(max(kh_[c, g, perm[c, g, rr]] for c in range(NC)))
        kl = int(max(kl_[c, g, perm[c, g, rr]] for c in range(NC)))
        if g == 0:
            Klo.append(kh); Khi.append(kl)
        else:
            Klo.append(kl); Khi.append(kh)
    T = [Klo[r] + Khi[r] for r in range(NT)]
    KM = max(max(Klo), max(Khi))

    rank_of = np.empty((NC, 2, NTG), np.int64)
    for c in range(NC):
        for g in range(2):
            rank_of[c, g, perm[c, g]] = np.arange(NTG)
    rank_glob = rank_of[core_of, grp_of, tile_of] + grp_of * NTG
    local_of = rank_glob * P + slot_of
    # table row within the node's half-table: [core][rank-in-group][slot]
    row_half = core_of * NTG * P + rank_of[core_of, grp_of, tile_of] * P + slot_of

    # --- per (core, group, tile) edge lists split by half
    e_core = core_of[dst]
    e_grp = grp_of[dst]
    e_tile = tile_of[dst]
    e_slot = slot_of[dst]
    lists_lo = {}
    lists_hi = {}
    for c in range(NC):
        for g in range(2):
            m_cg = (e_core == c) & (e_grp == g)
            for tl in range(NTG):
                m = m_cg & (e_tile == tl)
                ml = m & lo_src
                mh = m & ~lo_src
                lists_lo[(c, g, tl)] = (row_half[src[ml]], e_slot[ml])
                lists_hi[(c, g, tl)] = (row_half[src[mh]], e_slot[mh])
                # fake self-edges for empty (padding) node slots; split them
                # to match the greedy's half accounting (light first)
                npad = P - cnt_ct[c, g, tl]
                if npad:
                    pads = np.arange(P - npad, P)
                    lite_is_lo = g == 1
                    n_l = int(min(npad, max(
                        4 * P - len(lists_lo[(c, g, tl)][0]) if lite_is_lo
                        else 4 * P - len(lists_hi[(c, g, tl)][0]), 0)))
                    for tgt, idxs in ((lists_lo if lite_is_lo else lists_hi,
                                       pads[:n_l]),
                                      (lists_hi if lite_is_lo else lists_lo,
                                       pads[n_l:])):
                        if len(idxs):
                            a, b = tgt[(c, g, tl)]
                            tgt[(c, g, tl)] = (
                                np.concatenate([a, np.zeros(len(idxs),
                                                            np.int64)]),
                                np.concatenate([b, idxs]))

    # offsets
    od = np.concatenate([[0], np.cumsum(T)]).astype(np.int64)
    olo = np.concatenate([[0], np.cumsum(Klo)]).astype(np.int64)
    ohi = np.concatenate([[0], np.cumsum(Khi)]).astype(np.int64)
    OD = int(od[-1]); OLO = int(olo[-1]); OHI = int(ohi[-1])

    def pack_idx(flat):
        n = len(flat)
        s = (n + 15) // 16
        arr = np.zeros(s * 16, np.int16)
        arr[:n] = flat
        block = arr.reshape(s, 16).T
        return np.tile(block, (8, 1))

    gidx_lo = np.zeros((NC, P, OLO * 8), np.int16)
    gidx_hi = np.zeros((NC, P, OHI * 8), np.int16)
    drel = np.full((NC, P, OD), -1.0, np.float32)
    for c in range(NC):
        for r in range(NT):
            g, rr = (0, r) if r < NTG else (1, r - NTG)
            tl = perm[c, g, rr]
            for K, off8, dcol0, lst, gax in [
                    (Klo[r], olo[r], od[r], lists_lo[(c, g, tl)], gidx_lo),
                    (Khi[r], ohi[r], od[r] + Klo[r], lists_hi[(c, g, tl)],
                     gidx_hi)]:
                rows, slots = lst
                n = len(rows)
                assert n <= K * P
                flat = np.zeros(K * P, np.int64)
                flat[:n] = rows
                gax[c, :, off8 * 8:(off8 + K) * 8] = pack_idx(flat)
                dr = np.full(K * P, -1.0, np.float32)
                dr[:n] = slots
                drel[c, :, dcol0:dcol0 + K] = dr.reshape(K, P).T

    node_order = np.full((NC, NPC), -1, np.int64)
    for c in range(NC):
        nodes = np.where(core_of == c)[0]
        node_order[c, local_of[nodes]] = nodes

    # per tile col: [ohne (dst-major [d, e]) | ohen (edge-major [e, d])]
    ar = np.arange(P, dtype=np.float32)
    ohne = (ar[None, :, None, None]
            == drel.transpose(0, 2, 1)[:, None, :, :])
    ohen = (drel[:, :, :, None] == ar[None, None, None, :])
    ohb = np.empty((NC, P, 2 * OD * P), ml_dtypes.bfloat16)
    for r in range(NT):
        o0, o1 = int(od[r]), int(od[r + 1])
        t_ = o1 - o0
        ohb[:, :, 2 * o0 * P:(2 * o0 + t_) * P] = \
            ohne[:, :, o0:o1, :].reshape(NC, P, t_ * P)
        ohb[:, :, (2 * o0 + t_) * P:2 * o1 * P] = \
            ohen[:, :, o0:o1, :].reshape(NC, P, t_ * P)

    groups = [(0, NTG), (NTG, NT)]

    return dict(NPC=NPC, NT=NT, NTG=NTG, TBL_G=TBL_G,
                Klo=Klo, Khi=Khi, T=T, KM=KM,
                od=od.tolist(), olo=olo.tolist(), ohi=ohi.tolist(),
                OD=OD, OLO=OLO, OHI=OHI, groups=groups,
                gidx_lo=gidx_lo, gidx_hi=gidx_hi, ohb=ohb,
                node_order=node_order, core_of=core_of, local_of=local_of)


# --------------------------------------------------------------------------
# bass program
# --------------------------------------------------------------------------

def _build_program(dims, post_passes=True):
    PHASES = int(os.environ.get("GAT_PHASES", "3"))
    SHARED = os.environ.get("GAT_SHARED", "1") == "1"
    GB = int(os.environ.get("GAT_GB", "1"))        # gather batch (node tiles)
    import concourse.bass as bass
    import concourse.mybir as mybir
    import concourse.tile as tile
    from concourse import library_config
    from concourse.bass import _add_dep_helper
    import bass_rust as _br

    fp32 = mybir.dt.float32
    bf = mybir.dt.bfloat16
    i16 = mybir.dt.int16
    AX = mybir.AxisListType
    OP = mybir.AluOpType
    AF = mybir.ActivationFunctionType

    DIN = dims["DIN"]; HC = dims["HC"]; H = dims["H"]; CH = dims["CH"]
    CO = dims["CO"]
    NPC = dims["NPC"]; NT = dims["NT"]; NTG = dims["NTG"]
    TBL_G = dims["TBL_G"]
    Klo = dims["Klo"]; Khi = dims["Khi"]; T = dims["T"]; KM = dims["KM"]
    od = dims["od"]; olo = dims["olo"]; ohi = dims["ohi"]
    OD = dims["OD"]; OLO = dims["OLO"]; OHI = dims["OHI"]
    groups = dims["groups"]
    KD = DIN // P
    KH = HC // P
    CO_PAD = 128
    TM = max(T)
    addr_space = "Shared" if SHARED else "Local"

    # gather batches: consecutive ranks within each AG group, <= GB tiles
    batches = []        # (nt0, nt1)
    for g0, g1 in groups:
        nt = g0
        while nt < g1:
            batches.append((nt, min(nt + GB, g1)))
            nt = batches[-1][1]
    BKM = max(max(olo[b1] - olo[b0], ohi[b1] - ohi[b0]) for b0, b1 in batches)

    nc = bass.Bass(num_devices=NC, num_swdge_queues=4,
                   dynamic_dma_scratch_size=int(os.environ.get("GAT_DDS", "16384")))

    xkT_d = nc.dram_tensor("xkT", [P, NT * KD * P], bf, kind="ExternalInput")
    w1_d = nc.dram_tensor("w1", [DIN, 2 * HC], bf, kind="ExternalInput")
    w2_d = nc.dram_tensor("w2", [HC, 2 * CO], bf, kind="ExternalInput")
    CCOLS = KM * HC + HC + KM * CO + CO + P
    consts = nc.dram_tensor("consts", [P, CCOLS], bf, kind="ExternalInput")
    constf = nc.dram_tensor("constf", [P, 1], fp32, kind="ExternalInput")
    gidx_lo_d = nc.dram_tensor("gidx_lo", [P, OLO * 8], i16, kind="ExternalInput")
    gidx_hi_d = nc.dram_tensor("gidx_hi", [P, OHI * 8], i16, kind="ExternalInput")
    ohb_d = nc.dram_tensor("ohb", [P, 2 * OD * P], bf, kind="ExternalInput")
    h2_out = nc.dram_tensor("h2o", [NPC, CO], fp32, kind="ExternalOutput")
    ls_out = nc.dram_tensor("lso", [NPC, CO], fp32, kind="ExternalOutput")

    with tile.TileContext(nc) as tc:
        with (
            tc.tile_pool(name="dram", bufs=1, space="DRAM") as dram,
            tc.tile_pool(name="cst", bufs=1) as cst,
        ):
            lib = nc.gpsimd.load_library(library_config.mlp)
            regs = {}
            for b0, b1 in batches:
                for n in (olo[b1] - olo[b0], ohi[b1] - ohi[b0]):
                    if n not in regs:
                        regs[n] = nc.gpsimd.to_reg(n * P)

            ctile = cst.tile([P, CCOLS], bf)
            nc.sync.dma_start(out=ctile[:], in_=consts[:])
            cftile = cst.tile([P, 1], fp32)
            nc.sync.dma_start(out=cftile[:], in_=constf[:])
            o = 0
            ident = ctile[:, o:o + P]; o += P
            attBK = ctile[:, o:o + KM * HC]; o += KM * HC
            b1B = ctile[:, o:o + HC]; o += HC
            att2BK = ctile[:, o:o + KM * CO]; o += KM * CO
            b2B = ctile[:, o:o + CO]; o += CO
            alpha = cftile[:, 0:1]

            w1_sb = cst.tile([P, KD, 2 * HC], bf)
            nc.sync.dma_start(out=w1_sb[:], in_=w1_d.rearrange("(k p) c -> p k c", p=P))
            w2_sb = cst.tile([P, KH, 2 * CO], bf)
            nc.sync.dma_start(out=w2_sb[:], in_=w2_d.rearrange("(k p) c -> p k c", p=P))

            gidx_lo_sb = cst.tile([P, OLO * 8], i16)
            nc.sync.dma_start(out=gidx_lo_sb[:], in_=gidx_lo_d[:])
            gidx_hi_sb = cst.tile([P, OHI * 8], i16)
            nc.sync.dma_start(out=gidx_hi_sb[:], in_=gidx_hi_d[:])

            xr1_all = cst.tile([P, NT, HC], bf)
            xr2_all = cst.tile([P, NT, CO], bf)
            nc.vector.memset(xr2_all[:], 0.0)

            tbl1 = {}
            tbl2 = {}
            ag1_in = {}
            ag2_in = {}
            for gi, (g0, g1) in enumerate(groups):
                rows = (g1 - g0) * P
                ag1_in[gi] = dram.tile([rows, HC], bf, name=f"ag1i_{gi}")
                ag2_in[gi] = dram.tile([rows, CO_PAD], bf, name=f"ag2i_{gi}")
                tbl1[gi] = dram.tile([NC * rows, HC], bf,
                                     addr_space=addr_space, name=f"tbl1_{gi}")
                tbl2[gi] = dram.tile([NC * rows, CO_PAD], bf,
                                     addr_space=addr_space, name=f"tbl2_{gi}")

            def ag_chunk(src, dst):
                nc.gpsimd.collective_compute(
                    "AllGather", mybir.AluOpType.bypass,
                    replica_groups=[list(range(NC))],
                    ins=[src[:].opt()],
                    outs=[dst[:].opt()],
                )

            # ============ phase A: layer-1 projections ============
            with (tc.tile_pool(name="sbA", bufs=3) as sb,
                  tc.tile_pool(name="psA", bufs=2, space="PSUM") as ps):
                XB = 4
                for gi, (g0, g1) in enumerate(groups):
                    for nb in range(g0, g1, XB):
                        ne = min(nb + XB, g1)
                        xt = sb.tile([P, XB, KD, P], bf, tag="xt")
                        nc.sync.dma_start(
                            out=xt[:, 0:ne - nb, :, :],
                            in_=xkT_d[:, nb * KD * P:ne * KD * P])
                        for nt in range(nb, ne):
                            xlr_ps = ps.tile([P, 2 * HC], fp32, tag="mm",
                                             space="PSUM")
                            for k in range(KD):
                                nc.tensor.matmul(out=xlr_ps[:],
                                                 lhsT=xt[:, nt - nb, k, :],
                                                 rhs=w1_sb[:, k, :],
                                                 start=(k == 0),
                                                 stop=(k == KD - 1))
                            xl_sb = sb.tile([P, HC], bf, tag="xls")
                            nc.vector.tensor_copy(out=xl_sb[:],
                                                  in_=xlr_ps[:, 0:HC])
                            nc.vector.tensor_copy(out=xr1_all[:, nt, :],
                                                  in_=xlr_ps[:, HC:2 * HC])
                            nc.sync.dma_start(
                                out=ag1_in[gi][(nt - g0) * P:
                                               (nt - g0 + 1) * P, :],
                                in_=xl_sb[:])
                    if PHASES >= 2:
                        ag_chunk(ag1_in[gi], tbl1[gi])

            # ============ phase B: layer-1 edges ============
            if PHASES >= 2:
                grp_of_nt = {}
                for gi, (g0, g1) in enumerate(groups):
                    for nt in range(g0, g1):
                        grp_of_nt[nt] = gi
                with (tc.tile_pool(name="sbB", bufs=2) as sb,
                      tc.tile_pool(name="gbB", bufs=3) as gb,
                      tc.tile_pool(name="psB", bufs=2, space="PSUM") as ps):
                    PF = 6

                    def issue_lo1(i):
                        b0, b1 = batches[i]
                        nlo_b = olo[b1] - olo[b0]
                        glo = gb.tile([P, BKM, HC], bf, tag="glo",
                                      bufs=PF + 2)
                        gi_ = nc.gpsimd.dma_gather(
                            glo[:, 0:nlo_b, :], tbl1[0][:],
                            gidx_lo_sb[:, olo[b0] * 8:olo[b1] * 8],
                            nlo_b * P, regs[nlo_b], HC,
                            queue_num=0)
                        _add_dep_helper(gi_.ins, lib.ins, sync=False,
                                        reason="lib")
                        return glo

                    lo_pend = {}
                    for i in range(min(PF, len(batches))):
                        lo_pend[i] = issue_lo1(i)
                    for bi, (b0, b1) in enumerate(batches):
                        nlo_b = olo[b1] - olo[b0]
                        nhi_b = ohi[b1] - ohi[b0]
                        glo = lo_pend.pop(bi)
                        ghi = gb.tile([P, BKM, HC], bf, tag="ghi")
                        g2i = nc.gpsimd.dma_gather(
                            ghi[:, 0:nhi_b, :], tbl1[1][:],
                            gidx_hi_sb[:, ohi[b0] * 8:ohi[b1] * 8],
                            nhi_b * P, regs[nhi_b], HC,
                            queue_num=0)
                        _add_dep_helper(g2i.ins, lib.ins, sync=False,
                                        reason="lib")
                        if bi + PF < len(batches):
                            lo_pend[bi + PF] = issue_lo1(bi + PF)

                        for nt in range(b0, b1):
                            T_ = T[nt]; Klo_ = Klo[nt]; Khi_ = Khi[nt]
                            blo = olo[nt] - olo[b0]
                            bhi = ohi[nt] - ohi[b0]
                            ohb_t = sb.tile([P, 2 * TM * P], bf, tag="ohb")
                            nc.sync.dma_start(
                                out=ohb_t[:, 0:2 * T_ * P],
                                in_=ohb_d[:, 2 * od[nt] * P:2 * (od[nt] + T_) * P])
                            ohne = ohb_t[:, 0:T_ * P]
                            ohen = ohb_t[:, T_ * P:2 * T_ * P]

                            acc = ps.tile([P, HC + H], fp32, tag="acc",
                                          space="PSUM", bufs=2)
                            msg = sb.tile([P, TM, HC + H], bf, tag="msg")
                            for t0, K, gx, gb0 in [(0, Klo_, glo, blo),
                                                   (Klo_, Khi_, ghi, bhi)]:
                                t_sb = sb.tile([P, KM, HC], bf, tag="t")
                                for j in range(K):
                                    zj = ps.tile([P, HC], fp32, tag="z",
                                                 space="PSUM", bufs=4)
                                    nc.tensor.matmul(
                                        out=zj[:],
                                        lhsT=ohne[:, (t0 + j) * P:(t0 + j + 1) * P],
                                        rhs=xr1_all[:, nt, :],
                                        start=True, stop=False)
                                    nc.tensor.matmul(
                                        out=zj[:], lhsT=ident,
                                        rhs=gx[:, gb0 + j, :],
                                        start=False, stop=True)
                                    nc.scalar.activation(out=t_sb[:, j, :],
                                                         in_=zj[:],
                                                         func=AF.Prelu, alpha=alpha)
                                ta = sb.tile([P, KM, HC], bf, tag="ta")
                                nc.vector.tensor_tensor(
                                    out=ta[:, 0:K, :], in0=t_sb[:, 0:K, :],
                                    in1=attBK[:, 0:K * HC].rearrange(
                                        "p (k c) -> p k c", k=K), op=OP.mult)
                                sc = sb.tile([P, KM * H], fp32, tag="sc")
                                nc.vector.tensor_reduce(
                                    out=sc[:, 0:K * H],
                                    in_=ta[:, 0:K, :].rearrange(
                                        "p k (h c) -> p (k h) c", h=H),
                                    axis=AX.X, op=OP.add)
                                nc.scalar.activation(
                                    out=msg[:, t0:t0 + K, HC:HC + H],
                                    in_=sc[:, 0:K * H], func=AF.Exp)
                                nc.vector.tensor_tensor(
                                    out=msg[:, t0:t0 + K, 0:HC].rearrange(
                                        "p k (h c) -> p k h c", h=H),
                                    in0=gx[:, gb0:gb0 + K, :].rearrange(
                                        "p k (h c) -> p k h c", h=H),
                                    in1=msg[:, t0:t0 + K, HC:HC + H]
                                        [:, :, :, None]
                                        .to_broadcast([P, K, H, CH]),
                                    op=OP.mult)
                                for j in range(K):
                                    nc.tensor.matmul(
                                        out=acc[:],
                                        lhsT=ohen[:, (t0 + j) * P:(t0 + j + 1) * P],
                                        rhs=msg[:, t0 + j, :],
                                        start=(t0 + j == 0), stop=(t0 + j == T_ - 1))

                            rec = sb.tile([P, H], fp32, tag="rec")
                            nc.vector.reciprocal(out=rec[:], in_=acc[:, HC:HC + H])
                            h1 = sb.tile([P, HC], fp32, tag="h1")
                            nc.vector.tensor_tensor(
                                out=h1[:].rearrange("p (h c) -> p h c", h=H),
                                in0=acc[:, 0:HC].rearrange("p (h c) -> p h c", h=H),
                                in1=rec[:, :, None].to_broadcast([P, H, CH]),
                                op=OP.mult)
                            if dims["add_b1"]:
                                nc.vector.tensor_tensor(out=h1[:], in0=h1[:],
                                                        in1=b1B, op=OP.add)
                            eh = sb.tile([P, HC], fp32, tag="eh")
                            nc.scalar.activation(out=eh[:], in_=h1[:], func=AF.Exp)
                            em = sb.tile([P, HC], fp32, tag="em")
                            nc.vector.tensor_scalar(
                                out=em[:], in0=eh[:], scalar1=1.0, scalar2=0.0,
                                op0=OP.subtract, op1=OP.min)
                            elu = sb.tile([P, HC], bf, tag="elu")
                            nc.vector.tensor_scalar(out=elu[:], in0=h1[:],
                                                    scalar1=0.0, scalar2=None,
                                                    op0=OP.max)
                            nc.vector.tensor_tensor(out=elu[:], in0=elu[:],
                                                    in1=em[:], op=OP.add)

                            tail_ps = ps.tile([P, KH * P + 2 * CO], fp32,
                                              tag="tail", space="PSUM", bufs=2)
                            for k in range(KH):
                                nc.tensor.matmul(
                                    out=tail_ps[:, k * P:(k + 1) * P],
                                    lhsT=elu[:, k * P:(k + 1) * P],
                                    rhs=ident, start=True, stop=True)
                            hT_sb = sb.tile([P, KH, P], bf, tag="hTs")
                            nc.vector.tensor_copy(
                                out=hT_sb[:],
                                in_=tail_ps[:, 0:KH * P].rearrange(
                                    "p (k q) -> p k q", k=KH))
                            x2_ps = tail_ps[:, KH * P:KH * P + 2 * CO]
                            for k in range(KH):
                                nc.tensor.matmul(out=x2_ps, lhsT=hT_sb[:, k, :],
                                                 rhs=w2_sb[:, k, :],
                                                 start=(k == 0), stop=(k == KH - 1))
                            xl2_sb = sb.tile([P, CO], bf, tag="xl2s")
                            nc.vector.tensor_copy(out=xl2_sb[:], in_=x2_ps[:, 0:CO])
                            nc.vector.tensor_copy(out=xr2_all[:, nt, :],
                                                  in_=x2_ps[:, CO:2 * CO])
                            gi = grp_of_nt[nt]
                            g0_, _ = groups[gi]
                            nc.sync.dma_start(
                                out=ag2_in[gi][(nt - g0_) * P:(nt - g0_ + 1) * P,
                                               0:CO],
                                in_=xl2_sb[:])
                        if PHASES >= 3 and b1 == groups[grp_of_nt[b0]][1]:
                            gi = grp_of_nt[b0]
                            g0_, g1_ = groups[gi]
                            ag_chunk(ag2_in[gi], tbl2[gi])

            # ============ phase C: layer-2 edges ============
            if PHASES >= 3:
                with (tc.tile_pool(name="sbC", bufs=2) as sb,
                      tc.tile_pool(name="gbC", bufs=3) as gb,
                      tc.tile_pool(name="psC", bufs=2, space="PSUM") as ps):
                    PF = 6

                    def issue_lo2(i):
                        b0, b1 = batches[i]
                        nlo_b = olo[b1] - olo[b0]
                        g2lo = gb.tile([P, BKM, CO_PAD], bf, tag="g2lo",
                                       bufs=PF + 2)
                        gi_ = nc.gpsimd.dma_gather(
                            g2lo[:, 0:nlo_b, :], tbl2[0][:],
                            gidx_lo_sb[:, olo[b0] * 8:olo[b1] * 8],
                            nlo_b * P, regs[nlo_b], CO_PAD,
                            queue_num=0)
                        _add_dep_helper(gi_.ins, lib.ins, sync=False,
                                        reason="lib")
                        return g2lo

                    lo_pend = {}
                    for i in range(min(PF, len(batches))):
                        lo_pend[i] = issue_lo2(i)
                    for bi, (b0, b1) in enumerate(batches):
                        nlo_b = olo[b1] - olo[b0]
                        nhi_b = ohi[b1] - ohi[b0]
                        g2lo = lo_pend.pop(bi)
                        g2hi = gb.tile([P, BKM, CO_PAD], bf, tag="g2hi")
                        g2i = nc.gpsimd.dma_gather(
                            g2hi[:, 0:nhi_b, :], tbl2[1][:],
                            gidx_hi_sb[:, ohi[b0] * 8:ohi[b1] * 8],
                            nhi_b * P, regs[nhi_b], CO_PAD,
                            queue_num=0)
                        _add_dep_helper(g2i.ins, lib.ins, sync=False,
                                        reason="lib")
                        if bi + PF < len(batches):
                            lo_pend[bi + PF] = issue_lo2(bi + PF)

                        for nt in range(b0, b1):
                            T_ = T[nt]; Klo_ = Klo[nt]; Khi_ = Khi[nt]
                            blo = olo[nt] - olo[b0]
                            bhi = ohi[nt] - ohi[b0]
                            ohb_t = sb.tile([P, 2 * TM * P], bf, tag="ohb2")
                            nc.sync.dma_start(
                                out=ohb_t[:, 0:2 * T_ * P],
                                in_=ohb_d[:, 2 * od[nt] * P:2 * (od[nt] + T_) * P])
                            ohne = ohb_t[:, 0:T_ * P]
                            ohen = ohb_t[:, T_ * P:2 * T_ * P]

                            acc2 = ps.tile([P, CO + 1], fp32, tag="acc2",
                                           space="PSUM", bufs=2)
                            msg2 = sb.tile([P, TM, CO + 1], bf, tag="msg2")
                            for t0, K, gx, gb0 in [(0, Klo_, g2lo, blo),
                                                   (Klo_, Khi_, g2hi, bhi)]:
                                t2 = sb.tile([P, KM, CO], bf, tag="t2")
                                for j in range(K):
                                    zj = ps.tile([P, CO], fp32, tag="z2",
                                                 space="PSUM", bufs=4)
                                    nc.tensor.matmul(
                                        out=zj[:],
                                        lhsT=ohne[:, (t0 + j) * P:(t0 + j + 1) * P],
                                        rhs=xr2_all[:, nt, :],
                                        start=True, stop=False)
                                    nc.tensor.matmul(
                                        out=zj[:], lhsT=ident,
                                        rhs=gx[:, gb0 + j, 0:CO],
                                        start=False, stop=True)
                                    nc.scalar.activation(out=t2[:, j, :],
                                                         in_=zj[:],
                                                         func=AF.Prelu, alpha=alpha)
                                ta2 = sb.tile([P, KM, CO], bf, tag="ta2")
                                nc.vector.tensor_tensor(
                                    out=ta2[:, 0:K, :], in0=t2[:, 0:K, :],
                                    in1=att2BK[:, 0:K * CO].rearrange(
                                        "p (k c) -> p k c", k=K), op=OP.mult)
                                sc2 = sb.tile([P, KM], fp32, tag="sc2")
                                nc.vector.tensor_reduce(
                                    out=sc2[:, 0:K], in_=ta2[:, 0:K, :],
                                    axis=AX.X, op=OP.add)
                                nc.scalar.activation(
                                    out=msg2[:, t0:t0 + K, CO:CO + 1],
                                    in_=sc2[:, 0:K], func=AF.Exp)
                                nc.vector.tensor_tensor(
                                    out=msg2[:, t0:t0 + K, 0:CO],
                                    in0=gx[:, gb0:gb0 + K, 0:CO],
                                    in1=msg2[:, t0:t0 + K, CO:CO + 1]
                                        .to_broadcast([P, K, CO]),
                                    op=OP.mult)
                                for j in range(K):
                                    nc.tensor.matmul(
                                        out=acc2[:],
                                        lhsT=ohen[:, (t0 + j) * P:(t0 + j + 1) * P],
                                        rhs=msg2[:, t0 + j, :],
                                        start=(t0 + j == 0), stop=(t0 + j == T_ - 1))

                            rec2 = sb.tile([P, 1], fp32, tag="rec2")
                            nc.vector.reciprocal(out=rec2[:], in_=acc2[:, CO:CO + 1])
                            h2 = sb.tile([P, CO], fp32, tag="h2")
                            nc.vector.tensor_scalar(out=h2[:], in0=acc2[:, 0:CO],
                                                    scalar1=rec2[:, 0:1],
                                                    scalar2=None, op0=OP.mult)
                            if dims["add_b2"]:
                                nc.vector.tensor_tensor(out=h2[:], in0=h2[:],
                                                        in1=b2B, op=OP.add)
                            nc.sync.dma_start(
                                out=h2_out.rearrange("(a p) d -> p a d", p=P)
                                    [:, nt, :],
                                in_=h2[:])
                            nm = sb.tile([P, 1], fp32, tag="nm")
                            nc.vector.tensor_reduce(out=nm[:], in_=h2[:], axis=AX.X,
                                                    op=OP.max, negate=True)
                            esc = sb.tile([P, CO], fp32, tag="esc")
                            ssum = sb.tile([P, 1], fp32, tag="ssum")
                            nc.scalar.activation(out=esc[:], in_=h2[:], func=AF.Exp,
                                                 bias=nm[:, 0:1],
                                                 accum_out=ssum[:, 0:1])
                            lns = sb.tile([P, 1], fp32, tag="lns")
                            nc.scalar.activation(out=lns[:], in_=ssum[:], func=AF.Ln)
                            ls = sb.tile([P, CO], fp32, tag="ls")
                            nc.vector.tensor_scalar(
                                out=ls[:], in0=h2[:], scalar1=nm[:, 0:1],
                                scalar2=lns[:, 0:1], op0=OP.add, op1=OP.subtract)
                            nc.sync.dma_start(
                                out=ls_out.rearrange("(a p) d -> p a d", p=P)
                                    [:, nt, :],
                                in_=ls[:])



    if post_passes:
        _br.generate_event_semaphores(nc)
        _br.codegen_inst_isa_subclasses(nc)
    return nc


# --------------------------------------------------------------------------
# entry point
# --------------------------------------------------------------------------

def kernel(x, edge_index, W1l, W1r, att1, b1, W2l, W2r, att2, b2):
    x = np.asarray(x, np.float32)
    edge_index = np.asarray(edge_index)
    W1l = np.asarray(W1l, np.float32); W1r = np.asarray(W1r, np.float32)
    att1 = np.asarray(att1, np.float32); b1 = np.asarray(b1, np.float32)
    W2l = np.asarray(W2l, np.float32); W2r = np.asarray(W2r, np.float32)
    att2 = np.asarray(att2, np.float32); b2 = np.asarray(b2, np.float32)

    N, DIN = x.shape
    E = edge_index.shape[1]
    H, CH = att1.shape
    HC = W1l.shape[1]
    CO = W2l.shape[1]

    key = (N, E, DIN, H, CH, HC, CO,
           int(np.abs(b1).max() > 0), int(np.abs(b2).max() > 0),
           hash(edge_index.tobytes()))
    if key in _plan_cache:
        pp, nc, dims = _plan_cache[key]
    else:
        pp = _preprocess(N, E, edge_index)
        dims = dict(DIN=DIN, HC=HC, H=H, CH=CH, CO=CO,
                    NPC=pp["NPC"], NT=pp["NT"], NTG=pp["NTG"],
                    TBL_G=pp["TBL_G"],
                    Klo=pp["Klo"], Khi=pp["Khi"], T=pp["T"], KM=pp["KM"],
                    od=pp["od"], olo=pp["olo"], ohi=pp["ohi"],
                    OD=pp["OD"], OLO=pp["OLO"], OHI=pp["OHI"],
                    groups=pp["groups"],
                    add_b1=bool(np.abs(b1).max() > 0),
                    add_b2=bool(np.abs(b2).max() > 0))
        nc = _build_program(dims)
        _plan_cache[key] = (pp, nc, dims)

    NPC = pp["NPC"]; NT = pp["NT"]; KM = pp["KM"]
    KD = DIN // P
    bfdt = ml_dtypes.bfloat16

    # consts blob: ident | attBK | b1B | att2BK | b2B
    ident = np.eye(P, dtype=np.float32)
    attBK = np.broadcast_to(
        np.tile(att1.reshape(HC), KM)[None, :], (P, KM * HC))
    b1B = np.broadcast_to(b1.reshape(1, HC), (P, HC))
    att2BK = np.broadcast_to(
        np.tile(att2.reshape(CO), KM)[None, :], (P, KM * CO))
    b2B = np.broadcast_to(b2.reshape(1, CO), (P, CO))
    consts = np.concatenate([ident, attBK, b1B, att2BK, b2B],
                            axis=1).astype(bfdt)
    constf = np.full((P, 1), NEG_SLOPE, np.float32)
    w1cat = np.concatenate([W1l, W1r], axis=1).astype(bfdt)
    w2cat = np.concatenate([W2l, W2r], axis=1).astype(bfdt)

    in_maps = []
    for c in range(NC):
        xkc = np.zeros((NPC, DIN), np.float32)
        sel = pp["node_order"][c]
        real = sel >= 0
        xkc[real] = x[sel[real]]
        # [p, nt, k, q] = xkc[nt*P+q, k*P+p]
        xkT = np.ascontiguousarray(
            xkc.reshape(NT, P, KD, P).transpose(3, 0, 2, 1)
        ).reshape(P, NT * KD * P).astype(bfdt)
        in_maps.append(dict(
            xkT=xkT, w1=w1cat, w2=w2cat, consts=consts, constf=constf,
            gidx_lo=np.ascontiguousarray(pp["gidx_lo"][c]),
            gidx_hi=np.ascontiguousarray(pp["gidx_hi"][c]),
            ohb=np.ascontiguousarray(pp["ohb"][c]),
        ))

    from concourse.bass_utils import run_bass_kernel_spmd
    res = run_bass_kernel_spmd(nc, in_maps, core_ids=list(range(NC)))

    h = np.empty((N, CO), np.float32)
    ls = np.empty((N, CO), np.float32)
    r_core = pp["core_of"]
    r_loc = pp["local_of"]
    for c in range(NC):
        m = r_core == c
        h[m] = res.results[c]["h2o"][r_loc[m]]
        ls[m] = res.results[c]["lso"][r_loc[m]]
    return h, ls
